# revision 1
# baseline (speedup 1.0000x reference)
"""Trainium2 Bass kernel for nn_GAT_GraphSAGE (N=12000, E=192000, F=35, B=64).

Sharding: the attention "row" dimension (K_new index i, which is also the
softmax row) is sharded 1500 rows/core across 8 cores.  Q and V are computed
replicated (cheap [N,35] projections), so the only collective is one
AllGather of the post-attention node features h.  SAGEConv is sharded by dst
node (same 1500-row shard): h[src] rows are fetched with one batched
dma_gather and scatter-added via one-hot matmuls in PSUM.  Global max-pool +
MLP head run per-core on that core's 8 graphs (graph boundaries align with
the 1500-row shard exactly).

The whole K-branch (Wk/conv-center-taps/Wl/1-sqrt(F)) folds on the host into
a single affine [35->35] map; biases ride an appended ones-row of x^T.  The
attention matmuls run in FP32R (the PE's full-rate fp32 mode, ~10-11
mantissa bits; inputs are pre-rounded on the host so HW rounding is a
no-op), giving ~6e-4 relative output error vs the fp32 reference.
"""
import math
import numpy as np

N, E, F, B = 12000, 192000, 35, 64
NCORE = 8
ROWS = N // NCORE            # 1500
ICH = 512
NI = 3
IPAD = ICH * NI              # 1536
JT = 94                      # j chunks of 128
JPAD = JT * 128              # 12032
XW = 12064                   # padded x^T width (covers 7*1500 + 1536)
DBLK = 12                    # dst blocks (128 each) per core
GB = B // NCORE              # 8 graphs per core
HPAD = 64                    # h row padded to 64 f32 (256B) for dma_gather
GRAPH_BOUNDS = [int(math.ceil(g * (N / B))) for g in range(GB + 1)]
F1 = F + 1
AG0 = 1024                   # rows in first AllGather piece (i-chunks 0,1)
AG1 = ROWS - AG0             # 476 rows in second piece
# h_full row layout after the two chunked AllGathers (concat per piece):
#   src (c, r): r < AG0  -> c*AG0 + r ; else NCORE*AG0 + c*AG1 + (r - AG0)


# --------------------------------------------------------------------------
# host-side preprocessing
# --------------------------------------------------------------------------

def _prep_weights(p):
    f64 = np.float64
    f32 = np.float32
    Wq, bq = p['Wq'].astype(f64), p['bq'].astype(f64)
    Wk, bk = p['Wk'].astype(f64), p['bk'].astype(f64)
    Wv, bv = p['Wv'].astype(f64), p['bv'].astype(f64)
    W3c, b3 = p['W3'][:, :, 1].astype(f64), p['b3'].astype(f64)
    W5c, b5 = p['W5'][:, :, 2].astype(f64), p['b5'].astype(f64)
    Wl, bl = p['Wl'].astype(f64), p['bl'].astype(f64)
    Wl1, Wl2, Wl3 = Wl[:, :F], Wl[:, F:2 * F], Wl[:, 2 * F:]

    Weff = W3c.T @ Wl1.T + W5c.T @ Wl2.T + Wl3.T
    beff = b3 @ Wl1.T + b5 @ Wl2.T + bl
    Wkn = Wk.T @ Weff
    bkn = bk @ Weff + beff
    s = 1.0 / np.sqrt(F)
    Wkn, bkn = Wkn * s, bkn * s

    out = {}
    out['Wq_h'] = np.vstack([Wq.T, bq[None, :]]).astype(f32)
    out['Wkn_h'] = np.vstack([Wkn, bkn[None, :]]).astype(f32)
    out['Wv_h'] = np.vstack([Wv.T, bv[None, :]]).astype(f32)
    wva = np.zeros((F1, F1))
    wva[:, :F] = out['Wv_h']
    wva[F, F] = 1.0
    out['Wv_aug'] = wva.astype(f32)
    out['WllT'] = np.ascontiguousarray(p['Wll'].T).astype(f32)
    out['WlrT'] = np.ascontiguousarray(p['Wlr'].T).astype(f32)
    out['bll'] = p['bll'].astype(f32).reshape(F, 1)
    out['Wg1T'] = np.ascontiguousarray(p['Wg1'].T).astype(f32)      # [35,1500]
    bg1 = np.zeros((128, 12), f32)
    bg1.T.reshape(-1)[:1500] = p['bg1'].astype(f32)
    out['bg1'] = bg1
    w2 = np.zeros((12 * 128, 128), f32)
    w2[:1500, :] = p['Wg2'].T.astype(f32)
    out['Wg2Tr'] = np.ascontiguousarray(
        w2.reshape(12, 128, 128).transpose(1, 0, 2).reshape(128, 12 * 128))
    out['bg2'] = p['bg2'].astype(f32).reshape(128, 1)
    out['WoT'] = p['Wo'].astype(f32).reshape(1, 128).T.copy()        # [128,1]
    out['bo'] = float(np.asarray(p['bo']).reshape(-1)[0])
    return out


def _round_mant(a, bits=10):
    """Round fp32 to `bits` mantissa bits (pre-rounding for FP32R operands —
    the PE's full-rate fp32 mode keeps ~10-11 mantissa bits; feeding
    already-rounded values makes the HW rounding a no-op)."""
    a = np.ascontiguousarray(a, np.float32)
    u = a.view(np.uint32)
    shift = 23 - bits
    add = np.uint32(1 << (shift - 1))
    mask = np.uint32((0xFFFFFFFF << shift) & 0xFFFFFFFF)
    return ((u + add) & mask).view(np.float32)


def _prep_x(x):
    xhT = np.zeros((F1, XW), np.float32)
    xhT[:F, :N] = np.asarray(x, np.float32).T
    xhT[F, :] = 1.0
    xhT = _round_mant(xhT)
    xl = [np.ascontiguousarray(xhT[:, c * ROWS: c * ROWS + IPAD])
          for c in range(NCORE)]
    return xhT, xl


def _prep_edges(edge_index):
    src = np.asarray(edge_index[0], np.int64)
    dst = np.asarray(edge_index[1], np.int64)
    deg = np.bincount(dst, minlength=N).astype(np.float64)
    recip = (1.0 / np.maximum(deg, 1.0)).astype(np.float32)

    core_of = dst // ROWS
    blk_of = (dst - core_of * ROWS) // 128
    counts = np.zeros((NCORE, DBLK), np.int64)
    np.add.at(counts, (core_of, blk_of), 1)
    S = int(np.ceil(counts.max() / 128))

    order = np.lexsort((dst,))
    src_s, dst_s = src[order], dst[order]
    core_s, blk_s = core_of[order], blk_of[order]

    # map global src node -> its row in the piece-wise AllGathered h_full
    sc = src_s // ROWS
    sr = src_s - sc * ROWS
    src_pos = np.where(sr < AG0, sc * AG0 + sr,
                       NCORE * AG0 + sc * AG1 + (sr - AG0))

    gidx, dstrel = [], []
    for c in range(NCORE):
        idx_c = np.zeros(DBLK * S * 128, np.int16)
        rel_c = np.full(DBLK * S * 128, -1.0, np.float32)
        m_c = core_s == c
        for b in range(DBLK):
            m = m_c & (blk_s == b)
            n = int(m.sum())
            lo = b * S * 128
            idx_c[lo:lo + n] = src_pos[m].astype(np.int16)
            rel_c[lo:lo + n] = (dst_s[m] - c * ROWS - b * 128).astype(np.float32)
        # HW convention: the [16, n] packed index block must be replicated
        # across all eight 16-partition groups (sim reads only rows 0:16).
        gidx.append(np.ascontiguousarray(
            np.tile(idx_c.reshape(-1, 16).T, (8, 1))))
        dstrel.append(np.ascontiguousarray(rel_c.reshape(-1, 128).T))

    recipT = []
    for c in range(NCORE):
        r = np.ones(IPAD, np.float32)
        r[:ROWS] = recip[c * ROWS:(c + 1) * ROWS]
        recipT.append(np.ascontiguousarray(np.broadcast_to(r, (F, IPAD))))
    return gidx, dstrel, recipT, S


# --------------------------------------------------------------------------
# device program
# --------------------------------------------------------------------------

def _emit_body(nc, tc, d, S, bo_const, timeline, stop_after, sfx,
               no_collective=False):
    """Emit one full kernel body. `d` holds dram handles. `sfx` uniquifies
    pool names when the body is replicated for benchmarking."""
    import concourse.tile as tile
    from concourse import mybir

    f32 = mybir.dt.float32
    f32r = mybir.dt.float32r

    with tc.tile_pool(name="const" + sfx, bufs=1) as constp, \
         tc.tile_pool(name="main" + sfx, bufs=1) as main:
        # ---- constants / small weights ----
        Wq_t = constp.tile([F1, F], f32r, name="Wq_t" + sfx)
        nc.sync.dma_start(out=Wq_t[:], in_=d['Wq_h'][:, :])
        Wkn_t = constp.tile([F1, F], f32r, name="Wkn_t" + sfx)
        nc.sync.dma_start(out=Wkn_t[:], in_=d['Wkn_h'][:, :])
        Wv_t = constp.tile([F1, F], f32r, name="Wv_t" + sfx)
        nc.sync.dma_start(out=Wv_t[:], in_=d['Wv_h'][:, :])
        Wva_t = constp.tile([F1, F1], f32r, name="Wva_t" + sfx)
        nc.sync.dma_start(out=Wva_t[:], in_=d['Wv_aug'][:, :])
        ident_t = constp.tile([128, 128], f32, name="ident_t" + sfx)
        nc.sync.dma_start(out=ident_t[:], in_=d['ident'][:, :])

        # ---- big persistent sbuf tensors ----
        QT = main.tile([F, JPAD], f32r, name="QT" + sfx)
        KnT = main.tile([F, IPAD], f32r, name="KnT" + sfx)
        Vp = main.tile([128, JT, F1], f32r, name="Vp" + sfx)
        Vl = main.tile([128, DBLK, F], f32, name="Vl" + sfx)
        hnat = main.tile([128, DBLK, HPAD], f32, name="hnat" + sfx)

        with tc.tile_pool(name="prep" + sfx, bufs=2, space="PSUM") as pp, \
             tc.tile_pool(name="prepin" + sfx, bufs=1) as pin, \
             tc.tile_pool(name="prepsb" + sfx, bufs=3) as psb:
            xhT_t = pin.tile([F1, XW], f32r, name="xhT_t" + sfx)
            nc.sync.dma_start(out=xhT_t[:], in_=d['xhT'][:, :])
            xlT_t = pin.tile([F1, IPAD], f32r, name="xlT_t" + sfx)
            nc.sync.dma_start(out=xlT_t[:], in_=d['xlT'][:, :])

            # PSUM->SBUF copies: keep on DVE (ACT is the attention-critical
            # engine; prep copies overlap into the attention phase anyway).
            def cp(out, in_, alt):
                nc.vector.tensor_copy(out=out, in_=in_)

            # QT full over 512-chunks (covers JPAD=12032)
            for ci in range((JPAD + ICH - 1) // ICH):
                w = min(ICH, JPAD - ci * ICH)
                ps = pp.tile([F, ICH], f32, space="PSUM", tag="ppq",
                             name="psq" + sfx)
                nc.tensor.matmul(out=ps[:, :w], lhsT=Wq_t[:],
                                 rhs=xhT_t[:, ci * ICH: ci * ICH + w],
                                 start=True, stop=True)
                cp(QT[:, ci * ICH: ci * ICH + w], ps[:, :w], ci)
            # K_newT local
            for ci in range(NI):
                ps = pp.tile([F, ICH], f32, space="PSUM", tag="ppq",
                             name="psk" + sfx)
                nc.tensor.matmul(out=ps[:], lhsT=Wkn_t[:],
                                 rhs=xlT_t[:, ci * ICH:(ci + 1) * ICH],
                                 start=True, stop=True)
                cp(KnT[:, ci * ICH:(ci + 1) * ICH], ps[:], ci)
            # V natural local rows (12 x [128,35]); N=35 is odd which FP32R
            # rejects — run these few as plain fp32 via bitcast
            for t in range(DBLK):
                ps = pp.tile([128, F], f32, space="PSUM", tag="ppv",
                             name="psv" + sfx)
                nc.tensor.matmul(out=ps[:],
                                 lhsT=xlT_t[:, t * 128:(t + 1) * 128].bitcast(f32),
                                 rhs=Wv_t[:].bitcast(f32),
                                 start=True, stop=True)
                cp(Vl[:, t, :], ps[:], t)
            # V' natural full (94 x [128,36]); zero the 32 pad rows of the
            # last chunk (j in [12000,12032))
            for j in range(JT):
                ps = pp.tile([128, F1], f32, space="PSUM", tag="ppv",
                             name="psvp" + sfx)
                nc.tensor.matmul(out=ps[:], lhsT=xhT_t[:, j * 128:(j + 1) * 128],
                                 rhs=Wva_t[:], start=True, stop=True)
                if j == JT - 1:
                    nc.vector.tensor_copy(out=Vp[:96, j, :], in_=ps[:96, :])
                    nc.vector.memset(Vp[96:128, j, :].bitcast(f32), 0.0)
                else:
                    cp(Vp[:, j, :], ps[:], j)

        # ---------------- attention ----------------
        # j-chunks in groups of 3: one ACT exp instruction covers
        # [128, 1536] (3 PSUM banks), amortizing the ~352-cycle ACTIVATE
        # overhead — ACT is the bottleneck engine of this phase.
        GROUPS = [(g * 3, 3) for g in range(JT // 3)]
        if JT % 3:
            GROUPS.append((JT - JT % 3, JT % 3))
        with tc.tile_pool(name="mm1p" + sfx, bufs=2, space="PSUM") as mm1p, \
             tc.tile_pool(name="Up" + sfx, bufs=1, space="PSUM") as Upp, \
             tc.tile_pool(name="tp" + sfx, bufs=1, space="PSUM") as tpp, \
             tc.tile_pool(name="esb" + sfx, bufs=3) as esb, \
             tc.tile_pool(name="usb" + sfx, bufs=2) as usb, \
             tc.tile_pool(name="hsm" + sfx, bufs=4) as hsmall:
            exp_f = mybir.ActivationFunctionType.Exp
            for ci in range(NI):
                Ups = Upp.tile([F1, ICH], f32, space="PSUM", tag="U",
                               name="Ups" + sfx)
                prev = None  # (exp_tile, j0, glen)
                for (j0, glen) in GROUPS:
                    ps = mm1p.tile([128, 3 * ICH], f32, space="PSUM", tag="s",
                                   name="pss" + sfx)
                    for k in range(glen):
                        j = j0 + k
                        nc.tensor.matmul(
                            out=ps[:, k * ICH:(k + 1) * ICH],
                            lhsT=QT[:, j * 128:(j + 1) * 128],
                            rhs=KnT[:, ci * ICH:(ci + 1) * ICH],
                            start=True, stop=True)
                    et = esb.tile([128, 3 * ICH], f32r, tag="e",
                                  name="et" + sfx)
                    nc.scalar.activation(out=et[:, :glen * ICH],
                                         in_=ps[:, :glen * ICH], func=exp_f)
                    if prev is not None:
                        pe, pj0, pglen = prev
                        for k in range(pglen):
                            nc.tensor.matmul(
                                out=Ups[:], lhsT=Vp[:, pj0 + k, :],
                                rhs=pe[:, k * ICH:(k + 1) * ICH],
                                start=(pj0 + k == 0), stop=False,
                                skip_group_check=True)
                    prev = (et, j0, glen)
                pe, pj0, pglen = prev
                for k in range(pglen):
                    nc.tensor.matmul(out=Ups[:], lhsT=Vp[:, pj0 + k, :],
                                     rhs=pe[:, k * ICH:(k + 1) * ICH],
                                     start=False, stop=(k == pglen - 1),
                                     skip_group_check=True)
                # normalize + residual + relu -> h natural tiles
                Usb = usb.tile([F1, ICH], f32, tag="usb", name="Usb" + sfx)
                nc.vector.tensor_copy(out=Usb[:], in_=Ups[:])
                for t in range(4):
                    blk = ci * 4 + t
                    up = tpp.tile([128, F1], f32, space="PSUM", tag="unat",
                                  name="up" + sfx)
                    nc.tensor.transpose(out=up[:],
                                        in_=Usb[:, t * 128:(t + 1) * 128],
                                        identity=ident_t[:F1, :F1])
                    rec = hsmall.tile([128, 1], f32, tag="rec",
                                      name="rec" + sfx)
                    nc.vector.reciprocal(out=rec[:], in_=up[:, F:F1])
                    hh = hsmall.tile([128, F], f32, tag="hh", name="hh" + sfx)
                    nc.vector.scalar_tensor_tensor(
                        out=hh[:], in0=up[:, :F], scalar=rec[:],
                        in1=Vl[:, blk, :], op0=mybir.AluOpType.mult,
                        op1=mybir.AluOpType.add)
                    nc.vector.tensor_scalar_max(out=hnat[:, blk, :F],
                                                in0=hh[:], scalar1=0.0)
                    nc.vector.memset(hnat[:, blk, F:HPAD], 0.0)
                    lo = blk * 128
                    nrows = min(128, max(0, ROWS - lo))
                    if nrows > 0:
                        nc.sync.dma_start(
                            out=d['h_loc'][lo:lo + nrows, :],
                            in_=hnat[:nrows, blk, :])
                # first AllGather piece (rows 0:1024) issues while the last
                # i-chunk is still computing — hides most of the collective.
                if ci == 1 and stop_after != 'attn':
                    if timeline:
                        for c in range(NCORE):
                            nc.sync.dma_start(
                                out=d['h_full'][c * AG0:(c + 1) * AG0, :],
                                in_=d['h_loc'][0:AG0, :])
                    elif no_collective:
                        nc.sync.dma_start(out=d['h_full'][0:AG0, :],
                                          in_=d['h_loc'][0:AG0, :])
                    else:
                        nc.gpsimd.collective_compute(
                            "AllGather", mybir.AluOpType.bypass,
                            replica_groups=[list(range(NCORE))],
                            ins=[d['h_loc'][0:AG0, :]],
                            outs=[d['h_full'][0:NCORE * AG0, :]])

        if stop_after == 'attn':
            with tc.tile_pool(name="fin" + sfx, bufs=1) as fin:
                ot = fin.tile([1, GB], f32, name="ot" + sfx)
                nc.vector.tensor_copy(out=ot[:], in_=hnat[0:1, 0, 0:GB])
                nc.sync.dma_start(out=d['out8'][:, :], in_=ot[:])
            return

        # hT local (for SAGE lin_r): transpose the 12 h tiles
        hT = main.tile([F, IPAD], f32, name="hT" + sfx)
        with tc.tile_pool(name="htp" + sfx, bufs=2, space="PSUM") as htp:
            for t in range(DBLK):
                ps = htp.tile([F, 128], f32, space="PSUM", tag="ht",
                              name="psht" + sfx)
                nc.tensor.transpose(out=ps[:], in_=hnat[:, t, :F],
                                    identity=ident_t[:])
                nc.vector.tensor_copy(out=hT[:, t * 128:(t + 1) * 128],
                                      in_=ps[:])

        # ---------------- AllGather h: second piece (rows 1024:1500) -----
        base = NCORE * AG0
        if timeline:
            for c in range(NCORE):
                nc.sync.dma_start(
                    out=d['h_full'][base + c * AG1: base + (c + 1) * AG1, :],
                    in_=d['h_loc'][AG0:ROWS, :])
        elif no_collective:
            nc.sync.dma_start(out=d['h_full'][base:base + AG1, :],
                              in_=d['h_loc'][AG0:ROWS, :])
        else:
            nc.gpsimd.collective_compute(
                "AllGather", mybir.AluOpType.bypass,
                replica_groups=[list(range(NCORE))],
                ins=[d['h_loc'][AG0:ROWS, :]],
                outs=[d['h_full'][base:N, :]])

        if stop_after == 'ag':
            with tc.tile_pool(name="fin" + sfx, bufs=1) as fin:
                ot = fin.tile([1, GB], f32, name="ot" + sfx)
                nc.sync.dma_start(out=ot[:], in_=d['h_full'][0:1, 0:GB])
                nc.sync.dma_start(out=d['out8'][:, :], in_=ot[:])
            return

        # ---------------- SAGE scatter ----------------
        aggdT = main.tile([F, IPAD], f32, name="aggdT" + sfx)
        h2T = main.tile([F, IPAD], f32, name="h2T" + sfx)
        with tc.tile_pool(name="gat" + sfx, bufs=1) as gat, \
             tc.tile_pool(name="sca" + sfx, bufs=4) as sca, \
             tc.tile_pool(name="scp" + sfx, bufs=2, space="PSUM") as scp, \
             tc.tile_pool(name="sin" + sfx, bufs=1) as sin:
            iota_t = sin.tile([128, 128], f32, name="iota_t" + sfx)
            nc.sync.dma_start(out=iota_t[:], in_=d['iota'][:, :])
            drel_t = sin.tile([128, DBLK * S], f32, name="drel_t" + sfx)
            nc.sync.dma_start(out=drel_t[:], in_=d['dstrel'][:, :])
            idx_t = sin.tile([128, DBLK * S * 8], mybir.dt.int16,
                             name="idx_t" + sfx)
            nc.sync.dma_start(out=idx_t[:], in_=d['gidx'][:, :])
            recT_t = sin.tile([F, IPAD], f32, name="recT_t" + sfx)
            nc.sync.dma_start(out=recT_t[:], in_=d['recipT'][:, :])

            G = gat.tile([128, DBLK * S, HPAD], f32, name="G" + sfx)
            GSPLIT = 4
            assert (DBLK * S) % GSPLIT == 0
            cpg = DBLK * S // GSPLIT
            for g in range(GSPLIT):
                nc.gpsimd.dma_gather(
                    out_ap=G[:, g * cpg:(g + 1) * cpg, :],
                    in_ap=d['h_full'][:, :],
                    idxs_ap=idx_t[:, g * cpg * 8:(g + 1) * cpg * 8],
                    num_idxs=cpg * 128,
                    num_idxs_reg=cpg * 128,
                    elem_size=HPAD,
                    single_packet=False)

            for b in range(DBLK):
                acc = scp.tile([F, 128], f32, space="PSUM", tag="agg",
                               name="acc" + sfx)
                for s in range(S):
                    ch = b * S + s
                    P = sca.tile([128, 128], f32, tag="P", name="P" + sfx)
                    nc.vector.tensor_scalar(
                        out=P[:], in0=iota_t[:],
                        scalar1=drel_t[:, ch:ch + 1], scalar2=None,
                        op0=mybir.AluOpType.is_equal)
                    nc.tensor.matmul(out=acc[:], lhsT=G[:, ch, :F], rhs=P[:],
                                     start=(s == 0), stop=(s == S - 1),
                                     skip_group_check=True)
                nc.vector.tensor_mul(out=aggdT[:, b * 128:(b + 1) * 128],
                                     in0=acc[:],
                                     in1=recT_t[:, b * 128:(b + 1) * 128])

        # ---------------- SAGE linear + pool + MLP ----------------
        with tc.tile_pool(name="mlpw" + sfx, bufs=1) as mlpw, \
             tc.tile_pool(name="mlps" + sfx, bufs=2) as mlps, \
             tc.tile_pool(name="mlpp" + sfx, bufs=2, space="PSUM") as mlpp:
            WllT_t = mlpw.tile([F, F], f32, name="WllT_t" + sfx)
            nc.sync.dma_start(out=WllT_t[:], in_=d['WllT'][:, :])
            WlrT_t = mlpw.tile([F, F], f32, name="WlrT_t" + sfx)
            nc.sync.dma_start(out=WlrT_t[:], in_=d['WlrT'][:, :])
            bll_t = mlpw.tile([F, 1], f32, name="bll_t" + sfx)
            nc.sync.dma_start(out=bll_t[:], in_=d['bll'][:, :])
            Wg1T_t = mlpw.tile([F, 1500], f32, name="Wg1T_t" + sfx)
            nc.sync.dma_start(out=Wg1T_t[:], in_=d['Wg1T'][:, :])
            bg1_t = mlpw.tile([128, 12], f32, name="bg1_t" + sfx)
            nc.sync.dma_start(out=bg1_t[:], in_=d['bg1'][:, :])
            Wg2_t = mlpw.tile([128, 12 * 128], f32, name="Wg2_t" + sfx)
            nc.sync.dma_start(out=Wg2_t[:], in_=d['Wg2Tr'][:, :])
            bg2_t = mlpw.tile([128, 1], f32, name="bg2_t" + sfx)
            nc.sync.dma_start(out=bg2_t[:], in_=d['bg2'][:, :])
            WoT_t = mlpw.tile([128, 1], f32, name="WoT_t" + sfx)
            nc.sync.dma_start(out=WoT_t[:], in_=d['WoT'][:, :])

            relu_f = mybir.ActivationFunctionType.Relu
            for ci in range(NI):
                ps = mlpp.tile([F, ICH], f32, space="PSUM", tag="h2",
                               name="psh2" + sfx)
                nc.tensor.matmul(out=ps[:], lhsT=WllT_t[:],
                                 rhs=aggdT[:, ci * ICH:(ci + 1) * ICH],
                                 start=True, stop=False, skip_group_check=True)
                nc.tensor.matmul(out=ps[:], lhsT=WlrT_t[:],
                                 rhs=hT[:, ci * ICH:(ci + 1) * ICH],
                                 start=False, stop=True, skip_group_check=True)
                nc.scalar.activation(out=h2T[:, ci * ICH:(ci + 1) * ICH],
                                     in_=ps[:], func=relu_f, bias=bll_t[:])

            gT = mlps.tile([F, GB], f32, name="gT" + sfx)
            for g in range(GB):
                lo, hi = GRAPH_BOUNDS[g], GRAPH_BOUNDS[g + 1]
                nc.vector.tensor_reduce(out=gT[:, g:g + 1], in_=h2T[:, lo:hi],
                                        axis=mybir.AxisListType.X,
                                        op=mybir.AluOpType.max)
            g1T = mlps.tile([128, 12, GB], f32, name="g1T" + sfx)
            for j in range(12):
                w = min(128, 1500 - j * 128)
                ps = mlpp.tile([128, GB], f32, space="PSUM", tag="g1",
                               name="psg1" + sfx)
                nc.tensor.matmul(out=ps[:w, :],
                                 lhsT=Wg1T_t[:, j * 128:j * 128 + w],
                                 rhs=gT[:], start=True, stop=True)
                if w < 128:
                    nc.vector.memset(g1T[:, j, :], 0.0)
                nc.scalar.activation(out=g1T[:w, j, :], in_=ps[:w, :],
                                     func=relu_f, bias=bg1_t[:w, j:j + 1])
            g2ps = mlpp.tile([128, GB], f32, space="PSUM", tag="g2",
                             name="g2ps" + sfx)
            for j in range(12):
                nc.tensor.matmul(out=g2ps[:],
                                 lhsT=Wg2_t[:, j * 128:(j + 1) * 128],
                                 rhs=g1T[:, j, :], start=(j == 0),
                                 stop=(j == 11), skip_group_check=True)
            g2sb = mlps.tile([128, GB], f32, name="g2sb" + sfx)
            nc.vector.tensor_scalar_add(out=g2sb[:], in0=g2ps[:],
                                        scalar1=bg2_t[:])
            ops = mlpp.tile([1, GB], f32, space="PSUM", tag="o",
                            name="ops" + sfx)
            nc.tensor.matmul(out=ops[:], lhsT=WoT_t[:], rhs=g2sb[:],
                             start=True, stop=True)
            osb = mlps.tile([1, GB], f32, name="osb" + sfx)
            nc.vector.tensor_scalar_add(out=osb[:], in0=ops[:],
                                        scalar1=float(bo_const))
            nc.sync.dma_start(out=d['out8'][:, :], in_=osb[:])


def _build_program(S, bo_const, timeline=False, stop_after=None, reps=1,
                   no_collective=False):
    """timeline=True: 1-core variant, AllGather replaced by local DMA copies
    (TimelineSim cost estimation only). stop_after truncates for perf
    bisection. reps replicates the whole body in-NEFF for benchmarking."""
    import concourse.tile as tile
    from concourse import bacc, mybir

    f32 = mybir.dt.float32
    f32r = mybir.dt.float32r
    nc = bacc.Bacc("TRN2", target_bir_lowering=False, debug=False,
                   num_devices=1 if timeline else NCORE)

    d = {}

    def dram_in(name, shape, dt=f32):
        d[name] = nc.dram_tensor(name, list(shape), dt, kind="ExternalInput")

    dram_in("xhT", (F1, XW), f32r)
    dram_in("xlT", (F1, IPAD), f32r)
    dram_in("Wq_h", (F1, F), f32r)
    dram_in("Wkn_h", (F1, F), f32r)
    dram_in("Wv_h", (F1, F), f32r)
    dram_in("Wv_aug", (F1, F1), f32r)
    dram_in("WllT", (F, F))
    dram_in("WlrT", (F, F))
    dram_in("bll", (F, 1))
    dram_in("Wg1T", (F, 1500))
    dram_in("bg1", (128, 12))
    dram_in("Wg2Tr", (128, 12 * 128))
    dram_in("bg2", (128, 1))
    dram_in("WoT", (128, 1))
    dram_in("recipT", (F, IPAD))
    dram_in("iota", (128, 128))
    dram_in("ident", (128, 128))
    d['gidx'] = nc.dram_tensor("gidx", [128, DBLK * S * 8], mybir.dt.int16,
                               kind="ExternalInput")
    d['dstrel'] = nc.dram_tensor("dstrel", [128, DBLK * S], f32,
                                 kind="ExternalInput")
    d['out8'] = nc.dram_tensor("out8", [1, GB], f32, kind="ExternalOutput")
    d['h_loc'] = nc.dram_tensor("h_loc", [ROWS, HPAD], f32)
    d['h_full'] = nc.dram_tensor("h_full", [N, HPAD], f32,
                                 addr_space="Shared")
    # rename dram handles to match in_map keys
    d['xlT'] = d.pop('xlT') if 'xlT' in d else d['xlT']

    with tile.TileContext(nc) as tc:
        for r in range(reps):
            _emit_body(nc, tc, d, S, bo_const, timeline, stop_after,
                       "" if reps == 1 else f"_r{r}",
                       no_collective=no_collective)

    nc.compile()
    return nc


# --------------------------------------------------------------------------
# entry point
# --------------------------------------------------------------------------

_CACHE = {}


def _make_in_maps(inputs):
    x = np.asarray(inputs['x'], np.float32)
    edge_index = np.asarray(inputs['edge_index'])
    w = _prep_weights(inputs)
    xhT, xl = _prep_x(x)
    gidx, dstrel, recipT, S = _prep_edges(edge_index)
    iota = np.ascontiguousarray(
        np.broadcast_to(np.arange(128, dtype=np.float32), (128, 128)))
    ident = np.eye(128, dtype=np.float32)
    common = dict(
        xhT=xhT, Wq_h=_round_mant(w['Wq_h']), Wkn_h=_round_mant(w['Wkn_h']),
        Wv_h=_round_mant(w['Wv_h']), Wv_aug=_round_mant(w['Wv_aug']),
        WllT=w['WllT'], WlrT=w['WlrT'], bll=w['bll'],
        Wg1T=w['Wg1T'], bg1=w['bg1'], Wg2Tr=w['Wg2Tr'], bg2=w['bg2'],
        WoT=w['WoT'], iota=iota, ident=ident)
    in_maps = []
    for c in range(NCORE):
        m = dict(common)
        m['xlT'] = xl[c]
        m['gidx'] = gidx[c]
        m['dstrel'] = dstrel[c]
        m['recipT'] = recipT[c]
        in_maps.append(m)
    return in_maps, S, w['bo']


def kernel(**inputs):
    from concourse.bass_utils import run_bass_kernel_spmd

    in_maps, S, bo = _make_in_maps(inputs)
    key = ('prog', S, bo)
    if key not in _CACHE:
        _CACHE[key] = _build_program(S, bo)
    nc = _CACHE[key]

    res = run_bass_kernel_spmd(nc, in_maps, list(range(NCORE)))
    global LAST_RESULT
    LAST_RESULT = res
    out = np.zeros((B, 1), np.float32)
    for c in range(NCORE):
        out[c * GB:(c + 1) * GB, 0] = res.results[c]['out8'].reshape(-1)
    return out


LAST_RESULT = None



# revision 5
# speedup vs baseline: 1.0616x; 1.0616x over previous
"""Trainium2 Bass kernel for nn_GAT_GraphSAGE (N=12000, E=192000, F=35, B=64).

Sharding: attention rows (softmax row i = K_new index) sharded 1500/core on
8 cores; one AllGather of post-attention h (bf16, 2 chunked pieces); SAGE
sharded by dst with a batched dma_gather of h[src] rows + one-hot-matmul
scatter; per-core global-max-pool + MLP head on that core's 8 graphs.

Key structure (v2):
- The whole K branch (Wk/conv-taps/Wl/1/sqrt(F)) and Wq fold on the host
  into one [F1,F1] matrix M (F1=36 with a ones row for biases):
  scores = x~ M x~^T.  The i-side projection KQT = M^T x~_loc^T is also
  host-computed, so the device never projects Q or K.
- All big matmuls run in bf16 at full PE rate, packed 2x2 into the 64x64
  tile grid (contraction 36 <= 64): scores split into two M-halves
  (PSUM col groups), with the j-parity on row groups (operands duplicated
  at partitions 64:100); the attn@V accumulation contracts raw x~ against
  exp(scores) split into two K-halves (row groups -> two PSUM banks), and
  the V projection (Wv augmented with a ones column that also carries the
  softmax denominator) is applied afterwards to the tiny [36,512] result.
  The same post-matmul also produces h in natural layout, so no PE-mode
  switches happen inside the attention loop.
- exp on ACT in [128,1536] batches (3 PSUM banks, double-buffered) is the
  bottleneck engine (~150us); everything else hides under it.
- SAGE scatter: one-hot P matrices are host-precomputed bf16 inputs (DMA,
  not DVE is_equal), gathered h rows are bf16 256B rows.
"""
import math
import numpy as np
import ml_dtypes

BF16 = ml_dtypes.bfloat16

N, E, F, B = 12000, 192000, 35, 64
F1 = F + 1
NCORE = 8
ROWS = N // NCORE            # 1500
ICH = 512
NI = 3
IPAD = ICH * NI              # 1536
JT = 94                      # j chunks of 128
JPAD = JT * 128              # 12032
XW = 12064                   # padded x~^T width (covers 7*1500 + 1536)
DBLK = 12                    # dst blocks (128 each) per core
GB = B // NCORE              # 8 graphs per core
HPAD = 128                   # h row padded to 128 bf16 (256B) for dma_gather
GRAPH_BOUNDS = [int(math.ceil(g * (N / B))) for g in range(GB + 1)]
AG0 = 1024                   # rows in first AllGather piece (i-chunks 0,1)
AG1 = ROWS - AG0             # 476 rows in second piece
GSPLIT = 4                   # gather split (3 dst blocks each)
BPS = DBLK // GSPLIT         # blocks per split


# --------------------------------------------------------------------------
# host-side preprocessing
# --------------------------------------------------------------------------

def _prep_weights(p):
    f64 = np.float64
    f32 = np.float32
    Wq, bq = p['Wq'].astype(f64), p['bq'].astype(f64)
    Wk, bk = p['Wk'].astype(f64), p['bk'].astype(f64)
    Wv, bv = p['Wv'].astype(f64), p['bv'].astype(f64)
    W3c, b3 = p['W3'][:, :, 1].astype(f64), p['b3'].astype(f64)
    W5c, b5 = p['W5'][:, :, 2].astype(f64), p['b5'].astype(f64)
    Wl, bl = p['Wl'].astype(f64), p['bl'].astype(f64)
    Wl1, Wl2, Wl3 = Wl[:, :F], Wl[:, F:2 * F], Wl[:, 2 * F:]

    # K_new = x~ @ Wkn~  (F1 -> F affine, includes 1/sqrt(F))
    Weff = W3c.T @ Wl1.T + W5c.T @ Wl2.T + Wl3.T
    beff = b3 @ Wl1.T + b5 @ Wl2.T + bl
    Wkn = Wk.T @ Weff
    bkn = bk @ Weff + beff
    s = 1.0 / np.sqrt(F)
    Wkn_aug = np.vstack([Wkn, bkn[None, :]]) * s          # [F1, F]
    Wq_aug = np.vstack([Wq.T, bq[None, :]])               # [F1, F]
    M = Wkn_aug @ Wq_aug.T                                # [F1, F1]

    Wva = np.zeros((F1, F1))
    Wva[:F, :F] = Wv.T
    Wva[F, :F] = bv
    Wva[F, F] = 1.0                                       # denominator column

    out = {'M': M, 'Wva': Wva.astype(BF16)}
    out['WllT'] = np.ascontiguousarray(p['Wll'].T).astype(f32)
    out['WlrT'] = np.ascontiguousarray(p['Wlr'].T).astype(f32)
    out['bll'] = p['bll'].astype(f32).reshape(F, 1)
    out['Wg1T'] = np.ascontiguousarray(p['Wg1'].T).astype(f32)   # [35,1500]
    bg1 = np.zeros((128, 12), f32)
    bg1.T.reshape(-1)[:1500] = p['bg1'].astype(f32)
    out['bg1'] = bg1
    w2 = np.zeros((12 * 128, 128), f32)
    w2[:1500, :] = p['Wg2'].T.astype(f32)
    out['Wg2Tr'] = np.ascontiguousarray(
        w2.reshape(12, 128, 128).transpose(1, 0, 2).reshape(128, 12 * 128))
    out['bg2'] = p['bg2'].astype(f32).reshape(128, 1)
    out['WoT'] = p['Wo'].astype(f32).reshape(1, 128).T.copy()     # [128,1]
    out['bo'] = float(np.asarray(p['bo']).reshape(-1)[0])
    return out


def _prep_x(x, M, Wva_unused):
    """Host: x~^T (bf16), x~ natural chunked (bf16), per-core KQT + Vl."""
    x64 = np.asarray(x, np.float64)
    xa = np.concatenate([x64, np.ones((N, 1))], axis=1)       # [N, F1]
    xaT = np.zeros((F1, XW))
    xaT[:, :N] = xa.T                                         # pad cols zero
    xh = xaT.astype(BF16)                                     # [F1, XW]

    # natural chunks for the U accumulation: [128, JT, F1]
    xn = np.zeros((128, JT, F1))
    flat = xaT[:, :JPAD].T                                    # [JPAD, F1]
    xn[:, :, :] = flat.reshape(JT, 128, F1).transpose(1, 0, 2)
    xn = np.ascontiguousarray(xn.reshape(128, JT * F1)).astype(BF16)

    KQ = []
    for c in range(NCORE):
        sl = xaT[:, c * ROWS: c * ROWS + IPAD]                # [F1, IPAD]
        KQ.append(np.ascontiguousarray(M.T @ sl).astype(BF16))
    return xh, xn, KQ


def _prep_vl(x, p):
    """Per-core natural V' local [128, DBLK*F] f32 (for the residual)."""
    f64 = np.float64
    Wv, bv = p['Wv'].astype(f64), p['bv'].astype(f64)
    x64 = np.asarray(x, np.float64)
    V = x64 @ Wv.T + bv                                       # [N, F]
    out = []
    for c in range(NCORE):
        vl = np.zeros((DBLK * 128, F))
        vl[:ROWS] = V[c * ROWS:(c + 1) * ROWS]
        out.append(np.ascontiguousarray(
            vl.reshape(DBLK, 128, F).transpose(1, 0, 2).reshape(128, DBLK * F)
        ).astype(np.float32))
    return out


def _prep_edges(edge_index):
    src = np.asarray(edge_index[0], np.int64)
    dst = np.asarray(edge_index[1], np.int64)
    deg = np.bincount(dst, minlength=N).astype(np.float64)
    recip = (1.0 / np.maximum(deg, 1.0)).astype(np.float32)

    core_of = dst // ROWS
    blk_of = (dst - core_of * ROWS) // 128
    counts = np.zeros((NCORE, DBLK), np.int64)
    np.add.at(counts, (core_of, blk_of), 1)
    S = int(np.ceil(counts.max() / 128))
    CH = DBLK * S

    order = np.lexsort((dst,))
    src_s, dst_s = src[order], dst[order]
    core_s, blk_s = core_of[order], blk_of[order]

    # global src node -> row in the piece-wise AllGathered h_full
    sc = src_s // ROWS
    sr = src_s - sc * ROWS
    src_pos = np.where(sr < AG0, sc * AG0 + sr,
                       NCORE * AG0 + sc * AG1 + (sr - AG0))

    gidx, Ps = [], []
    for c in range(NCORE):
        idx_c = np.zeros(CH * 128, np.int16)
        rel_c = np.full(CH * 128, -1, np.int64)
        m_c = core_s == c
        for b in range(DBLK):
            m = m_c & (blk_s == b)
            n = int(m.sum())
            lo = b * S * 128
            idx_c[lo:lo + n] = src_pos[m].astype(np.int16)
            rel_c[lo:lo + n] = dst_s[m] - c * ROWS - b * 128
        gidx.append(np.ascontiguousarray(
            np.tile(idx_c.reshape(-1, 16).T, (8, 1))))
        # one-hot P: [128 slot, CH*128], P[e, ch*128 + rel] = 1
        P = np.zeros((128, CH * 128), BF16)
        rel2 = rel_c.reshape(CH, 128)
        ch_i, e_i = np.nonzero(rel2 >= 0)
        P[e_i, ch_i * 128 + rel2[ch_i, e_i]] = 1
        Ps.append(np.ascontiguousarray(P))

    recipT = []
    for c in range(NCORE):
        r = np.ones(IPAD, np.float32)
        r[:ROWS] = recip[c * ROWS:(c + 1) * ROWS]
        recipT.append(np.ascontiguousarray(np.broadcast_to(r, (F, IPAD))))
    return gidx, Ps, recipT, S


# --------------------------------------------------------------------------
# device program
# --------------------------------------------------------------------------

def _emit_body(nc, tc, d, S, bo_const):
    import concourse.tile as tile
    from concourse import mybir

    f32 = mybir.dt.float32
    bf16 = mybir.dt.bfloat16
    CH = DBLK * S
    CPS = BPS * S            # gather chunks per split

    with tc.tile_pool(name="const", bufs=1) as constp, \
         tc.tile_pool(name="main", bufs=1) as main:
        # ---- inputs ----
        KQT = main.tile([128, IPAD], bf16, name="KQT")
        nc.sync.dma_start(out=KQT[0:F1, :], in_=d['KQ'][:, :])
        nc.sync.dma_start(out=KQT[64:64 + F1, :], in_=d['KQ'][:, :])
        xhT = main.tile([128, XW], bf16, name="xhT")
        nc.sync.dma_start(out=xhT[0:F1, :], in_=d['xh'][:, :])
        nc.sync.dma_start(out=xhT[64:64 + F1, :], in_=d['xh'][:, :])
        xn = main.tile([128, JT * F1], bf16, name="xn")
        nc.sync.dma_start(out=xn[:], in_=d['xn'][:, :])
        Wva_t = constp.tile([F1, F1], bf16, name="Wva_t")
        nc.sync.dma_start(out=Wva_t[:], in_=d['Wva'][:, :])
        Vl = main.tile([128, DBLK * F], f32, name="Vl")
        nc.sync.dma_start(out=Vl[:], in_=d['Vl'][:, :])
        ident_t = constp.tile([128, 128], bf16, name="ident_t")
        nc.sync.dma_start(out=ident_t[:], in_=d['ident'][:, :])

        hnat = main.tile([128, DBLK, HPAD], bf16, name="hnat")
        nc.vector.memset(hnat[:, :, F:HPAD], 0.0)

        # ---------------- attention ----------------
        # groups of 3 j-chunks; one [128,1536] exp per group (double-buffered
        # PSUM). U' = sum_j x~_j^T exp[j,:] accumulated in two K-half chains
        # (row groups 0/64 -> banks C/D); V-projection applied after.
        GROUPS = [(g * 3, min(3, JT - g * 3)) for g in range((JT + 2) // 3)]
        exp_f = mybir.ActivationFunctionType.Exp
        with tc.tile_pool(name="mm1p", bufs=2, space="PSUM") as mm1p, \
             tc.tile_pool(name="Up", bufs=1, space="PSUM") as Upp, \
             tc.tile_pool(name="esb", bufs=3) as esb, \
             tc.tile_pool(name="usb", bufs=2) as usb, \
             tc.tile_pool(name="hsm", bufs=4) as hsmall:
            UC = Upp.tile([128, ICH], f32, name="UC")
            UD = Upp.tile([128, ICH], f32, name="UD")
            for ci in range(NI):
                prev = None
                for (j0, glen) in GROUPS:
                    ps = mm1p.tile([128, 3 * ICH], f32, space="PSUM",
                                   tag="s", name="pss")
                    for k in range(glen):
                        j = j0 + k
                        r = 64 * (j & 1)
                        for ch in range(2):
                            nc.tensor.matmul(
                                out=ps[64 * ch:64 * ch + 64,
                                       k * ICH:(k + 1) * ICH],
                                lhsT=xhT[r:r + F1,
                                         j * 128 + 64 * ch:
                                         j * 128 + 64 * ch + 64],
                                rhs=KQT[r:r + F1,
                                        ci * ICH:(ci + 1) * ICH],
                                start=True, stop=True)
                    et = esb.tile([128, 3 * ICH], bf16, tag="e", name="et")
                    nc.scalar.activation(out=et[:, :glen * ICH],
                                         in_=ps[:, :glen * ICH], func=exp_f)
                    if prev is not None:
                        pe, pj0, pglen = prev
                        for k in range(pglen):
                            j = pj0 + k
                            for r in range(2):
                                nc.tensor.matmul(
                                    out=(UC if r == 0 else UD)[0:F1, :],
                                    lhsT=xn[64 * r:64 * r + 64, j * F1:(j + 1) * F1],
                                    rhs=pe[64 * r:64 * r + 64,
                                           k * ICH:(k + 1) * ICH],
                                    start=(j == 0), stop=False,
                                    skip_group_check=True)
                    prev = (et, j0, glen)
                pe, pj0, pglen = prev
                for k in range(pglen):
                    j = pj0 + k
                    for r in range(2):
                        nc.tensor.matmul(
                            out=(UC if r == 0 else UD)[0:F1, :],
                            lhsT=xn[64 * r:64 * r + 64, j * F1:(j + 1) * F1],
                            rhs=pe[64 * r:64 * r + 64,
                                   k * ICH:(k + 1) * ICH],
                            start=False, stop=(k == pglen - 1),
                            skip_group_check=True)
                # combine K-halves -> U'sb bf16 [F1, 512]
                # (avoid a two-PSUM-operand tensor_tensor: copy then add)
                Ucs = usb.tile([F1, ICH], f32, tag="ucs", name="Ucs")
                nc.vector.tensor_copy(out=Ucs[:], in_=UC[0:F1, :])
                Usb = usb.tile([F1, ICH], bf16, tag="usb", name="Usb")
                nc.vector.tensor_add(out=Usb[:], in0=Ucs[:],
                                     in1=UD[0:F1, :])
                # h natural: hraw[i,g] = sum_f U'sb[f,i] Wva[f,g]
                # (two 64-col halves to stay in the 64x64 tile grid)
                for t in range(4):
                    blk = ci * 4 + t
                    for ch in range(2):
                        nc.tensor.matmul(
                            out=UD[64 * ch:64 * ch + 64,
                                   t * 128:t * 128 + F1],
                            lhsT=Usb[:, t * 128 + 64 * ch:
                                     t * 128 + 64 * ch + 64],
                            rhs=Wva_t[:],
                            start=True, stop=True, skip_group_check=True)
                    hraw = UD[:, t * 128:t * 128 + F1]
                    rec = hsmall.tile([128, 1], f32, tag="rec", name="rec")
                    nc.vector.reciprocal(out=rec[:], in_=hraw[:, F:F1])
                    hh = hsmall.tile([128, F], f32, tag="hh", name="hh")
                    nc.vector.scalar_tensor_tensor(
                        out=hh[:], in0=hraw[:, :F], scalar=rec[:],
                        in1=Vl[:, blk * F:(blk + 1) * F],
                        op0=mybir.AluOpType.mult,
                        op1=mybir.AluOpType.add)
                    nc.vector.tensor_scalar_max(out=hnat[:, blk, :F],
                                                in0=hh[:], scalar1=0.0)
                    lo = blk * 128
                    nrows = min(128, max(0, ROWS - lo))
                    if nrows > 0:
                        nc.sync.dma_start(
                            out=d['h_loc'][lo:lo + nrows, :],
                            in_=hnat[:nrows, blk, :])
                if ci == 1:
                    nc.gpsimd.collective_compute(
                        "AllGather", mybir.AluOpType.bypass,
                        replica_groups=[list(range(NCORE))],
                        ins=[d['h_loc'][0:AG0, :]],
                        outs=[d['h_full'][0:NCORE * AG0, :]])

        # hT (bf16) for SAGE lin_r: transpose the 12 h tiles
        hT = main.tile([F, IPAD], bf16, name="hT")
        with tc.tile_pool(name="htp", bufs=2, space="PSUM") as htp:
            for t in range(DBLK):
                ps = htp.tile([F, 128], bf16, space="PSUM", tag="ht",
                              name="psht")
                nc.tensor.transpose(out=ps[:], in_=hnat[:, t, :F],
                                    identity=ident_t[:])
                nc.vector.tensor_copy(out=hT[:, t * 128:(t + 1) * 128],
                                      in_=ps[:])

        # ---------------- AllGather piece 2 (rows 1024:1500) ------------
        base = NCORE * AG0
        nc.gpsimd.collective_compute(
            "AllGather", mybir.AluOpType.bypass,
            replica_groups=[list(range(NCORE))],
            ins=[d['h_loc'][AG0:ROWS, :]],
            outs=[d['h_full'][base:N, :]])

        # ---------------- SAGE scatter ----------------
        aggdT = main.tile([F, IPAD], f32, name="aggdT")
        with tc.tile_pool(name="gat", bufs=2) as gat, \
             tc.tile_pool(name="pin", bufs=2) as pin, \
             tc.tile_pool(name="scp", bufs=2, space="PSUM") as scp, \
         tc.tile_pool(name="sin", bufs=1) as sin:
            idx_t = sin.tile([128, CH * 8], mybir.dt.int16, name="idx_t")
            nc.sync.dma_start(out=idx_t[:], in_=d['gidx'][:, :])
            recT_t = sin.tile([F, IPAD], f32, name="recT_t")
            nc.sync.dma_start(out=recT_t[:], in_=d['recipT'][:, :])

            for g in range(GSPLIT):
                G = gat.tile([128, CPS, HPAD], bf16, tag="G", name="G")
                nc.gpsimd.dma_gather(
                    out_ap=G[:, :, :],
                    in_ap=d['h_full'][:, :],
                    idxs_ap=idx_t[:, g * CPS * 8:(g + 1) * CPS * 8],
                    num_idxs=CPS * 128,
                    num_idxs_reg=CPS * 128,
                    elem_size=HPAD,
                    single_packet=False)
                Pt = pin.tile([128, CPS * 128], bf16, tag="P", name="Pt")
                nc.sync.dma_start(
                    out=Pt[:],
                    in_=d['P'][:, g * CPS * 128:(g + 1) * CPS * 128])
                for bb in range(BPS):
                    b = g * BPS + bb
                    acc = scp.tile([F, 128], f32, space="PSUM", tag="agg",
                                   name="acc")
                    for s in range(S):
                        ch = bb * S + s
                        nc.tensor.matmul(
                            out=acc[:], lhsT=G[:, ch, :F],
                            rhs=Pt[:, ch * 128:(ch + 1) * 128],
                            start=(s == 0), stop=(s == S - 1),
                            skip_group_check=True)
                    nc.vector.tensor_mul(
                        out=aggdT[:, b * 128:(b + 1) * 128], in0=acc[:],
                        in1=recT_t[:, b * 128:(b + 1) * 128])

        # ---------------- SAGE linear + pool + MLP ----------------
        with tc.tile_pool(name="mlpw", bufs=1) as mlpw, \
             tc.tile_pool(name="mlps", bufs=2) as mlps, \
             tc.tile_pool(name="mlpp", bufs=2, space="PSUM") as mlpp:
            WllT_t = mlpw.tile([F, F], bf16, name="WllT_t")
            nc.sync.dma_start(out=WllT_t[:], in_=d['WllT'][:, :])
            WlrT_t = mlpw.tile([F, F], bf16, name="WlrT_t")
            nc.sync.dma_start(out=WlrT_t[:], in_=d['WlrT'][:, :])
            bll_t = mlpw.tile([F, 1], f32, name="bll_t")
            nc.sync.dma_start(out=bll_t[:], in_=d['bll'][:, :])
            Wg1T_t = mlpw.tile([F, 1500], f32, name="Wg1T_t")
            nc.sync.dma_start(out=Wg1T_t[:], in_=d['Wg1T'][:, :])
            bg1_t = mlpw.tile([128, 12], f32, name="bg1_t")
            nc.sync.dma_start(out=bg1_t[:], in_=d['bg1'][:, :])
            Wg2_t = mlpw.tile([128, 12 * 128], f32, name="Wg2_t")
            nc.sync.dma_start(out=Wg2_t[:], in_=d['Wg2Tr'][:, :])
            bg2_t = mlpw.tile([128, 1], f32, name="bg2_t")
            nc.sync.dma_start(out=bg2_t[:], in_=d['bg2'][:, :])
            WoT_t = mlpw.tile([128, 1], f32, name="WoT_t")
            nc.sync.dma_start(out=WoT_t[:], in_=d['WoT'][:, :])

            # aggdT (f32) -> bf16 for the bf16 linear
            aggb = mlps.tile([F, IPAD], bf16, tag="aggb", name="aggb")
            nc.vector.tensor_copy(out=aggb[:], in_=aggdT[:])

            relu_f = mybir.ActivationFunctionType.Relu
            h2T = mlps.tile([F, IPAD], f32, tag="h2T", name="h2T")
            for ci in range(NI):
                ps = mlpp.tile([F, ICH], f32, space="PSUM", tag="h2",
                               name="psh2")
                nc.tensor.matmul(out=ps[:], lhsT=WllT_t[:],
                                 rhs=aggb[:, ci * ICH:(ci + 1) * ICH],
                                 start=True, stop=False,
                                 skip_group_check=True)
                nc.tensor.matmul(out=ps[:], lhsT=WlrT_t[:],
                                 rhs=hT[:, ci * ICH:(ci + 1) * ICH],
                                 start=False, stop=True,
                                 skip_group_check=True)
                nc.scalar.activation(out=h2T[:, ci * ICH:(ci + 1) * ICH],
                                     in_=ps[:], func=relu_f, bias=bll_t[:])

            gT = mlps.tile([F, GB], f32, tag="gT", name="gT")
            for g in range(GB):
                lo, hi = GRAPH_BOUNDS[g], GRAPH_BOUNDS[g + 1]
                nc.vector.tensor_reduce(out=gT[:, g:g + 1], in_=h2T[:, lo:hi],
                                        axis=mybir.AxisListType.X,
                                        op=mybir.AluOpType.max)
            g1T = mlps.tile([128, 12, GB], f32, tag="g1T", name="g1T")
            for j in range(12):
                w = min(128, 1500 - j * 128)
                ps = mlpp.tile([128, GB], f32, space="PSUM", tag="g1",
                               name="psg1")
                nc.tensor.matmul(out=ps[:w, :],
                                 lhsT=Wg1T_t[:, j * 128:j * 128 + w],
                                 rhs=gT[:], start=True, stop=True)
                if w < 128:
                    nc.vector.memset(g1T[:, j, :], 0.0)
                nc.scalar.activation(out=g1T[:w, j, :], in_=ps[:w, :],
                                     func=relu_f, bias=bg1_t[:w, j:j + 1])
            g2ps = mlpp.tile([128, GB], f32, space="PSUM", tag="g2",
                             name="g2ps")
            for j in range(12):
                nc.tensor.matmul(out=g2ps[:],
                                 lhsT=Wg2_t[:, j * 128:(j + 1) * 128],
                                 rhs=g1T[:, j, :], start=(j == 0),
                                 stop=(j == 11), skip_group_check=True)
            g2sb = mlps.tile([128, GB], f32, tag="g2sb", name="g2sb")
            nc.vector.tensor_scalar_add(out=g2sb[:], in0=g2ps[:],
                                        scalar1=bg2_t[:])
            ops = mlpp.tile([1, GB], f32, space="PSUM", tag="o", name="ops")
            nc.tensor.matmul(out=ops[:], lhsT=WoT_t[:], rhs=g2sb[:],
                             start=True, stop=True)
            osb = mlps.tile([1, GB], f32, tag="osb", name="osb")
            nc.vector.tensor_scalar_add(out=osb[:], in0=ops[:],
                                        scalar1=float(bo_const))
            nc.sync.dma_start(out=d['out8'][:, :], in_=osb[:])


def _build_program(S, bo_const):
    import concourse.tile as tile
    from concourse import bacc, mybir

    f32 = mybir.dt.float32
    bf16 = mybir.dt.bfloat16
    CH = DBLK * S
    nc = bacc.Bacc("TRN2", target_bir_lowering=False, debug=False,
                   num_devices=NCORE)

    d = {}

    def dram_in(name, shape, dt=f32):
        d[name] = nc.dram_tensor(name, list(shape), dt, kind="ExternalInput")

    dram_in("xh", (F1, XW), bf16)
    dram_in("xn", (128, JT * F1), bf16)
    dram_in("KQ", (F1, IPAD), bf16)
    dram_in("Wva", (F1, F1), bf16)
    dram_in("Vl", (128, DBLK * F), f32)
    dram_in("ident", (128, 128), bf16)
    dram_in("WllT", (F, F), bf16)
    dram_in("WlrT", (F, F), bf16)
    dram_in("bll", (F, 1))
    dram_in("Wg1T", (F, 1500))
    dram_in("bg1", (128, 12))
    dram_in("Wg2Tr", (128, 12 * 128))
    dram_in("bg2", (128, 1))
    dram_in("WoT", (128, 1))
    dram_in("recipT", (F, IPAD))
    dram_in("P", (128, CH * 128), bf16)
    d['gidx'] = nc.dram_tensor("gidx", [128, CH * 8], mybir.dt.int16,
                               kind="ExternalInput")
    d['out8'] = nc.dram_tensor("out8", [1, GB], f32, kind="ExternalOutput")
    d['h_loc'] = nc.dram_tensor("h_loc", [ROWS, HPAD], bf16)
    d['h_full'] = nc.dram_tensor("h_full", [N, HPAD], bf16,
                                 addr_space="Shared")

    with tile.TileContext(nc) as tc:
        _emit_body(nc, tc, d, S, bo_const)

    nc.compile()
    return nc


# --------------------------------------------------------------------------
# entry point
# --------------------------------------------------------------------------

_CACHE = {}


def _make_in_maps(inputs):
    x = np.asarray(inputs['x'], np.float32)
    edge_index = np.asarray(inputs['edge_index'])
    w = _prep_weights(inputs)
    xh, xn, KQ = _prep_x(x, w['M'], w['Wva'])
    Vl = _prep_vl(x, inputs)
    gidx, Ps, recipT, S = _prep_edges(edge_index)
    ident = np.eye(128, dtype=BF16)
    common = dict(
        xh=xh, xn=xn, Wva=w['Wva'], ident=ident,
        WllT=w['WllT'].astype(BF16), WlrT=w['WlrT'].astype(BF16),
        bll=w['bll'], Wg1T=w['Wg1T'], bg1=w['bg1'], Wg2Tr=w['Wg2Tr'],
        bg2=w['bg2'], WoT=w['WoT'])
    in_maps = []
    for c in range(NCORE):
        m = dict(common)
        m['KQ'] = KQ[c]
        m['Vl'] = Vl[c]
        m['gidx'] = gidx[c]
        m['P'] = Ps[c]
        m['recipT'] = recipT[c]
        in_maps.append(m)
    return in_maps, S, w['bo']


def kernel(**inputs):
    from concourse.bass_utils import run_bass_kernel_spmd

    in_maps, S, bo = _make_in_maps(inputs)
    key = ('prog', S, bo)
    if key not in _CACHE:
        _CACHE[key] = _build_program(S, bo)
    nc = _CACHE[key]

    res = run_bass_kernel_spmd(nc, in_maps, list(range(NCORE)))
    global LAST_RESULT
    LAST_RESULT = res
    out = np.zeros((B, 1), np.float32)
    for c in range(NCORE):
        out[c * GB:(c + 1) * GB, 0] = res.results[c]['out8'].reshape(-1)
    return out


LAST_RESULT = None


# revision 19
# speedup vs baseline: 1.0871x; 1.0240x over previous
"""Trainium2 Bass kernel for nn_GAT_GraphSAGE (N=12000, E=192000, F=35, B=64).

Sharding: attention rows (softmax row i = K_new index) sharded 1500/core on
8 cores; one AllGather of post-attention h (bf16, 2 chunked pieces); SAGE
sharded by dst with a batched dma_gather of h[src] rows + one-hot-matmul
scatter; per-core global-max-pool + MLP head on that core's 8 graphs.

Key structure (v2):
- The whole K branch (Wk/conv-taps/Wl/1/sqrt(F)) and Wq fold on the host
  into one [F1,F1] matrix M (F1=36 with a ones row for biases):
  scores = x~ M x~^T.  The i-side projection KQT = M^T x~_loc^T is also
  host-computed, so the device never projects Q or K.
- All big matmuls run in bf16 at full PE rate, packed 2x2 into the 64x64
  tile grid (contraction 36 <= 64): scores split into two M-halves
  (PSUM col groups), with the j-parity on row groups (operands duplicated
  at partitions 64:100); the attn@V accumulation contracts raw x~ against
  exp(scores) split into two K-halves (row groups -> two PSUM banks), and
  the V projection (Wv augmented with a ones column that also carries the
  softmax denominator) is applied afterwards to the tiny [36,512] result.
  The same post-matmul also produces h in natural layout, so no PE-mode
  switches happen inside the attention loop.
- exp on ACT in [128,1536] batches (3 PSUM banks, double-buffered) is the
  bottleneck engine (~150us); everything else hides under it.
- SAGE scatter: one-hot P matrices are host-precomputed bf16 inputs (DMA,
  not DVE is_equal), gathered h rows are bf16 256B rows.
"""
import math
import numpy as np
import ml_dtypes

BF16 = ml_dtypes.bfloat16

N, E, F, B = 12000, 192000, 35, 64
F1 = F + 1
NCORE = 8
ROWS = N // NCORE            # 1500
ICH = 512
NI = 3
IPAD = ICH * NI              # 1536
JT = 94                      # j chunks of 128
JPAD = JT * 128              # 12032
XW = 12064                   # padded x~^T width (covers 7*1500 + 1536)
DBLK = 12                    # dst blocks (128 each) per core
GB = B // NCORE              # 8 graphs per core
HPAD = 128                   # h row padded to 128 bf16 (256B) for dma_gather
GRAPH_BOUNDS = [int(math.ceil(g * (N / B))) for g in range(GB + 1)]
# 3 AllGather pieces, one per attention i-chunk (local rows 512/512/476).
PLO = [0, 512, 1024]
PHI = [512, 1024, ROWS]
PLEN = [PHI[p] - PLO[p] for p in range(3)]
NP = 3


# --------------------------------------------------------------------------
# host-side preprocessing
# --------------------------------------------------------------------------

def _prep_weights(p):
    f64 = np.float64
    f32 = np.float32
    Wq, bq = p['Wq'].astype(f64), p['bq'].astype(f64)
    Wk, bk = p['Wk'].astype(f64), p['bk'].astype(f64)
    Wv, bv = p['Wv'].astype(f64), p['bv'].astype(f64)
    W3c, b3 = p['W3'][:, :, 1].astype(f64), p['b3'].astype(f64)
    W5c, b5 = p['W5'][:, :, 2].astype(f64), p['b5'].astype(f64)
    Wl, bl = p['Wl'].astype(f64), p['bl'].astype(f64)
    Wl1, Wl2, Wl3 = Wl[:, :F], Wl[:, F:2 * F], Wl[:, 2 * F:]

    # K_new = x~ @ Wkn~  (F1 -> F affine, includes 1/sqrt(F))
    Weff = W3c.T @ Wl1.T + W5c.T @ Wl2.T + Wl3.T
    beff = b3 @ Wl1.T + b5 @ Wl2.T + bl
    Wkn = Wk.T @ Weff
    bkn = bk @ Weff + beff
    s = 1.0 / np.sqrt(F)
    Wkn_aug = np.vstack([Wkn, bkn[None, :]]) * s          # [F1, F]
    Wq_aug = np.vstack([Wq.T, bq[None, :]])               # [F1, F]
    M = Wkn_aug @ Wq_aug.T                                # [F1, F1]

    Wva = np.zeros((F1, F1))
    Wva[:F, :F] = Wv.T
    Wva[F, :F] = bv
    Wva[F, F] = 1.0                                       # denominator column

    out = {'M': M, 'Wva': Wva.astype(BF16)}
    out['WllT'] = np.ascontiguousarray(p['Wll'].T).astype(f32)
    out['WlrT'] = np.ascontiguousarray(p['Wlr'].T).astype(f32)
    out['bll'] = p['bll'].astype(f32).reshape(F, 1)
    out['Wg1T'] = np.ascontiguousarray(p['Wg1'].T).astype(f32)   # [35,1500]
    bg1 = np.zeros((128, 12), f32)
    bg1.T.reshape(-1)[:1500] = p['bg1'].astype(f32)
    out['bg1'] = bg1
    w2 = np.zeros((12 * 128, 128), f32)
    w2[:1500, :] = p['Wg2'].T.astype(f32)
    out['Wg2Tr'] = np.ascontiguousarray(
        w2.reshape(12, 128, 128).transpose(1, 0, 2).reshape(128, 12 * 128))
    out['bg2'] = p['bg2'].astype(f32).reshape(128, 1)
    out['WoT'] = p['Wo'].astype(f32).reshape(1, 128).T.copy()     # [128,1]
    out['bo'] = float(np.asarray(p['bo']).reshape(-1)[0])
    return out


def _prep_x(x, M, Wva_unused):
    """Host: x~^T (bf16), x~ natural chunked (bf16), per-core KQT + Vl."""
    x64 = np.asarray(x, np.float64)
    xa = np.concatenate([x64, np.ones((N, 1))], axis=1)       # [N, F1]
    xaT = np.zeros((F1, XW))
    xaT[:, :N] = xa.T                                         # pad cols zero
    xh = xaT.astype(BF16)                                     # [F1, XW]

    # natural chunks for the U accumulation: [128, JT, F1]
    xn = np.zeros((128, JT, F1))
    flat = xaT[:, :JPAD].T                                    # [JPAD, F1]
    xn[:, :, :] = flat.reshape(JT, 128, F1).transpose(1, 0, 2)
    xn = np.ascontiguousarray(xn.reshape(128, JT * F1)).astype(BF16)

    KQ = []
    for c in range(NCORE):
        sl = xaT[:, c * ROWS: c * ROWS + IPAD]                # [F1, IPAD]
        KQ.append(np.ascontiguousarray(M.T @ sl).astype(BF16))
    return xh, xn, KQ


def _prep_vl(x, p):
    """Per-core natural V' local [128, DBLK*F] f32 (for the residual)."""
    f64 = np.float64
    Wv, bv = p['Wv'].astype(f64), p['bv'].astype(f64)
    x64 = np.asarray(x, np.float64)
    V = x64 @ Wv.T + bv                                       # [N, F]
    out = []
    for c in range(NCORE):
        vl = np.zeros((DBLK * 128, F))
        vl[:ROWS] = V[c * ROWS:(c + 1) * ROWS]
        out.append(np.ascontiguousarray(
            vl.reshape(DBLK, 128, F).transpose(1, 0, 2).reshape(128, DBLK * F)
        ).astype(np.float32))
    return out


def _prep_edges(edge_index):
    """Edges keyed (piece p of src, dst block b): chunk stream is p-major
    [p0: b0..b11][p1: ...][p2: ...] with S_bp (global max over cores)
    128-slot chunks per (p, b).  Returns per-core gather idx (relative to
    that piece's h_full_p tensor), one-hot P, recipT, and S_bp [NP][DBLK].
    """
    src = np.asarray(edge_index[0], np.int64)
    dst = np.asarray(edge_index[1], np.int64)
    deg = np.bincount(dst, minlength=N).astype(np.float64)
    recip = (1.0 / np.maximum(deg, 1.0)).astype(np.float32)

    core_of = dst // ROWS
    blk_of = (dst - core_of * ROWS) // 128
    sc = src // ROWS
    sr = src - sc * ROWS
    piece_of = np.where(sr < PLO[1], 0, np.where(sr < PLO[2], 1, 2))
    # position within piece p's gathered tensor [NCORE*PLEN[p], :]
    plen = np.array(PLEN)[piece_of]
    plo = np.array(PLO)[piece_of]
    pos = sc * plen + (sr - plo)

    counts = np.zeros((NCORE, NP, DBLK), np.int64)
    np.add.at(counts, (core_of, piece_of, blk_of), 1)
    S_bp = np.ceil(counts.max(axis=0) / 128).astype(np.int64)  # [NP, DBLK]
    CH = int(S_bp.sum())

    # chunk start offset for (p, b)
    ch_off = np.zeros((NP, DBLK), np.int64)
    acc = 0
    for p in range(NP):
        for b in range(DBLK):
            ch_off[p, b] = acc
            acc += S_bp[p, b]

    gidx, Ps = [], []
    for c in range(NCORE):
        idx_c = np.zeros(CH * 128, np.int16)
        rel_c = np.full(CH * 128, -1, np.int64)
        for p in range(NP):
            for b in range(DBLK):
                m = (core_of == c) & (piece_of == p) & (blk_of == b)
                n = int(m.sum())
                lo = int(ch_off[p, b]) * 128
                idx_c[lo:lo + n] = pos[m].astype(np.int16)
                rel_c[lo:lo + n] = dst[m] - c * ROWS - b * 128
        gidx.append(np.ascontiguousarray(
            np.tile(idx_c.reshape(-1, 16).T, (8, 1))))
        P = np.zeros((128, CH * 128), BF16)
        rel2 = rel_c.reshape(CH, 128)
        ch_i, e_i = np.nonzero(rel2 >= 0)
        P[e_i, ch_i * 128 + rel2[ch_i, e_i]] = 1
        Ps.append(np.ascontiguousarray(P))

    recipT = []
    for c in range(NCORE):
        r = np.ones(IPAD, np.float32)
        r[:ROWS] = recip[c * ROWS:(c + 1) * ROWS]
        recipT.append(np.ascontiguousarray(np.broadcast_to(r, (F, IPAD))))
    return gidx, Ps, recipT, tuple(int(v) for v in S_bp.reshape(-1))


# --------------------------------------------------------------------------
# device program
# --------------------------------------------------------------------------

def _emit_body(nc, tc, d, S_bp, bo_const):
    import concourse.tile as tile
    from concourse import mybir

    f32 = mybir.dt.float32
    bf16 = mybir.dt.bfloat16
    S_bp = [list(S_bp[p * DBLK:(p + 1) * DBLK]) for p in range(NP)]
    NCH = [int(sum(S_bp[p])) for p in range(NP)]       # chunks per piece
    CH = sum(NCH)
    POFF = [0, NCH[0], NCH[0] + NCH[1]]                # piece chunk offsets

    with tc.tile_pool(name="const", bufs=1) as constp, \
         tc.tile_pool(name="main", bufs=1) as main:
        # ---- inputs ----
        KQT = main.tile([128, IPAD], bf16, name="KQT")
        nc.sync.dma_start(out=KQT[0:F1, :], in_=d['KQ'][:, :])
        nc.sync.dma_start(out=KQT[64:64 + F1, :], in_=d['KQ'][:, :])
        xhT = main.tile([128, XW], bf16, name="xhT")
        HW = XW // 4
        for q in range(4):
            nc.sync.dma_start(out=xhT[0:F1, q * HW:(q + 1) * HW],
                              in_=d['xh'][:, q * HW:(q + 1) * HW])
            nc.sync.dma_start(out=xhT[64:64 + F1, q * HW:(q + 1) * HW],
                              in_=d['xh'][:, q * HW:(q + 1) * HW])
        xn = main.tile([128, JT * F1], bf16, name="xn")
        nc.sync.dma_start(out=xn[:], in_=d['xn'][:, :])
        Wva_t = constp.tile([F1, F1], bf16, name="Wva_t")
        nc.sync.dma_start(out=Wva_t[:], in_=d['Wva'][:, :])
        Vl = main.tile([128, DBLK * F], f32, name="Vl")
        nc.sync.dma_start(out=Vl[:], in_=d['Vl'][:, :])
        ident_t = constp.tile([128, 128], bf16, name="ident_t")
        nc.sync.dma_start(out=ident_t[:], in_=d['ident'][:, :])

        hnat = main.tile([128, DBLK, HPAD], bf16, name="hnat")
        nc.vector.memset(hnat[:, :, F:HPAD], 0.0)

        # ---------------- attention ----------------
        # groups of 3 j-chunks; one [128,1536] exp per group (double-buffered
        # PSUM). U' = sum_j x~_j^T exp[j,:] accumulated in two K-half chains
        # (row groups 0/64 -> banks C/D); V-projection applied after.
        GROUPS = [(g * 3, min(3, JT - g * 3)) for g in range((JT + 2) // 3)]
        exp_f = mybir.ActivationFunctionType.Exp
        with tc.tile_pool(name="mm1p", bufs=2, space="PSUM") as mm1p, \
             tc.tile_pool(name="Up", bufs=1, space="PSUM") as Upp, \
             tc.tile_pool(name="esb", bufs=3) as esb, \
             tc.tile_pool(name="usb", bufs=2) as usb, \
             tc.tile_pool(name="hsm", bufs=4) as hsmall:
            UC = Upp.tile([128, ICH], f32, name="UC")
            UD = Upp.tile([128, ICH], f32, name="UD")
            for ci in range(NI):
                prev = None
                for (j0, glen) in GROUPS:
                    ps = mm1p.tile([128, 3 * ICH], f32, space="PSUM",
                                   tag="s", name="pss")
                    for k in range(glen):
                        j = j0 + k
                        r = 64 * (j & 1)
                        for ch in range(2):
                            nc.tensor.matmul(
                                out=ps[64 * ch:64 * ch + 64,
                                       k * ICH:(k + 1) * ICH],
                                lhsT=xhT[r:r + F1,
                                         j * 128 + 64 * ch:
                                         j * 128 + 64 * ch + 64],
                                rhs=KQT[r:r + F1,
                                        ci * ICH:(ci + 1) * ICH],
                                start=True, stop=True)
                    et = esb.tile([128, 3 * ICH], bf16, tag="e", name="et")
                    nc.scalar.activation(out=et[:, :glen * ICH],
                                         in_=ps[:, :glen * ICH], func=exp_f)
                    if prev is not None:
                        pe, pj0, pglen = prev
                        for k in range(pglen):
                            j = pj0 + k
                            for r in range(2):
                                nc.tensor.matmul(
                                    out=(UC if r == 0 else UD)[0:F1, :],
                                    lhsT=xn[64 * r:64 * r + 64, j * F1:(j + 1) * F1],
                                    rhs=pe[64 * r:64 * r + 64,
                                           k * ICH:(k + 1) * ICH],
                                    start=(j == 0), stop=False,
                                    skip_group_check=True)
                    prev = (et, j0, glen)
                pe, pj0, pglen = prev
                for k in range(pglen):
                    j = pj0 + k
                    for r in range(2):
                        nc.tensor.matmul(
                            out=(UC if r == 0 else UD)[0:F1, :],
                            lhsT=xn[64 * r:64 * r + 64, j * F1:(j + 1) * F1],
                            rhs=pe[64 * r:64 * r + 64,
                                   k * ICH:(k + 1) * ICH],
                            start=False, stop=(k == pglen - 1),
                            skip_group_check=True)
                # combine K-halves -> U'sb bf16 [F1, 512]
                # (avoid a two-PSUM-operand tensor_tensor: copy then add)
                Ucs = usb.tile([F1, ICH], f32, tag="ucs", name="Ucs")
                nc.vector.tensor_copy(out=Ucs[:], in_=UC[0:F1, :])
                Usb = usb.tile([F1, ICH], bf16, tag="usb", name="Usb")
                nc.vector.tensor_add(out=Usb[:], in0=Ucs[:],
                                     in1=UD[0:F1, :])
                # h natural: hraw[i,g] = sum_f U'sb[f,i] Wva[f,g]
                # (two 64-col halves to stay in the 64x64 tile grid)
                for t in range(4):
                    blk = ci * 4 + t
                    for ch in range(2):
                        nc.tensor.matmul(
                            out=UD[64 * ch:64 * ch + 64,
                                   t * 128:t * 128 + F1],
                            lhsT=Usb[:, t * 128 + 64 * ch:
                                     t * 128 + 64 * ch + 64],
                            rhs=Wva_t[:],
                            start=True, stop=True, skip_group_check=True)
                    hraw = UD[:, t * 128:t * 128 + F1]
                    rec = hsmall.tile([128, 1], f32, tag="rec", name="rec")
                    nc.vector.reciprocal(out=rec[:], in_=hraw[:, F:F1])
                    hh = hsmall.tile([128, F], f32, tag="hh", name="hh")
                    nc.vector.scalar_tensor_tensor(
                        out=hh[:], in0=hraw[:, :F], scalar=rec[:],
                        in1=Vl[:, blk * F:(blk + 1) * F],
                        op0=mybir.AluOpType.mult,
                        op1=mybir.AluOpType.add)
                    nc.vector.tensor_scalar_max(out=hnat[:, blk, :F],
                                                in0=hh[:], scalar1=0.0)
                    lo = blk * 128
                    nrows = min(128, max(0, ROWS - lo))
                    if nrows > 0:
                        nc.sync.dma_start(
                            out=d['h_loc'][lo:lo + nrows, :],
                            in_=hnat[:nrows, blk, :])
                if ci < 2:
                    nc.gpsimd.collective_compute(
                        "AllGather", mybir.AluOpType.bypass,
                        replica_groups=[list(range(NCORE))],
                        ins=[d['h_loc'][PLO[ci]:PHI[ci], :]],
                        outs=[d['h_full%d' % ci][:, :]])

        # ---------------- SAGE scatter (+ deferred AG piece 2) -----------
        # GpSimd FIFO order matters: [AG0, AG1, gather0, AG2, gather1,
        # gather2] lets gather0 run while attention finishes without
        # blocking the AG2 trigger behind a 60us gather.
        aggS = main.tile([F, IPAD], f32, name="aggS")
        aggb = main.tile([F, IPAD], bf16, name="aggb")
        hT = main.tile([F, IPAD], bf16, name="hT")
        with tc.tile_pool(name="gat", bufs=2) as gat, \
             tc.tile_pool(name="pin", bufs=2) as pin, \
             tc.tile_pool(name="scp", bufs=3, space="PSUM") as scp, \
             tc.tile_pool(name="htp", bufs=2, space="PSUM") as htp, \
             tc.tile_pool(name="sin", bufs=1) as sin:
            idx_t = sin.tile([128, CH * 8], mybir.dt.int16, name="idx_t")
            nc.sync.dma_start(out=idx_t[:], in_=d['gidx'][:, :])
            recT_t = sin.tile([F, IPAD], f32, name="recT_t")
            nc.sync.dma_start(out=recT_t[:], in_=d['recipT'][:, :])

            GMAX = max(NCH)

            def piece(p):
                G = gat.tile([128, GMAX, HPAD], bf16, tag="G", name="G")
                nc.gpsimd.dma_gather(
                    out_ap=G[:, :NCH[p], :],
                    in_ap=d['h_full%d' % p][:, :],
                    idxs_ap=idx_t[:, POFF[p] * 8:(POFF[p] + NCH[p]) * 8],
                    num_idxs=NCH[p] * 128,
                    num_idxs_reg=NCH[p] * 128,
                    elem_size=HPAD,
                    single_packet=False)
                Pt = pin.tile([128, GMAX * 128], bf16, tag="P", name="Pt")
                nc.sync.dma_start(
                    out=Pt[:, :NCH[p] * 128],
                    in_=d['P'][:, POFF[p] * 128:(POFF[p] + NCH[p]) * 128])
                ch = 0
                for b in range(DBLK):
                    if S_bp[p][b] == 0:
                        if p == 0:
                            nc.vector.memset(
                                aggS[:, b * 128:(b + 1) * 128], 0.0)
                        continue
                    acc = scp.tile([F, 128], f32, space="PSUM", tag="agg",
                                   name="acc")
                    for s in range(S_bp[p][b]):
                        nc.tensor.matmul(
                            out=acc[:], lhsT=G[:, ch, :F],
                            rhs=Pt[:, ch * 128:(ch + 1) * 128],
                            start=(s == 0), stop=(s == S_bp[p][b] - 1),
                            skip_group_check=True)
                        ch += 1
                    sl = aggS[:, b * 128:(b + 1) * 128]
                    if p == 0:
                        nc.vector.tensor_copy(out=sl, in_=acc[:])
                    else:
                        nc.vector.tensor_add(out=sl, in0=sl, in1=acc[:])

            piece(0)
            nc.gpsimd.collective_compute(
                "AllGather", mybir.AluOpType.bypass,
                replica_groups=[list(range(NCORE))],
                ins=[d['h_loc'][PLO[2]:PHI[2], :]],
                outs=[d['h_full2'][:, :]])
            # hT (bf16) for SAGE lin_r: transpose the 12 h tiles (PE work
            # that fills the gap while gathers run on GpSimd)
            for t in range(DBLK):
                ps = htp.tile([F, 128], bf16, space="PSUM", tag="ht",
                              name="psht")
                nc.tensor.transpose(out=ps[:], in_=hnat[:, t, :F],
                                    identity=ident_t[:])
                nc.vector.tensor_copy(out=hT[:, t * 128:(t + 1) * 128],
                                      in_=ps[:])
            piece(1)
            piece(2)
            # normalize by degree -> bf16 for the SAGE linear
            nc.vector.tensor_mul(out=aggb[:], in0=aggS[:], in1=recT_t[:])

        # ---------------- SAGE linear + pool + MLP ----------------
        with tc.tile_pool(name="mlpw", bufs=1) as mlpw, \
             tc.tile_pool(name="mlps", bufs=2) as mlps, \
             tc.tile_pool(name="mlpp", bufs=2, space="PSUM") as mlpp:
            WllT_t = mlpw.tile([F, F], bf16, name="WllT_t")
            nc.sync.dma_start(out=WllT_t[:], in_=d['WllT'][:, :])
            WlrT_t = mlpw.tile([F, F], bf16, name="WlrT_t")
            nc.sync.dma_start(out=WlrT_t[:], in_=d['WlrT'][:, :])
            bll_t = mlpw.tile([F, 1], f32, name="bll_t")
            nc.sync.dma_start(out=bll_t[:], in_=d['bll'][:, :])
            Wg1T_t = mlpw.tile([F, 1500], f32, name="Wg1T_t")
            nc.sync.dma_start(out=Wg1T_t[:], in_=d['Wg1T'][:, :])
            bg1_t = mlpw.tile([128, 12], f32, name="bg1_t")
            nc.sync.dma_start(out=bg1_t[:], in_=d['bg1'][:, :])
            Wg2_t = mlpw.tile([128, 12 * 128], f32, name="Wg2_t")
            nc.sync.dma_start(out=Wg2_t[:], in_=d['Wg2Tr'][:, :])
            bg2_t = mlpw.tile([128, 1], f32, name="bg2_t")
            nc.sync.dma_start(out=bg2_t[:], in_=d['bg2'][:, :])
            WoT_t = mlpw.tile([128, 1], f32, name="WoT_t")
            nc.sync.dma_start(out=WoT_t[:], in_=d['WoT'][:, :])

            relu_f = mybir.ActivationFunctionType.Relu
            h2T = mlps.tile([F, IPAD], f32, tag="h2T", name="h2T")
            for ci in range(NI):
                ps = mlpp.tile([F, ICH], f32, space="PSUM", tag="h2",
                               name="psh2")
                nc.tensor.matmul(out=ps[:], lhsT=WllT_t[:],
                                 rhs=aggb[:, ci * ICH:(ci + 1) * ICH],
                                 start=True, stop=False,
                                 skip_group_check=True)
                nc.tensor.matmul(out=ps[:], lhsT=WlrT_t[:],
                                 rhs=hT[:, ci * ICH:(ci + 1) * ICH],
                                 start=False, stop=True,
                                 skip_group_check=True)
                nc.scalar.activation(out=h2T[:, ci * ICH:(ci + 1) * ICH],
                                     in_=ps[:], func=relu_f, bias=bll_t[:])

            gT = mlps.tile([F, GB], f32, tag="gT", name="gT")
            for g in range(GB):
                lo, hi = GRAPH_BOUNDS[g], GRAPH_BOUNDS[g + 1]
                nc.vector.tensor_reduce(out=gT[:, g:g + 1], in_=h2T[:, lo:hi],
                                        axis=mybir.AxisListType.X,
                                        op=mybir.AluOpType.max)
            g1T = mlps.tile([128, 12, GB], f32, tag="g1T", name="g1T")
            for j in range(12):
                w = min(128, 1500 - j * 128)
                ps = mlpp.tile([128, GB], f32, space="PSUM", tag="g1",
                               name="psg1")
                nc.tensor.matmul(out=ps[:w, :],
                                 lhsT=Wg1T_t[:, j * 128:j * 128 + w],
                                 rhs=gT[:], start=True, stop=True)
                if w < 128:
                    nc.vector.memset(g1T[:, j, :], 0.0)
                nc.scalar.activation(out=g1T[:w, j, :], in_=ps[:w, :],
                                     func=relu_f, bias=bg1_t[:w, j:j + 1])
            g2ps = mlpp.tile([128, GB], f32, space="PSUM", tag="g2",
                             name="g2ps")
            for j in range(12):
                nc.tensor.matmul(out=g2ps[:],
                                 lhsT=Wg2_t[:, j * 128:(j + 1) * 128],
                                 rhs=g1T[:, j, :], start=(j == 0),
                                 stop=(j == 11), skip_group_check=True)
            g2sb = mlps.tile([128, GB], f32, tag="g2sb", name="g2sb")
            nc.vector.tensor_scalar_add(out=g2sb[:], in0=g2ps[:],
                                        scalar1=bg2_t[:])
            ops = mlpp.tile([1, GB], f32, space="PSUM", tag="o", name="ops")
            nc.tensor.matmul(out=ops[:], lhsT=WoT_t[:], rhs=g2sb[:],
                             start=True, stop=True)
            osb = mlps.tile([1, GB], f32, tag="osb", name="osb")
            nc.vector.tensor_scalar_add(out=osb[:], in0=ops[:],
                                        scalar1=float(bo_const))
            nc.sync.dma_start(out=d['out8'][:, :], in_=osb[:])


def _build_program(S_bp, bo_const):
    import concourse.tile as tile
    from concourse import bacc, mybir

    f32 = mybir.dt.float32
    bf16 = mybir.dt.bfloat16
    CH = int(sum(S_bp))
    nc = bacc.Bacc("TRN2", target_bir_lowering=False, debug=False,
                   num_devices=NCORE)

    d = {}

    def dram_in(name, shape, dt=f32):
        d[name] = nc.dram_tensor(name, list(shape), dt, kind="ExternalInput")

    dram_in("xh", (F1, XW), bf16)
    dram_in("xn", (128, JT * F1), bf16)
    dram_in("KQ", (F1, IPAD), bf16)
    dram_in("Wva", (F1, F1), bf16)
    dram_in("Vl", (128, DBLK * F), f32)
    dram_in("ident", (128, 128), bf16)
    dram_in("WllT", (F, F), bf16)
    dram_in("WlrT", (F, F), bf16)
    dram_in("bll", (F, 1))
    dram_in("Wg1T", (F, 1500))
    dram_in("bg1", (128, 12))
    dram_in("Wg2Tr", (128, 12 * 128))
    dram_in("bg2", (128, 1))
    dram_in("WoT", (128, 1))
    dram_in("recipT", (F, IPAD))
    dram_in("P", (128, CH * 128), bf16)
    d['gidx'] = nc.dram_tensor("gidx", [128, CH * 8], mybir.dt.int16,
                               kind="ExternalInput")
    d['out8'] = nc.dram_tensor("out8", [1, GB], f32, kind="ExternalOutput")
    d['h_loc'] = nc.dram_tensor("h_loc", [ROWS, HPAD], bf16)
    for p in range(NP):
        d['h_full%d' % p] = nc.dram_tensor(
            "h_full%d" % p, [NCORE * PLEN[p], HPAD], bf16,
            addr_space="Shared")

    with tile.TileContext(nc) as tc:
        _emit_body(nc, tc, d, S_bp, bo_const)

    nc.compile()
    return nc


# --------------------------------------------------------------------------
# entry point
# --------------------------------------------------------------------------

_CACHE = {}


def _make_in_maps(inputs):
    x = np.asarray(inputs['x'], np.float32)
    edge_index = np.asarray(inputs['edge_index'])
    w = _prep_weights(inputs)
    xh, xn, KQ = _prep_x(x, w['M'], w['Wva'])
    Vl = _prep_vl(x, inputs)
    gidx, Ps, recipT, S_bp = _prep_edges(edge_index)
    ident = np.eye(128, dtype=BF16)
    common = dict(
        xh=xh, xn=xn, Wva=w['Wva'], ident=ident,
        WllT=w['WllT'].astype(BF16), WlrT=w['WlrT'].astype(BF16),
        bll=w['bll'], Wg1T=w['Wg1T'], bg1=w['bg1'], Wg2Tr=w['Wg2Tr'],
        bg2=w['bg2'], WoT=w['WoT'])
    in_maps = []
    for c in range(NCORE):
        m = dict(common)
        m['KQ'] = KQ[c]
        m['Vl'] = Vl[c]
        m['gidx'] = gidx[c]
        m['P'] = Ps[c]
        m['recipT'] = recipT[c]
        in_maps.append(m)
    return in_maps, S_bp, w['bo']


def kernel(**inputs):
    from concourse.bass_utils import run_bass_kernel_spmd

    in_maps, S_bp, bo = _make_in_maps(inputs)
    key = ('prog', S_bp, bo)
    if key not in _CACHE:
        _CACHE[key] = _build_program(S_bp, bo)
    nc = _CACHE[key]

    res = run_bass_kernel_spmd(nc, in_maps, list(range(NCORE)))
    global LAST_RESULT
    LAST_RESULT = res
    out = np.zeros((B, 1), np.float32)
    for c in range(NCORE):
        out[c * GB:(c + 1) * GB, 0] = res.results[c]['out8'].reshape(-1)
    return out


LAST_RESULT = None


# revision 20
# speedup vs baseline: 1.2384x; 1.1392x over previous
"""Trainium2 Bass kernel for nn_GAT_GraphSAGE (N=12000, E=192000, F=35, B=64).

Sharding: attention rows (softmax row i = K_new index) sharded 1500/core on
8 cores; one AllGather of post-attention h (bf16, 2 chunked pieces); SAGE
sharded by dst with a batched dma_gather of h[src] rows + one-hot-matmul
scatter; per-core global-max-pool + MLP head on that core's 8 graphs.

Key structure (v2):
- The whole K branch (Wk/conv-taps/Wl/1/sqrt(F)) and Wq fold on the host
  into one [F1,F1] matrix M (F1=36 with a ones row for biases):
  scores = x~ M x~^T.  The i-side projection KQT = M^T x~_loc^T is also
  host-computed, so the device never projects Q or K.
- All big matmuls run in bf16 at full PE rate, packed 2x2 into the 64x64
  tile grid (contraction 36 <= 64): scores split into two M-halves
  (PSUM col groups), with the j-parity on row groups (operands duplicated
  at partitions 64:100); the attn@V accumulation contracts raw x~ against
  exp(scores) split into two K-halves (row groups -> two PSUM banks), and
  the V projection (Wv augmented with a ones column that also carries the
  softmax denominator) is applied afterwards to the tiny [36,512] result.
  The same post-matmul also produces h in natural layout, so no PE-mode
  switches happen inside the attention loop.
- exp on ACT in [128,1536] batches (3 PSUM banks, double-buffered) is the
  bottleneck engine (~150us); everything else hides under it.
- SAGE scatter: one-hot P matrices are host-precomputed bf16 inputs (DMA,
  not DVE is_equal), gathered h rows are bf16 256B rows.
"""
import math
import numpy as np
import ml_dtypes

BF16 = ml_dtypes.bfloat16

N, E, F, B = 12000, 192000, 35, 64
F1 = F + 1
NCORE = 8
ROWS = N // NCORE            # 1500
ICH = 512
NI = 3
IPAD = ICH * NI              # 1536
JT = 94                      # j chunks of 128
JPAD = JT * 128              # 12032
XW = 12064                   # padded x~^T width (covers 7*1500 + 1536)
DBLK = 12                    # dst blocks (128 each) per core
GB = B // NCORE              # 8 graphs per core
HPAD = 128                   # h row padded to 128 bf16 (256B) for dma_gather
GRAPH_BOUNDS = [int(math.ceil(g * (N / B))) for g in range(GB + 1)]
# 3 AllGather pieces, one per attention i-chunk (local rows 512/512/476).
PLO = [0, 512, 1024]
PHI = [512, 1024, ROWS]
PLEN = [PHI[p] - PLO[p] for p in range(3)]
NP = 3


# --------------------------------------------------------------------------
# host-side preprocessing
# --------------------------------------------------------------------------

def _prep_weights(p):
    f64 = np.float64
    f32 = np.float32
    Wq, bq = p['Wq'].astype(f64), p['bq'].astype(f64)
    Wk, bk = p['Wk'].astype(f64), p['bk'].astype(f64)
    Wv, bv = p['Wv'].astype(f64), p['bv'].astype(f64)
    W3c, b3 = p['W3'][:, :, 1].astype(f64), p['b3'].astype(f64)
    W5c, b5 = p['W5'][:, :, 2].astype(f64), p['b5'].astype(f64)
    Wl, bl = p['Wl'].astype(f64), p['bl'].astype(f64)
    Wl1, Wl2, Wl3 = Wl[:, :F], Wl[:, F:2 * F], Wl[:, 2 * F:]

    # K_new = x~ @ Wkn~  (F1 -> F affine, includes 1/sqrt(F))
    Weff = W3c.T @ Wl1.T + W5c.T @ Wl2.T + Wl3.T
    beff = b3 @ Wl1.T + b5 @ Wl2.T + bl
    Wkn = Wk.T @ Weff
    bkn = bk @ Weff + beff
    s = 1.0 / np.sqrt(F)
    Wkn_aug = np.vstack([Wkn, bkn[None, :]]) * s          # [F1, F]
    Wq_aug = np.vstack([Wq.T, bq[None, :]])               # [F1, F]
    M = Wkn_aug @ Wq_aug.T                                # [F1, F1]

    Wva = np.zeros((F1, F1))
    Wva[:F, :F] = Wv.T
    Wva[F, :F] = bv
    Wva[F, F] = 1.0                                       # denominator column

    out = {'M': M, 'Wva': Wva.astype(BF16)}
    out['WllT'] = np.ascontiguousarray(p['Wll'].T).astype(f32)
    out['WlrT'] = np.ascontiguousarray(p['Wlr'].T).astype(f32)
    out['bll'] = p['bll'].astype(f32).reshape(F, 1)
    out['Wg1T'] = np.ascontiguousarray(p['Wg1'].T).astype(f32)   # [35,1500]
    bg1 = np.zeros((128, 12), f32)
    bg1.T.reshape(-1)[:1500] = p['bg1'].astype(f32)
    out['bg1'] = bg1
    w2 = np.zeros((12 * 128, 128), f32)
    w2[:1500, :] = p['Wg2'].T.astype(f32)
    out['Wg2Tr'] = np.ascontiguousarray(
        w2.reshape(12, 128, 128).transpose(1, 0, 2).reshape(128, 12 * 128))
    out['bg2'] = p['bg2'].astype(f32).reshape(128, 1)
    out['WoT'] = p['Wo'].astype(f32).reshape(1, 128).T.copy()     # [128,1]
    out['bo'] = float(np.asarray(p['bo']).reshape(-1)[0])
    return out


def _prep_x(x, M, Wva_unused):
    """Host: x~^T (bf16), x~ natural chunked (bf16), per-core KQT + Vl."""
    x64 = np.asarray(x, np.float64)
    xa = np.concatenate([x64, np.ones((N, 1))], axis=1)       # [N, F1]
    xaT = np.zeros((F1, XW))
    xaT[:, :N] = xa.T                                         # pad cols zero
    xh = xaT.astype(BF16)                                     # [F1, XW]

    # natural chunks for the U accumulation: [128, JT, F1]
    xn = np.zeros((128, JT, F1))
    flat = xaT[:, :JPAD].T                                    # [JPAD, F1]
    xn[:, :, :] = flat.reshape(JT, 128, F1).transpose(1, 0, 2)
    xn = np.ascontiguousarray(xn.reshape(128, JT * F1)).astype(BF16)

    KQ = []
    for c in range(NCORE):
        sl = xaT[:, c * ROWS: c * ROWS + IPAD]                # [F1, IPAD]
        KQ.append(np.ascontiguousarray(M.T @ sl).astype(BF16))
    return xh, xn, KQ


def _prep_vl(x, p):
    """Per-core natural V' local [128, DBLK*F] f32 (for the residual)."""
    f64 = np.float64
    Wv, bv = p['Wv'].astype(f64), p['bv'].astype(f64)
    x64 = np.asarray(x, np.float64)
    V = x64 @ Wv.T + bv                                       # [N, F]
    out = []
    for c in range(NCORE):
        vl = np.zeros((DBLK * 128, F))
        vl[:ROWS] = V[c * ROWS:(c + 1) * ROWS]
        out.append(np.ascontiguousarray(
            vl.reshape(DBLK, 128, F).transpose(1, 0, 2).reshape(128, DBLK * F)
        ).astype(np.float32))
    return out


def _prep_edges(edge_index):
    """Edges keyed (piece p of src, dst block b): chunk stream is p-major
    [p0: b0..b11][p1: ...][p2: ...] with S_bp (global max over cores)
    128-slot chunks per (p, b).  Returns per-core gather idx (relative to
    that piece's h_full_p tensor), one-hot P, recipT, and S_bp [NP][DBLK].
    """
    src = np.asarray(edge_index[0], np.int64)
    dst = np.asarray(edge_index[1], np.int64)
    deg = np.bincount(dst, minlength=N).astype(np.float64)
    recip = (1.0 / np.maximum(deg, 1.0)).astype(np.float32)

    core_of = dst // ROWS
    blk_of = (dst - core_of * ROWS) // 128
    sc = src // ROWS
    sr = src - sc * ROWS
    piece_of = np.where(sr < PLO[1], 0, np.where(sr < PLO[2], 1, 2))
    # position within piece p's gathered tensor [NCORE*PLEN[p], :]
    plen = np.array(PLEN)[piece_of]
    plo = np.array(PLO)[piece_of]
    pos = sc * plen + (sr - plo)

    counts = np.zeros((NCORE, NP, DBLK), np.int64)
    np.add.at(counts, (core_of, piece_of, blk_of), 1)
    S_bp = np.ceil(counts.max(axis=0) / 128).astype(np.int64)  # [NP, DBLK]
    CH = int(S_bp.sum())

    # chunk start offset for (p, b)
    ch_off = np.zeros((NP, DBLK), np.int64)
    acc = 0
    for p in range(NP):
        for b in range(DBLK):
            ch_off[p, b] = acc
            acc += S_bp[p, b]

    gidx, Ps = [], []
    for c in range(NCORE):
        idx_c = np.zeros(CH * 128, np.int16)
        rel_c = np.full(CH * 128, -1, np.int64)
        for p in range(NP):
            for b in range(DBLK):
                m = (core_of == c) & (piece_of == p) & (blk_of == b)
                n = int(m.sum())
                lo = int(ch_off[p, b]) * 128
                idx_c[lo:lo + n] = pos[m].astype(np.int16)
                rel_c[lo:lo + n] = dst[m] - c * ROWS - b * 128
        gidx.append(np.ascontiguousarray(
            np.tile(idx_c.reshape(-1, 16).T, (8, 1))))
        P = np.zeros((128, CH * 128), BF16)
        rel2 = rel_c.reshape(CH, 128)
        ch_i, e_i = np.nonzero(rel2 >= 0)
        P[e_i, ch_i * 128 + rel2[ch_i, e_i]] = 1
        Ps.append(np.ascontiguousarray(P))

    recipT = []
    for c in range(NCORE):
        r = np.ones(IPAD, np.float32)
        r[:ROWS] = recip[c * ROWS:(c + 1) * ROWS]
        recipT.append(np.ascontiguousarray(np.broadcast_to(r, (F, IPAD))))
    return gidx, Ps, recipT, tuple(int(v) for v in S_bp.reshape(-1))


# --------------------------------------------------------------------------
# device program
# --------------------------------------------------------------------------

def _emit_body(nc, tc, d, S_bp, bo_const):
    import concourse.tile as tile
    from concourse import mybir

    f32 = mybir.dt.float32
    bf16 = mybir.dt.bfloat16
    S_bp = [list(S_bp[p * DBLK:(p + 1) * DBLK]) for p in range(NP)]
    NCH = [int(sum(S_bp[p])) for p in range(NP)]       # chunks per piece
    CH = sum(NCH)
    POFF = [0, NCH[0], NCH[0] + NCH[1]]                # piece chunk offsets

    with tc.tile_pool(name="const", bufs=1) as constp, \
         tc.tile_pool(name="main", bufs=1) as main:
        # ---- inputs ----
        KQT = main.tile([128, IPAD], bf16, name="KQT")
        nc.sync.dma_start(out=KQT[0:F1, :], in_=d['KQ'][:, :])
        nc.sync.dma_start(out=KQT[64:64 + F1, :], in_=d['KQ'][:, :])
        xhT = main.tile([128, XW], bf16, name="xhT")
        HW = XW // 4
        for q in range(4):
            nc.sync.dma_start(out=xhT[0:F1, q * HW:(q + 1) * HW],
                              in_=d['xh'][:, q * HW:(q + 1) * HW])
            nc.sync.dma_start(out=xhT[64:64 + F1, q * HW:(q + 1) * HW],
                              in_=d['xh'][:, q * HW:(q + 1) * HW])
        xn = main.tile([128, JT * F1], bf16, name="xn")
        nc.sync.dma_start(out=xn[:], in_=d['xn'][:, :])
        Wva_t = constp.tile([F1, F1], bf16, name="Wva_t")
        nc.sync.dma_start(out=Wva_t[:], in_=d['Wva'][:, :])
        Vl = main.tile([128, DBLK * F], f32, name="Vl")
        nc.sync.dma_start(out=Vl[:], in_=d['Vl'][:, :])
        ident_t = constp.tile([128, 128], bf16, name="ident_t")
        nc.sync.dma_start(out=ident_t[:], in_=d['ident'][:, :])

        hnat = main.tile([128, DBLK, HPAD], bf16, name="hnat")
        nc.vector.memset(hnat[:, :, F:HPAD], 0.0)

        # ---------------- attention ----------------
        # groups of 3 j-chunks; one [128,1536] exp per group (double-buffered
        # PSUM). U' = sum_j x~_j^T exp[j,:] accumulated in two K-half chains
        # (row groups 0/64 -> banks C/D); V-projection applied after.
        GROUPS = [(g * 3, min(3, JT - g * 3)) for g in range((JT + 2) // 3)]
        exp_f = mybir.ActivationFunctionType.Exp
        with tc.tile_pool(name="mm1p", bufs=2, space="PSUM") as mm1p, \
             tc.tile_pool(name="Up", bufs=1, space="PSUM") as Upp, \
             tc.tile_pool(name="esb", bufs=3) as esb, \
             tc.tile_pool(name="usb", bufs=2) as usb, \
             tc.tile_pool(name="hsm", bufs=4) as hsmall:
            UC = Upp.tile([128, ICH], f32, name="UC")
            UD = Upp.tile([128, ICH], f32, name="UD")
            for ci in range(NI):
                prev = None
                for (j0, glen) in GROUPS:
                    ps = mm1p.tile([128, 3 * ICH], f32, space="PSUM",
                                   tag="s", name="pss")
                    for k in range(glen):
                        j = j0 + k
                        r = 64 * (j & 1)
                        for ch in range(2):
                            nc.tensor.matmul(
                                out=ps[64 * ch:64 * ch + 64,
                                       k * ICH:(k + 1) * ICH],
                                lhsT=xhT[r:r + F1,
                                         j * 128 + 64 * ch:
                                         j * 128 + 64 * ch + 64],
                                rhs=KQT[r:r + F1,
                                        ci * ICH:(ci + 1) * ICH],
                                start=True, stop=True)
                    et = esb.tile([128, 3 * ICH], bf16, tag="e", name="et")
                    nc.scalar.activation(out=et[:, :glen * ICH],
                                         in_=ps[:, :glen * ICH], func=exp_f)
                    if prev is not None:
                        pe, pj0, pglen = prev
                        for k in range(pglen):
                            j = pj0 + k
                            for r in range(2):
                                nc.tensor.matmul(
                                    out=(UC if r == 0 else UD)[0:F1, :],
                                    lhsT=xn[64 * r:64 * r + 64, j * F1:(j + 1) * F1],
                                    rhs=pe[64 * r:64 * r + 64,
                                           k * ICH:(k + 1) * ICH],
                                    start=(j == 0), stop=False,
                                    skip_group_check=True)
                    prev = (et, j0, glen)
                pe, pj0, pglen = prev
                for k in range(pglen):
                    j = pj0 + k
                    for r in range(2):
                        nc.tensor.matmul(
                            out=(UC if r == 0 else UD)[0:F1, :],
                            lhsT=xn[64 * r:64 * r + 64, j * F1:(j + 1) * F1],
                            rhs=pe[64 * r:64 * r + 64,
                                   k * ICH:(k + 1) * ICH],
                            start=False, stop=(k == pglen - 1),
                            skip_group_check=True)
                # combine K-halves -> U'sb bf16 [F1, 512]
                # (avoid a two-PSUM-operand tensor_tensor: copy then add)
                Ucs = usb.tile([F1, ICH], f32, tag="ucs", name="Ucs")
                nc.vector.tensor_copy(out=Ucs[:], in_=UC[0:F1, :])
                Usb = usb.tile([F1, ICH], bf16, tag="usb", name="Usb")
                nc.vector.tensor_add(out=Usb[:], in0=Ucs[:],
                                     in1=UD[0:F1, :])
                # h natural: hraw[i,g] = sum_f U'sb[f,i] Wva[f,g]
                # (two 64-col halves to stay in the 64x64 tile grid)
                for t in range(4):
                    blk = ci * 4 + t
                    for ch in range(2):
                        nc.tensor.matmul(
                            out=UD[64 * ch:64 * ch + 64,
                                   t * 128:t * 128 + F1],
                            lhsT=Usb[:, t * 128 + 64 * ch:
                                     t * 128 + 64 * ch + 64],
                            rhs=Wva_t[:],
                            start=True, stop=True, skip_group_check=True)
                    hraw = UD[:, t * 128:t * 128 + F1]
                    rec = hsmall.tile([128, 1], f32, tag="rec", name="rec")
                    nc.vector.reciprocal(out=rec[:], in_=hraw[:, F:F1])
                    hh = hsmall.tile([128, F], f32, tag="hh", name="hh")
                    nc.vector.scalar_tensor_tensor(
                        out=hh[:], in0=hraw[:, :F], scalar=rec[:],
                        in1=Vl[:, blk * F:(blk + 1) * F],
                        op0=mybir.AluOpType.mult,
                        op1=mybir.AluOpType.add)
                    nc.vector.tensor_scalar_max(out=hnat[:, blk, :F],
                                                in0=hh[:], scalar1=0.0)
                    lo = blk * 128
                    nrows = min(128, max(0, ROWS - lo))
                    if nrows > 0:
                        nc.sync.dma_start(
                            out=d['h_loc'][lo:lo + nrows, :],
                            in_=hnat[:nrows, blk, :])
                if ci < 2:
                    nc.gpsimd.collective_compute(
                        "AllGather", mybir.AluOpType.bypass,
                        replica_groups=[list(range(NCORE))],
                        ins=[d['h_loc'][PLO[ci]:PHI[ci], :]],
                        outs=[d['h_full%d' % ci][:, :]])

        # ---------------- SAGE scatter (+ deferred AG piece 2) -----------
        # GpSimd FIFO order matters: [AG0, AG1, gather0, AG2, gather1,
        # gather2] lets gather0 run while attention finishes without
        # blocking the AG2 trigger behind a 60us gather.
        aggS = main.tile([F, IPAD], f32, name="aggS")
        aggb = main.tile([F, IPAD], bf16, name="aggb")
        hT = main.tile([F, IPAD], bf16, name="hT")
        with tc.tile_pool(name="gat", bufs=2) as gat, \
             tc.tile_pool(name="pin", bufs=2) as pin, \
             tc.tile_pool(name="scp", bufs=3, space="PSUM") as scp, \
             tc.tile_pool(name="htp", bufs=2, space="PSUM") as htp, \
             tc.tile_pool(name="sin", bufs=1) as sin:
            idx_t = sin.tile([128, CH * 8], mybir.dt.int16, name="idx_t")
            nc.sync.dma_start(out=idx_t[:], in_=d['gidx'][:, :])
            recT_t = sin.tile([F, IPAD], f32, name="recT_t")
            nc.sync.dma_start(out=recT_t[:], in_=d['recipT'][:, :])

            GMAX = max(NCH)
            from concourse.tile import add_dep_helper
            gp_chain = []        # enforce GpSimd FIFO order across gathers/AG

            def piece(p):
                # two gather calls per piece: ~4.6K idxs each stays within
                # the SWDGE ring so calls pipeline back-to-back
                G = gat.tile([128, GMAX, HPAD], bf16, tag="G", name="G")
                h0 = NCH[p] // 2
                for (c0, c1) in ((0, h0), (h0, NCH[p])):
                    gi = nc.gpsimd.dma_gather(
                        out_ap=G[:, c0:c1, :],
                        in_ap=d['h_full%d' % p][:, :],
                        idxs_ap=idx_t[:, (POFF[p] + c0) * 8:
                                      (POFF[p] + c1) * 8],
                        num_idxs=(c1 - c0) * 128,
                        num_idxs_reg=(c1 - c0) * 128,
                        elem_size=HPAD,
                        single_packet=False)
                    if gp_chain:
                        add_dep_helper(gi.ins, gp_chain[-1],
                                       reason="gpsimd fifo order")
                    gp_chain.append(gi.ins)
                Pt = pin.tile([128, GMAX * 128], bf16, tag="P", name="Pt")
                nc.sync.dma_start(
                    out=Pt[:, :NCH[p] * 128],
                    in_=d['P'][:, POFF[p] * 128:(POFF[p] + NCH[p]) * 128])
                ch = 0
                for b in range(DBLK):
                    if S_bp[p][b] == 0:
                        if p == 0:
                            nc.vector.memset(
                                aggS[:, b * 128:(b + 1) * 128], 0.0)
                        continue
                    acc = scp.tile([F, 128], f32, space="PSUM", tag="agg",
                                   name="acc")
                    for s in range(S_bp[p][b]):
                        nc.tensor.matmul(
                            out=acc[:], lhsT=G[:, ch, :F],
                            rhs=Pt[:, ch * 128:(ch + 1) * 128],
                            start=(s == 0), stop=(s == S_bp[p][b] - 1),
                            skip_group_check=True)
                        ch += 1
                    sl = aggS[:, b * 128:(b + 1) * 128]
                    if p == 0:
                        nc.vector.tensor_copy(out=sl, in_=acc[:])
                    else:
                        nc.vector.tensor_add(out=sl, in0=sl, in1=acc[:])

            piece(0)
            ag2 = nc.gpsimd.collective_compute(
                "AllGather", mybir.AluOpType.bypass,
                replica_groups=[list(range(NCORE))],
                ins=[d['h_loc'][PLO[2]:PHI[2], :]],
                outs=[d['h_full2'][:, :]])
            add_dep_helper(ag2.ins, gp_chain[-1], reason="gpsimd fifo order")
            gp_chain.append(ag2.ins)
            # hT (bf16) for SAGE lin_r: transpose the 12 h tiles (PE work
            # that fills the gap while gathers run on GpSimd)
            for t in range(DBLK):
                ps = htp.tile([F, 128], bf16, space="PSUM", tag="ht",
                              name="psht")
                nc.tensor.transpose(out=ps[:], in_=hnat[:, t, :F],
                                    identity=ident_t[:])
                nc.vector.tensor_copy(out=hT[:, t * 128:(t + 1) * 128],
                                      in_=ps[:])
            piece(1)
            piece(2)
            # normalize by degree -> bf16 for the SAGE linear
            nc.vector.tensor_mul(out=aggb[:], in0=aggS[:], in1=recT_t[:])

        # ---------------- SAGE linear + pool + MLP ----------------
        with tc.tile_pool(name="mlpw", bufs=1) as mlpw, \
             tc.tile_pool(name="mlps", bufs=2) as mlps, \
             tc.tile_pool(name="mlpp", bufs=2, space="PSUM") as mlpp:
            WllT_t = mlpw.tile([F, F], bf16, name="WllT_t")
            nc.sync.dma_start(out=WllT_t[:], in_=d['WllT'][:, :])
            WlrT_t = mlpw.tile([F, F], bf16, name="WlrT_t")
            nc.sync.dma_start(out=WlrT_t[:], in_=d['WlrT'][:, :])
            bll_t = mlpw.tile([F, 1], f32, name="bll_t")
            nc.sync.dma_start(out=bll_t[:], in_=d['bll'][:, :])
            Wg1T_t = mlpw.tile([F, 1500], f32, name="Wg1T_t")
            nc.sync.dma_start(out=Wg1T_t[:], in_=d['Wg1T'][:, :])
            bg1_t = mlpw.tile([128, 12], f32, name="bg1_t")
            nc.sync.dma_start(out=bg1_t[:], in_=d['bg1'][:, :])
            Wg2_t = mlpw.tile([128, 12 * 128], f32, name="Wg2_t")
            nc.sync.dma_start(out=Wg2_t[:], in_=d['Wg2Tr'][:, :])
            bg2_t = mlpw.tile([128, 1], f32, name="bg2_t")
            nc.sync.dma_start(out=bg2_t[:], in_=d['bg2'][:, :])
            WoT_t = mlpw.tile([128, 1], f32, name="WoT_t")
            nc.sync.dma_start(out=WoT_t[:], in_=d['WoT'][:, :])

            relu_f = mybir.ActivationFunctionType.Relu
            h2T = mlps.tile([F, IPAD], f32, tag="h2T", name="h2T")
            for ci in range(NI):
                ps = mlpp.tile([F, ICH], f32, space="PSUM", tag="h2",
                               name="psh2")
                nc.tensor.matmul(out=ps[:], lhsT=WllT_t[:],
                                 rhs=aggb[:, ci * ICH:(ci + 1) * ICH],
                                 start=True, stop=False,
                                 skip_group_check=True)
                nc.tensor.matmul(out=ps[:], lhsT=WlrT_t[:],
                                 rhs=hT[:, ci * ICH:(ci + 1) * ICH],
                                 start=False, stop=True,
                                 skip_group_check=True)
                nc.scalar.activation(out=h2T[:, ci * ICH:(ci + 1) * ICH],
                                     in_=ps[:], func=relu_f, bias=bll_t[:])

            gT = mlps.tile([F, GB], f32, tag="gT", name="gT")
            for g in range(GB):
                lo, hi = GRAPH_BOUNDS[g], GRAPH_BOUNDS[g + 1]
                nc.vector.tensor_reduce(out=gT[:, g:g + 1], in_=h2T[:, lo:hi],
                                        axis=mybir.AxisListType.X,
                                        op=mybir.AluOpType.max)
            g1T = mlps.tile([128, 12, GB], f32, tag="g1T", name="g1T")
            for j in range(12):
                w = min(128, 1500 - j * 128)
                ps = mlpp.tile([128, GB], f32, space="PSUM", tag="g1",
                               name="psg1")
                nc.tensor.matmul(out=ps[:w, :],
                                 lhsT=Wg1T_t[:, j * 128:j * 128 + w],
                                 rhs=gT[:], start=True, stop=True)
                if w < 128:
                    nc.vector.memset(g1T[:, j, :], 0.0)
                nc.scalar.activation(out=g1T[:w, j, :], in_=ps[:w, :],
                                     func=relu_f, bias=bg1_t[:w, j:j + 1])
            g2ps = mlpp.tile([128, GB], f32, space="PSUM", tag="g2",
                             name="g2ps")
            for j in range(12):
                nc.tensor.matmul(out=g2ps[:],
                                 lhsT=Wg2_t[:, j * 128:(j + 1) * 128],
                                 rhs=g1T[:, j, :], start=(j == 0),
                                 stop=(j == 11), skip_group_check=True)
            g2sb = mlps.tile([128, GB], f32, tag="g2sb", name="g2sb")
            nc.vector.tensor_scalar_add(out=g2sb[:], in0=g2ps[:],
                                        scalar1=bg2_t[:])
            ops = mlpp.tile([1, GB], f32, space="PSUM", tag="o", name="ops")
            nc.tensor.matmul(out=ops[:], lhsT=WoT_t[:], rhs=g2sb[:],
                             start=True, stop=True)
            osb = mlps.tile([1, GB], f32, tag="osb", name="osb")
            nc.vector.tensor_scalar_add(out=osb[:], in0=ops[:],
                                        scalar1=float(bo_const))
            nc.sync.dma_start(out=d['out8'][:, :], in_=osb[:])


def _build_program(S_bp, bo_const):
    import concourse.tile as tile
    from concourse import bacc, mybir

    f32 = mybir.dt.float32
    bf16 = mybir.dt.bfloat16
    CH = int(sum(S_bp))
    nc = bacc.Bacc("TRN2", target_bir_lowering=False, debug=False,
                   num_devices=NCORE)

    d = {}

    def dram_in(name, shape, dt=f32):
        d[name] = nc.dram_tensor(name, list(shape), dt, kind="ExternalInput")

    dram_in("xh", (F1, XW), bf16)
    dram_in("xn", (128, JT * F1), bf16)
    dram_in("KQ", (F1, IPAD), bf16)
    dram_in("Wva", (F1, F1), bf16)
    dram_in("Vl", (128, DBLK * F), f32)
    dram_in("ident", (128, 128), bf16)
    dram_in("WllT", (F, F), bf16)
    dram_in("WlrT", (F, F), bf16)
    dram_in("bll", (F, 1))
    dram_in("Wg1T", (F, 1500))
    dram_in("bg1", (128, 12))
    dram_in("Wg2Tr", (128, 12 * 128))
    dram_in("bg2", (128, 1))
    dram_in("WoT", (128, 1))
    dram_in("recipT", (F, IPAD))
    dram_in("P", (128, CH * 128), bf16)
    d['gidx'] = nc.dram_tensor("gidx", [128, CH * 8], mybir.dt.int16,
                               kind="ExternalInput")
    d['out8'] = nc.dram_tensor("out8", [1, GB], f32, kind="ExternalOutput")
    d['h_loc'] = nc.dram_tensor("h_loc", [ROWS, HPAD], bf16)
    for p in range(NP):
        d['h_full%d' % p] = nc.dram_tensor(
            "h_full%d" % p, [NCORE * PLEN[p], HPAD], bf16,
            addr_space="Shared")

    with tile.TileContext(nc) as tc:
        _emit_body(nc, tc, d, S_bp, bo_const)

    nc.compile()
    return nc


# --------------------------------------------------------------------------
# entry point
# --------------------------------------------------------------------------

_CACHE = {}


def _make_in_maps(inputs):
    x = np.asarray(inputs['x'], np.float32)
    edge_index = np.asarray(inputs['edge_index'])
    w = _prep_weights(inputs)
    xh, xn, KQ = _prep_x(x, w['M'], w['Wva'])
    Vl = _prep_vl(x, inputs)
    gidx, Ps, recipT, S_bp = _prep_edges(edge_index)
    ident = np.eye(128, dtype=BF16)
    common = dict(
        xh=xh, xn=xn, Wva=w['Wva'], ident=ident,
        WllT=w['WllT'].astype(BF16), WlrT=w['WlrT'].astype(BF16),
        bll=w['bll'], Wg1T=w['Wg1T'], bg1=w['bg1'], Wg2Tr=w['Wg2Tr'],
        bg2=w['bg2'], WoT=w['WoT'])
    in_maps = []
    for c in range(NCORE):
        m = dict(common)
        m['KQ'] = KQ[c]
        m['Vl'] = Vl[c]
        m['gidx'] = gidx[c]
        m['P'] = Ps[c]
        m['recipT'] = recipT[c]
        in_maps.append(m)
    return in_maps, S_bp, w['bo']


def kernel(**inputs):
    from concourse.bass_utils import run_bass_kernel_spmd

    in_maps, S_bp, bo = _make_in_maps(inputs)
    key = ('prog', S_bp, bo)
    if key not in _CACHE:
        _CACHE[key] = _build_program(S_bp, bo)
    nc = _CACHE[key]

    res = run_bass_kernel_spmd(nc, in_maps, list(range(NCORE)))
    global LAST_RESULT
    LAST_RESULT = res
    out = np.zeros((B, 1), np.float32)
    for c in range(NCORE):
        out[c * GB:(c + 1) * GB, 0] = res.results[c]['out8'].reshape(-1)
    return out


LAST_RESULT = None


# revision 22
# speedup vs baseline: 1.6856x; 1.3612x over previous
"""Trainium2 Bass kernel for nn_GAT_GraphSAGE (N=12000, E=192000, F=35, B=64).

Sharding: attention rows (softmax row i = K_new index) sharded 1500/core on
8 cores; one AllGather of post-attention h (bf16, 2 chunked pieces); SAGE
sharded by dst with a batched dma_gather of h[src] rows + one-hot-matmul
scatter; per-core global-max-pool + MLP head on that core's 8 graphs.

Key structure (v2):
- The whole K branch (Wk/conv-taps/Wl/1/sqrt(F)) and Wq fold on the host
  into one [F1,F1] matrix M (F1=36 with a ones row for biases):
  scores = x~ M x~^T.  The i-side projection KQT = M^T x~_loc^T is also
  host-computed, so the device never projects Q or K.
- All big matmuls run in bf16 at full PE rate, packed 2x2 into the 64x64
  tile grid (contraction 36 <= 64): scores split into two M-halves
  (PSUM col groups), with the j-parity on row groups (operands duplicated
  at partitions 64:100); the attn@V accumulation contracts raw x~ against
  exp(scores) split into two K-halves (row groups -> two PSUM banks), and
  the V projection (Wv augmented with a ones column that also carries the
  softmax denominator) is applied afterwards to the tiny [36,512] result.
  The same post-matmul also produces h in natural layout, so no PE-mode
  switches happen inside the attention loop.
- exp on ACT in [128,1536] batches (3 PSUM banks, double-buffered) is the
  bottleneck engine (~150us); everything else hides under it.
- SAGE scatter: one-hot P matrices are host-precomputed bf16 inputs (DMA,
  not DVE is_equal), gathered h rows are bf16 256B rows.
"""
import math
import numpy as np
import ml_dtypes

BF16 = ml_dtypes.bfloat16

N, E, F, B = 12000, 192000, 35, 64
F1 = F + 1
NCORE = 8
ROWS = N // NCORE            # 1500
ICH = 512
NI = 3
IPAD = ICH * NI              # 1536
JT = 94                      # j chunks of 128
JPAD = JT * 128              # 12032
XW = 12064                   # padded x~^T width (covers 7*1500 + 1536)
DBLK = 12                    # dst blocks (128 each) per core
GB = B // NCORE              # 8 graphs per core
HPAD = 128                   # h row padded to 128 bf16 (256B) for dma_gather
GRAPH_BOUNDS = [int(math.ceil(g * (N / B))) for g in range(GB + 1)]
# 3 AllGather pieces, one per attention i-chunk (local rows 512/512/476).
PLO = [0, 512, 1024]
PHI = [512, 1024, ROWS]
PLEN = [PHI[p] - PLO[p] for p in range(3)]
NP = 3


# --------------------------------------------------------------------------
# host-side preprocessing
# --------------------------------------------------------------------------

def _prep_weights(p):
    f64 = np.float64
    f32 = np.float32
    Wq, bq = p['Wq'].astype(f64), p['bq'].astype(f64)
    Wk, bk = p['Wk'].astype(f64), p['bk'].astype(f64)
    Wv, bv = p['Wv'].astype(f64), p['bv'].astype(f64)
    W3c, b3 = p['W3'][:, :, 1].astype(f64), p['b3'].astype(f64)
    W5c, b5 = p['W5'][:, :, 2].astype(f64), p['b5'].astype(f64)
    Wl, bl = p['Wl'].astype(f64), p['bl'].astype(f64)
    Wl1, Wl2, Wl3 = Wl[:, :F], Wl[:, F:2 * F], Wl[:, 2 * F:]

    # K_new = x~ @ Wkn~  (F1 -> F affine, includes 1/sqrt(F))
    Weff = W3c.T @ Wl1.T + W5c.T @ Wl2.T + Wl3.T
    beff = b3 @ Wl1.T + b5 @ Wl2.T + bl
    Wkn = Wk.T @ Weff
    bkn = bk @ Weff + beff
    s = 1.0 / np.sqrt(F)
    Wkn_aug = np.vstack([Wkn, bkn[None, :]]) * s          # [F1, F]
    Wq_aug = np.vstack([Wq.T, bq[None, :]])               # [F1, F]
    M = Wkn_aug @ Wq_aug.T                                # [F1, F1]

    Wva = np.zeros((F1, F1))
    Wva[:F, :F] = Wv.T
    Wva[F, :F] = bv
    Wva[F, F] = 1.0                                       # denominator column

    out = {'M': M, 'Wva': Wva.astype(BF16)}
    out['WllT'] = np.ascontiguousarray(p['Wll'].T).astype(f32)
    out['WlrT'] = np.ascontiguousarray(p['Wlr'].T).astype(f32)
    out['bll'] = p['bll'].astype(f32).reshape(F, 1)
    out['Wg1T'] = np.ascontiguousarray(p['Wg1'].T).astype(f32)   # [35,1500]
    bg1 = np.zeros((128, 12), f32)
    bg1.T.reshape(-1)[:1500] = p['bg1'].astype(f32)
    out['bg1'] = bg1
    w2 = np.zeros((12 * 128, 128), f32)
    w2[:1500, :] = p['Wg2'].T.astype(f32)
    out['Wg2Tr'] = np.ascontiguousarray(
        w2.reshape(12, 128, 128).transpose(1, 0, 2).reshape(128, 12 * 128))
    out['bg2'] = p['bg2'].astype(f32).reshape(128, 1)
    out['WoT'] = p['Wo'].astype(f32).reshape(1, 128).T.copy()     # [128,1]
    out['bo'] = float(np.asarray(p['bo']).reshape(-1)[0])
    return out


def _prep_x(x, M, Wva_unused):
    """Host: x~^T (bf16), x~ natural chunked (bf16), per-core KQT + Vl."""
    x64 = np.asarray(x, np.float64)
    xa = np.concatenate([x64, np.ones((N, 1))], axis=1)       # [N, F1]
    xaT = np.zeros((F1, XW))
    xaT[:, :N] = xa.T                                         # pad cols zero
    xh = xaT.astype(BF16)                                     # [F1, XW]

    # natural chunks for the U accumulation: [128, JT, F1]
    xn = np.zeros((128, JT, F1))
    flat = xaT[:, :JPAD].T                                    # [JPAD, F1]
    xn[:, :, :] = flat.reshape(JT, 128, F1).transpose(1, 0, 2)
    xn = np.ascontiguousarray(xn.reshape(128, JT * F1)).astype(BF16)

    KQ = []
    for c in range(NCORE):
        sl = xaT[:, c * ROWS: c * ROWS + IPAD]                # [F1, IPAD]
        KQ.append(np.ascontiguousarray(M.T @ sl).astype(BF16))
    return xh, xn, KQ


def _prep_vl(x, p):
    """Per-core natural V' local [128, DBLK*F] f32 (for the residual)."""
    f64 = np.float64
    Wv, bv = p['Wv'].astype(f64), p['bv'].astype(f64)
    x64 = np.asarray(x, np.float64)
    V = x64 @ Wv.T + bv                                       # [N, F]
    out = []
    for c in range(NCORE):
        vl = np.zeros((DBLK * 128, F))
        vl[:ROWS] = V[c * ROWS:(c + 1) * ROWS]
        out.append(np.ascontiguousarray(
            vl.reshape(DBLK, 128, F).transpose(1, 0, 2).reshape(128, DBLK * F)
        ).astype(np.float32))
    return out


def _prep_edges(edge_index):
    """Edges keyed (piece p of src, dst block b): chunk stream is p-major
    [p0: b0..b11][p1: ...][p2: ...] with S_bp (global max over cores)
    128-slot chunks per (p, b).  Returns per-core gather idx (relative to
    that piece's h_full_p tensor), one-hot P, recipT, and S_bp [NP][DBLK].
    """
    src = np.asarray(edge_index[0], np.int64)
    dst = np.asarray(edge_index[1], np.int64)
    deg = np.bincount(dst, minlength=N).astype(np.float64)
    recip = (1.0 / np.maximum(deg, 1.0)).astype(np.float32)

    core_of = dst // ROWS
    blk_of = (dst - core_of * ROWS) // 128
    sc = src // ROWS
    sr = src - sc * ROWS
    piece_of = np.where(sr < PLO[1], 0, np.where(sr < PLO[2], 1, 2))
    # position within piece p's gathered tensor [NCORE*PLEN[p], :]
    plen = np.array(PLEN)[piece_of]
    plo = np.array(PLO)[piece_of]
    pos = sc * plen + (sr - plo)

    counts = np.zeros((NCORE, NP, DBLK), np.int64)
    np.add.at(counts, (core_of, piece_of, blk_of), 1)
    S_bp = np.ceil(counts.max(axis=0) / 128).astype(np.int64)  # [NP, DBLK]
    CH = int(S_bp.sum())

    # chunk start offset for (p, b)
    ch_off = np.zeros((NP, DBLK), np.int64)
    acc = 0
    for p in range(NP):
        for b in range(DBLK):
            ch_off[p, b] = acc
            acc += S_bp[p, b]

    gidx, Ps = [], []
    for c in range(NCORE):
        idx_c = np.zeros(CH * 128, np.int16)
        rel_c = np.full(CH * 128, -1, np.int64)
        for p in range(NP):
            for b in range(DBLK):
                m = (core_of == c) & (piece_of == p) & (blk_of == b)
                n = int(m.sum())
                lo = int(ch_off[p, b]) * 128
                idx_c[lo:lo + n] = pos[m].astype(np.int16)
                rel_c[lo:lo + n] = dst[m] - c * ROWS - b * 128
        gidx.append(np.ascontiguousarray(
            np.tile(idx_c.reshape(-1, 16).T, (8, 1))))
        P = np.zeros((128, CH * 128), BF16)
        rel2 = rel_c.reshape(CH, 128)
        ch_i, e_i = np.nonzero(rel2 >= 0)
        P[e_i, ch_i * 128 + rel2[ch_i, e_i]] = 1
        Ps.append(np.ascontiguousarray(P))

    recipT = []
    for c in range(NCORE):
        r = np.ones(IPAD, np.float32)
        r[:ROWS] = recip[c * ROWS:(c + 1) * ROWS]
        recipT.append(np.ascontiguousarray(np.broadcast_to(r, (F, IPAD))))
    return gidx, Ps, recipT, tuple(int(v) for v in S_bp.reshape(-1))


# --------------------------------------------------------------------------
# device program
# --------------------------------------------------------------------------

def _emit_body(nc, tc, d, S_bp, bo_const):
    import concourse.tile as tile
    from concourse import mybir

    f32 = mybir.dt.float32
    bf16 = mybir.dt.bfloat16
    S_bp = [list(S_bp[p * DBLK:(p + 1) * DBLK]) for p in range(NP)]
    NCH = [int(sum(S_bp[p])) for p in range(NP)]       # chunks per piece
    CH = sum(NCH)
    POFF = [0, NCH[0], NCH[0] + NCH[1]]                # piece chunk offsets

    with tc.tile_pool(name="const", bufs=1) as constp, \
         tc.tile_pool(name="main", bufs=1) as main, \
         tc.tile_pool(name="gat", bufs=4) as gat, \
         tc.tile_pool(name="pin", bufs=2) as pin, \
         tc.tile_pool(name="sin", bufs=1) as sin:
        # ---- inputs ----
        KQT = main.tile([128, IPAD], bf16, name="KQT")
        nc.sync.dma_start(out=KQT[0:F1, :], in_=d['KQ'][:, :])
        nc.sync.dma_start(out=KQT[64:64 + F1, :], in_=d['KQ'][:, :])
        xhT = main.tile([128, XW], bf16, name="xhT")
        HW = XW // 4
        for q in range(4):
            nc.sync.dma_start(out=xhT[0:F1, q * HW:(q + 1) * HW],
                              in_=d['xh'][:, q * HW:(q + 1) * HW])
            nc.sync.dma_start(out=xhT[64:64 + F1, q * HW:(q + 1) * HW],
                              in_=d['xh'][:, q * HW:(q + 1) * HW])
        xn = main.tile([128, JT * F1], bf16, name="xn")
        nc.sync.dma_start(out=xn[:], in_=d['xn'][:, :])
        Wva_t = constp.tile([F1, F1], bf16, name="Wva_t")
        nc.sync.dma_start(out=Wva_t[:], in_=d['Wva'][:, :])
        Vl = main.tile([128, DBLK * F], f32, name="Vl")
        nc.sync.dma_start(out=Vl[:], in_=d['Vl'][:, :])
        ident_t = constp.tile([128, 128], bf16, name="ident_t")
        nc.sync.dma_start(out=ident_t[:], in_=d['ident'][:, :])

        hnat = main.tile([128, DBLK, HPAD], bf16, name="hnat")
        nc.vector.memset(hnat[:, :, F:HPAD], 0.0)

        # ---------------- attention ----------------
        # groups of 3 j-chunks; one [128,1536] exp per group (double-buffered
        # PSUM). U' = sum_j x~_j^T exp[j,:] accumulated in two K-half chains
        # (row groups 0/64 -> banks C/D); V-projection applied after.
        GROUPS = [(g * 3, min(3, JT - g * 3)) for g in range((JT + 2) // 3)]
        exp_f = mybir.ActivationFunctionType.Exp
        with tc.tile_pool(name="mm1p", bufs=2, space="PSUM") as mm1p, \
             tc.tile_pool(name="Up", bufs=1, space="PSUM") as Upp, \
             tc.tile_pool(name="esb", bufs=3) as esb, \
             tc.tile_pool(name="usb", bufs=2) as usb, \
             tc.tile_pool(name="hsm", bufs=4) as hsmall:
            UC = Upp.tile([128, ICH], f32, name="UC")
            UD = Upp.tile([128, ICH], f32, name="UD")
            for ci in range(NI):
                prev = None
                for (j0, glen) in GROUPS:
                    ps = mm1p.tile([128, 3 * ICH], f32, space="PSUM",
                                   tag="s", name="pss")
                    for k in range(glen):
                        j = j0 + k
                        r = 64 * (j & 1)
                        for ch in range(2):
                            nc.tensor.matmul(
                                out=ps[64 * ch:64 * ch + 64,
                                       k * ICH:(k + 1) * ICH],
                                lhsT=xhT[r:r + F1,
                                         j * 128 + 64 * ch:
                                         j * 128 + 64 * ch + 64],
                                rhs=KQT[r:r + F1,
                                        ci * ICH:(ci + 1) * ICH],
                                start=True, stop=True)
                    et = esb.tile([128, 3 * ICH], bf16, tag="e", name="et")
                    nc.scalar.activation(out=et[:, :glen * ICH],
                                         in_=ps[:, :glen * ICH], func=exp_f)
                    if prev is not None:
                        pe, pj0, pglen = prev
                        for k in range(pglen):
                            j = pj0 + k
                            for r in range(2):
                                nc.tensor.matmul(
                                    out=(UC if r == 0 else UD)[0:F1, :],
                                    lhsT=xn[64 * r:64 * r + 64, j * F1:(j + 1) * F1],
                                    rhs=pe[64 * r:64 * r + 64,
                                           k * ICH:(k + 1) * ICH],
                                    start=(j == 0), stop=False,
                                    skip_group_check=True)
                    prev = (et, j0, glen)
                pe, pj0, pglen = prev
                for k in range(pglen):
                    j = pj0 + k
                    for r in range(2):
                        nc.tensor.matmul(
                            out=(UC if r == 0 else UD)[0:F1, :],
                            lhsT=xn[64 * r:64 * r + 64, j * F1:(j + 1) * F1],
                            rhs=pe[64 * r:64 * r + 64,
                                   k * ICH:(k + 1) * ICH],
                            start=False, stop=(k == pglen - 1),
                            skip_group_check=True)
                # combine K-halves -> U'sb bf16 [F1, 512]
                # (avoid a two-PSUM-operand tensor_tensor: copy then add)
                Ucs = usb.tile([F1, ICH], f32, tag="ucs", name="Ucs")
                nc.vector.tensor_copy(out=Ucs[:], in_=UC[0:F1, :])
                Usb = usb.tile([F1, ICH], bf16, tag="usb", name="Usb")
                nc.vector.tensor_add(out=Usb[:], in0=Ucs[:],
                                     in1=UD[0:F1, :])
                # h natural: hraw[i,g] = sum_f U'sb[f,i] Wva[f,g]
                # (two 64-col halves to stay in the 64x64 tile grid)
                for t in range(4):
                    blk = ci * 4 + t
                    for ch in range(2):
                        nc.tensor.matmul(
                            out=UD[64 * ch:64 * ch + 64,
                                   t * 128:t * 128 + F1],
                            lhsT=Usb[:, t * 128 + 64 * ch:
                                     t * 128 + 64 * ch + 64],
                            rhs=Wva_t[:],
                            start=True, stop=True, skip_group_check=True)
                    hraw = UD[:, t * 128:t * 128 + F1]
                    rec = hsmall.tile([128, 1], f32, tag="rec", name="rec")
                    nc.vector.reciprocal(out=rec[:], in_=hraw[:, F:F1])
                    hh = hsmall.tile([128, F], f32, tag="hh", name="hh")
                    nc.vector.scalar_tensor_tensor(
                        out=hh[:], in0=hraw[:, :F], scalar=rec[:],
                        in1=Vl[:, blk * F:(blk + 1) * F],
                        op0=mybir.AluOpType.mult,
                        op1=mybir.AluOpType.add)
                    nc.vector.tensor_scalar_max(out=hnat[:, blk, :F],
                                                in0=hh[:], scalar1=0.0)
                    lo = blk * 128
                    nrows = min(128, max(0, ROWS - lo))
                    if nrows > 0:
                        nc.sync.dma_start(
                            out=d['h_loc'][lo:lo + nrows, :],
                            in_=hnat[:nrows, blk, :])
                if ci < 2:
                    nc.gpsimd.collective_compute(
                        "AllGather", mybir.AluOpType.bypass,
                        replica_groups=[list(range(NCORE))],
                        ins=[d['h_loc'][PLO[ci]:PHI[ci], :]],
                        outs=[d['h_full%d' % ci][:, :]])

        # ---------------- SAGE scatter (+ deferred AG piece 2) -----------
        # The SBUF pools for G/Pt/idx are hoisted to the outer scope so
        # their addresses never alias attention tiles (aliasing would delay
        # the gathers to attention end).  Gathers run free on the GpSimd
        # FIFO; the only forced edges are AG2-trigger after gather0 (so a
        # not-yet-ready trigger can't block it) and gather2 after AG2
        # (matches its real data dependency).
        aggS = main.tile([F, IPAD], f32, name="aggS")
        aggb = main.tile([F, IPAD], bf16, name="aggb")
        hT = main.tile([F, IPAD], bf16, name="hT")
        idx_t = sin.tile([128, CH * 8], mybir.dt.int16, name="idx_t")
        nc.sync.dma_start(out=idx_t[:], in_=d['gidx'][:, :])
        recT_t = sin.tile([F, IPAD], f32, name="recT_t")
        nc.sync.dma_start(out=recT_t[:], in_=d['recipT'][:, :])
        with tc.tile_pool(name="scp", bufs=3, space="PSUM") as scp, \
             tc.tile_pool(name="htp", bufs=2, space="PSUM") as htp:
            GH = (max(NCH) + 1) // 2
            from concourse.tile import add_dep_helper
            g0_insts = []

            def piece(p):
                # two gather calls per piece (separate tiles for precise
                # consumer deps); ~4.6K idxs each pipelines in the ring
                h0 = NCH[p] // 2
                Gs, spans = [], [(0, h0), (h0, NCH[p])]
                for (c0, c1) in spans:
                    G = gat.tile([128, GH, HPAD], bf16, tag="G", name="G")
                    gi = nc.gpsimd.dma_gather(
                        out_ap=G[:, :c1 - c0, :],
                        in_ap=d['h_full%d' % p][:, :],
                        idxs_ap=idx_t[:, (POFF[p] + c0) * 8:
                                      (POFF[p] + c1) * 8],
                        num_idxs=(c1 - c0) * 128,
                        num_idxs_reg=(c1 - c0) * 128,
                        elem_size=HPAD,
                        single_packet=False)
                    Gs.append(G)
                    if p == 0:
                        g0_insts.append(gi.ins)
                    if p == 2:
                        add_dep_helper(gi.ins, ag2.ins,
                                       reason="gather2 after AG2 trigger")
                Pt = pin.tile([128, max(NCH) * 128], bf16, tag="P",
                              name="Pt")
                nc.sync.dma_start(
                    out=Pt[:, :NCH[p] * 128],
                    in_=d['P'][:, POFF[p] * 128:(POFF[p] + NCH[p]) * 128])
                ch = 0
                for b in range(DBLK):
                    if S_bp[p][b] == 0:
                        if p == 0:
                            nc.vector.memset(
                                aggS[:, b * 128:(b + 1) * 128], 0.0)
                        continue
                    acc = scp.tile([F, 128], f32, space="PSUM", tag="agg",
                                   name="acc")
                    for s in range(S_bp[p][b]):
                        gsel = 0 if ch < h0 else 1
                        gch = ch if ch < h0 else ch - h0
                        nc.tensor.matmul(
                            out=acc[:], lhsT=Gs[gsel][:, gch, :F],
                            rhs=Pt[:, ch * 128:(ch + 1) * 128],
                            start=(s == 0), stop=(s == S_bp[p][b] - 1),
                            skip_group_check=True)
                        ch += 1
                    sl = aggS[:, b * 128:(b + 1) * 128]
                    if p == 0:
                        nc.vector.tensor_copy(out=sl, in_=acc[:])
                    else:
                        nc.vector.tensor_add(out=sl, in0=sl, in1=acc[:])

            ag2 = nc.gpsimd.collective_compute(
                "AllGather", mybir.AluOpType.bypass,
                replica_groups=[list(range(NCORE))],
                ins=[d['h_loc'][PLO[2]:PHI[2], :]],
                outs=[d['h_full2'][:, :]])
            piece(0)
            for g0i in g0_insts:
                add_dep_helper(ag2.ins, g0i,
                               reason="AG2 trigger after gather0")
            # hT (bf16) for SAGE lin_r: transpose the 12 h tiles (PE work
            # that fills the gap while gathers run on GpSimd)
            for t in range(DBLK):
                ps = htp.tile([F, 128], bf16, space="PSUM", tag="ht",
                              name="psht")
                nc.tensor.transpose(out=ps[:], in_=hnat[:, t, :F],
                                    identity=ident_t[:])
                nc.vector.tensor_copy(out=hT[:, t * 128:(t + 1) * 128],
                                      in_=ps[:])
            piece(1)
            piece(2)
            # normalize by degree -> bf16 for the SAGE linear
            nc.vector.tensor_mul(out=aggb[:], in0=aggS[:], in1=recT_t[:])

        # ---------------- SAGE linear + pool + MLP ----------------
        with tc.tile_pool(name="mlpw", bufs=1) as mlpw, \
             tc.tile_pool(name="mlps", bufs=2) as mlps, \
             tc.tile_pool(name="mlpp", bufs=2, space="PSUM") as mlpp:
            WllT_t = mlpw.tile([F, F], bf16, name="WllT_t")
            nc.sync.dma_start(out=WllT_t[:], in_=d['WllT'][:, :])
            WlrT_t = mlpw.tile([F, F], bf16, name="WlrT_t")
            nc.sync.dma_start(out=WlrT_t[:], in_=d['WlrT'][:, :])
            bll_t = mlpw.tile([F, 1], f32, name="bll_t")
            nc.sync.dma_start(out=bll_t[:], in_=d['bll'][:, :])
            Wg1T_t = mlpw.tile([F, 1500], f32, name="Wg1T_t")
            nc.sync.dma_start(out=Wg1T_t[:], in_=d['Wg1T'][:, :])
            bg1_t = mlpw.tile([128, 12], f32, name="bg1_t")
            nc.sync.dma_start(out=bg1_t[:], in_=d['bg1'][:, :])
            Wg2_t = mlpw.tile([128, 12 * 128], f32, name="Wg2_t")
            nc.sync.dma_start(out=Wg2_t[:], in_=d['Wg2Tr'][:, :])
            bg2_t = mlpw.tile([128, 1], f32, name="bg2_t")
            nc.sync.dma_start(out=bg2_t[:], in_=d['bg2'][:, :])
            WoT_t = mlpw.tile([128, 1], f32, name="WoT_t")
            nc.sync.dma_start(out=WoT_t[:], in_=d['WoT'][:, :])

            relu_f = mybir.ActivationFunctionType.Relu
            h2T = mlps.tile([F, IPAD], f32, tag="h2T", name="h2T")
            for ci in range(NI):
                ps = mlpp.tile([F, ICH], f32, space="PSUM", tag="h2",
                               name="psh2")
                nc.tensor.matmul(out=ps[:], lhsT=WllT_t[:],
                                 rhs=aggb[:, ci * ICH:(ci + 1) * ICH],
                                 start=True, stop=False,
                                 skip_group_check=True)
                nc.tensor.matmul(out=ps[:], lhsT=WlrT_t[:],
                                 rhs=hT[:, ci * ICH:(ci + 1) * ICH],
                                 start=False, stop=True,
                                 skip_group_check=True)
                nc.scalar.activation(out=h2T[:, ci * ICH:(ci + 1) * ICH],
                                     in_=ps[:], func=relu_f, bias=bll_t[:])

            gT = mlps.tile([F, GB], f32, tag="gT", name="gT")
            for g in range(GB):
                lo, hi = GRAPH_BOUNDS[g], GRAPH_BOUNDS[g + 1]
                nc.vector.tensor_reduce(out=gT[:, g:g + 1], in_=h2T[:, lo:hi],
                                        axis=mybir.AxisListType.X,
                                        op=mybir.AluOpType.max)
            g1T = mlps.tile([128, 12, GB], f32, tag="g1T", name="g1T")
            for j in range(12):
                w = min(128, 1500 - j * 128)
                ps = mlpp.tile([128, GB], f32, space="PSUM", tag="g1",
                               name="psg1")
                nc.tensor.matmul(out=ps[:w, :],
                                 lhsT=Wg1T_t[:, j * 128:j * 128 + w],
                                 rhs=gT[:], start=True, stop=True)
                if w < 128:
                    nc.vector.memset(g1T[:, j, :], 0.0)
                nc.scalar.activation(out=g1T[:w, j, :], in_=ps[:w, :],
                                     func=relu_f, bias=bg1_t[:w, j:j + 1])
            g2ps = mlpp.tile([128, GB], f32, space="PSUM", tag="g2",
                             name="g2ps")
            for j in range(12):
                nc.tensor.matmul(out=g2ps[:],
                                 lhsT=Wg2_t[:, j * 128:(j + 1) * 128],
                                 rhs=g1T[:, j, :], start=(j == 0),
                                 stop=(j == 11), skip_group_check=True)
            g2sb = mlps.tile([128, GB], f32, tag="g2sb", name="g2sb")
            nc.vector.tensor_scalar_add(out=g2sb[:], in0=g2ps[:],
                                        scalar1=bg2_t[:])
            ops = mlpp.tile([1, GB], f32, space="PSUM", tag="o", name="ops")
            nc.tensor.matmul(out=ops[:], lhsT=WoT_t[:], rhs=g2sb[:],
                             start=True, stop=True)
            osb = mlps.tile([1, GB], f32, tag="osb", name="osb")
            nc.vector.tensor_scalar_add(out=osb[:], in0=ops[:],
                                        scalar1=float(bo_const))
            nc.sync.dma_start(out=d['out8'][:, :], in_=osb[:])


def _build_program(S_bp, bo_const):
    import concourse.tile as tile
    from concourse import bacc, mybir

    f32 = mybir.dt.float32
    bf16 = mybir.dt.bfloat16
    CH = int(sum(S_bp))
    nc = bacc.Bacc("TRN2", target_bir_lowering=False, debug=False,
                   num_devices=NCORE)

    d = {}

    def dram_in(name, shape, dt=f32):
        d[name] = nc.dram_tensor(name, list(shape), dt, kind="ExternalInput")

    dram_in("xh", (F1, XW), bf16)
    dram_in("xn", (128, JT * F1), bf16)
    dram_in("KQ", (F1, IPAD), bf16)
    dram_in("Wva", (F1, F1), bf16)
    dram_in("Vl", (128, DBLK * F), f32)
    dram_in("ident", (128, 128), bf16)
    dram_in("WllT", (F, F), bf16)
    dram_in("WlrT", (F, F), bf16)
    dram_in("bll", (F, 1))
    dram_in("Wg1T", (F, 1500))
    dram_in("bg1", (128, 12))
    dram_in("Wg2Tr", (128, 12 * 128))
    dram_in("bg2", (128, 1))
    dram_in("WoT", (128, 1))
    dram_in("recipT", (F, IPAD))
    dram_in("P", (128, CH * 128), bf16)
    d['gidx'] = nc.dram_tensor("gidx", [128, CH * 8], mybir.dt.int16,
                               kind="ExternalInput")
    d['out8'] = nc.dram_tensor("out8", [1, GB], f32, kind="ExternalOutput")
    d['h_loc'] = nc.dram_tensor("h_loc", [ROWS, HPAD], bf16)
    for p in range(NP):
        d['h_full%d' % p] = nc.dram_tensor(
            "h_full%d" % p, [NCORE * PLEN[p], HPAD], bf16,
            addr_space="Shared")

    with tile.TileContext(nc) as tc:
        _emit_body(nc, tc, d, S_bp, bo_const)

    nc.compile()
    return nc


# --------------------------------------------------------------------------
# entry point
# --------------------------------------------------------------------------

_CACHE = {}


def _make_in_maps(inputs):
    x = np.asarray(inputs['x'], np.float32)
    edge_index = np.asarray(inputs['edge_index'])
    w = _prep_weights(inputs)
    xh, xn, KQ = _prep_x(x, w['M'], w['Wva'])
    Vl = _prep_vl(x, inputs)
    gidx, Ps, recipT, S_bp = _prep_edges(edge_index)
    ident = np.eye(128, dtype=BF16)
    common = dict(
        xh=xh, xn=xn, Wva=w['Wva'], ident=ident,
        WllT=w['WllT'].astype(BF16), WlrT=w['WlrT'].astype(BF16),
        bll=w['bll'], Wg1T=w['Wg1T'], bg1=w['bg1'], Wg2Tr=w['Wg2Tr'],
        bg2=w['bg2'], WoT=w['WoT'])
    in_maps = []
    for c in range(NCORE):
        m = dict(common)
        m['KQ'] = KQ[c]
        m['Vl'] = Vl[c]
        m['gidx'] = gidx[c]
        m['P'] = Ps[c]
        m['recipT'] = recipT[c]
        in_maps.append(m)
    return in_maps, S_bp, w['bo']


def kernel(**inputs):
    from concourse.bass_utils import run_bass_kernel_spmd

    in_maps, S_bp, bo = _make_in_maps(inputs)
    key = ('prog', S_bp, bo)
    if key not in _CACHE:
        _CACHE[key] = _build_program(S_bp, bo)
    nc = _CACHE[key]

    res = run_bass_kernel_spmd(nc, in_maps, list(range(NCORE)))
    global LAST_RESULT
    LAST_RESULT = res
    out = np.zeros((B, 1), np.float32)
    for c in range(NCORE):
        out[c * GB:(c + 1) * GB, 0] = res.results[c]['out8'].reshape(-1)
    return out


LAST_RESULT = None


# revision 26
# speedup vs baseline: 1.7169x; 1.0186x over previous
"""Trainium2 Bass kernel for nn_GAT_GraphSAGE (N=12000, E=192000, F=35, B=64).

Sharding: attention rows (softmax row i = K_new index) sharded 1500/core on
8 cores; one AllGather of post-attention h (bf16, 2 chunked pieces); SAGE
sharded by dst with a batched dma_gather of h[src] rows + one-hot-matmul
scatter; per-core global-max-pool + MLP head on that core's 8 graphs.

Key structure (v2):
- The whole K branch (Wk/conv-taps/Wl/1/sqrt(F)) and Wq fold on the host
  into one [F1,F1] matrix M (F1=36 with a ones row for biases):
  scores = x~ M x~^T.  The i-side projection KQT = M^T x~_loc^T is also
  host-computed, so the device never projects Q or K.
- All big matmuls run in bf16 at full PE rate, packed 2x2 into the 64x64
  tile grid (contraction 36 <= 64): scores split into two M-halves
  (PSUM col groups), with the j-parity on row groups (operands duplicated
  at partitions 64:100); the attn@V accumulation contracts raw x~ against
  exp(scores) split into two K-halves (row groups -> two PSUM banks), and
  the V projection (Wv augmented with a ones column that also carries the
  softmax denominator) is applied afterwards to the tiny [36,512] result.
  The same post-matmul also produces h in natural layout, so no PE-mode
  switches happen inside the attention loop.
- exp on ACT in [128,1536] batches (3 PSUM banks, double-buffered) is the
  bottleneck engine (~150us); everything else hides under it.
- SAGE scatter: one-hot P matrices are host-precomputed bf16 inputs (DMA,
  not DVE is_equal), gathered h rows are bf16 256B rows.
"""
import math
import numpy as np
import ml_dtypes

BF16 = ml_dtypes.bfloat16

N, E, F, B = 12000, 192000, 35, 64
F1 = F + 1
NCORE = 8
ROWS = N // NCORE            # 1500
ICH = 512
NI = 3
IPAD = ICH * NI              # 1536
JT = 94                      # j chunks of 128
JPAD = JT * 128              # 12032
XW = 12064                   # padded x~^T width (covers 7*1500 + 1536)
DBLK = 12                    # dst blocks (128 each) per core
GB = B // NCORE              # 8 graphs per core
HPAD = 128                   # h row padded to 128 bf16 (256B) for dma_gather
GRAPH_BOUNDS = [int(math.ceil(g * (N / B))) for g in range(GB + 1)]
# 3 AllGather pieces, one per attention i-chunk (local rows 512/512/476).
PLO = [0, 512, 1024]
PHI = [512, 1024, ROWS]
PLEN = [PHI[p] - PLO[p] for p in range(3)]
NP = 3


# --------------------------------------------------------------------------
# host-side preprocessing
# --------------------------------------------------------------------------

def _prep_weights(p):
    f64 = np.float64
    f32 = np.float32
    Wq, bq = p['Wq'].astype(f64), p['bq'].astype(f64)
    Wk, bk = p['Wk'].astype(f64), p['bk'].astype(f64)
    Wv, bv = p['Wv'].astype(f64), p['bv'].astype(f64)
    W3c, b3 = p['W3'][:, :, 1].astype(f64), p['b3'].astype(f64)
    W5c, b5 = p['W5'][:, :, 2].astype(f64), p['b5'].astype(f64)
    Wl, bl = p['Wl'].astype(f64), p['bl'].astype(f64)
    Wl1, Wl2, Wl3 = Wl[:, :F], Wl[:, F:2 * F], Wl[:, 2 * F:]

    # K_new = x~ @ Wkn~  (F1 -> F affine, includes 1/sqrt(F))
    Weff = W3c.T @ Wl1.T + W5c.T @ Wl2.T + Wl3.T
    beff = b3 @ Wl1.T + b5 @ Wl2.T + bl
    Wkn = Wk.T @ Weff
    bkn = bk @ Weff + beff
    s = 1.0 / np.sqrt(F)
    Wkn_aug = np.vstack([Wkn, bkn[None, :]]) * s          # [F1, F]
    Wq_aug = np.vstack([Wq.T, bq[None, :]])               # [F1, F]
    M = Wkn_aug @ Wq_aug.T                                # [F1, F1]

    Wva = np.zeros((F1, F1))
    Wva[:F, :F] = Wv.T
    Wva[F, :F] = bv
    Wva[F, F] = 1.0                                       # denominator column

    out = {'M': M, 'Wva': Wva.astype(BF16)}
    out['WllT'] = np.ascontiguousarray(p['Wll'].T).astype(f32)
    out['WlrT'] = np.ascontiguousarray(p['Wlr'].T).astype(f32)
    out['bll'] = p['bll'].astype(f32).reshape(F, 1)
    out['Wg1T'] = np.ascontiguousarray(p['Wg1'].T).astype(f32)   # [35,1500]
    bg1 = np.zeros((128, 12), f32)
    bg1.T.reshape(-1)[:1500] = p['bg1'].astype(f32)
    out['bg1'] = bg1
    w2 = np.zeros((12 * 128, 128), f32)
    w2[:1500, :] = p['Wg2'].T.astype(f32)
    out['Wg2Tr'] = np.ascontiguousarray(
        w2.reshape(12, 128, 128).transpose(1, 0, 2).reshape(128, 12 * 128))
    out['bg2'] = p['bg2'].astype(f32).reshape(128, 1)
    out['WoT'] = p['Wo'].astype(f32).reshape(1, 128).T.copy()     # [128,1]
    out['bo'] = float(np.asarray(p['bo']).reshape(-1)[0])
    return out


def _prep_x(x, M, Wva_unused):
    """Host: x~^T (bf16), x~ natural chunked (bf16), per-core KQT + Vl."""
    x64 = np.asarray(x, np.float64)
    xa = np.concatenate([x64, np.ones((N, 1))], axis=1)       # [N, F1]
    xaT = np.zeros((F1, XW))
    xaT[:, :N] = xa.T                                         # pad cols zero
    xh = xaT.astype(BF16)                                     # [F1, XW]

    # natural chunks for the U accumulation: [128, JT, F1]
    xn = np.zeros((128, JT, F1))
    flat = xaT[:, :JPAD].T                                    # [JPAD, F1]
    xn[:, :, :] = flat.reshape(JT, 128, F1).transpose(1, 0, 2)
    xn = np.ascontiguousarray(xn.reshape(128, JT * F1)).astype(BF16)

    KQ = []
    for c in range(NCORE):
        sl = xaT[:, c * ROWS: c * ROWS + IPAD]                # [F1, IPAD]
        KQ.append(np.ascontiguousarray(M.T @ sl).astype(BF16))
    return xh, xn, KQ


def _prep_vl(x, p):
    """Per-core natural V' local [128, DBLK*F] f32 (for the residual)."""
    f64 = np.float64
    Wv, bv = p['Wv'].astype(f64), p['bv'].astype(f64)
    x64 = np.asarray(x, np.float64)
    V = x64 @ Wv.T + bv                                       # [N, F]
    out = []
    for c in range(NCORE):
        vl = np.zeros((DBLK * 128, F))
        vl[:ROWS] = V[c * ROWS:(c + 1) * ROWS]
        out.append(np.ascontiguousarray(
            vl.reshape(DBLK, 128, F).transpose(1, 0, 2).reshape(128, DBLK * F)
        ).astype(np.float32))
    return out


def _prep_edges(edge_index):
    """Edges keyed (piece p of src, dst block b): chunk stream is p-major
    [p0: b0..b11][p1: ...][p2: ...] with S_bp (global max over cores)
    128-slot chunks per (p, b).  Returns per-core gather idx (relative to
    that piece's h_full_p tensor), one-hot P, recipT, and S_bp [NP][DBLK].
    """
    src = np.asarray(edge_index[0], np.int64)
    dst = np.asarray(edge_index[1], np.int64)
    deg = np.bincount(dst, minlength=N).astype(np.float64)
    recip = (1.0 / np.maximum(deg, 1.0)).astype(np.float32)

    core_of = dst // ROWS
    blk_of = (dst - core_of * ROWS) // 128
    sc = src // ROWS
    sr = src - sc * ROWS
    piece_of = np.where(sr < PLO[1], 0, np.where(sr < PLO[2], 1, 2))
    # position within piece p's gathered tensor [NCORE*PLEN[p], :]
    plen = np.array(PLEN)[piece_of]
    plo = np.array(PLO)[piece_of]
    pos = sc * plen + (sr - plo)

    counts = np.zeros((NCORE, NP, DBLK), np.int64)
    np.add.at(counts, (core_of, piece_of, blk_of), 1)
    S_bp = np.ceil(counts.max(axis=0) / 128).astype(np.int64)  # [NP, DBLK]
    CH = int(S_bp.sum())

    # chunk start offset for (p, b)
    ch_off = np.zeros((NP, DBLK), np.int64)
    acc = 0
    for p in range(NP):
        for b in range(DBLK):
            ch_off[p, b] = acc
            acc += S_bp[p, b]

    gidx, Ps = [], []
    for c in range(NCORE):
        idx_c = np.zeros(CH * 128, np.int16)
        rel_c = np.full(CH * 128, -1, np.int64)
        for p in range(NP):
            for b in range(DBLK):
                m = (core_of == c) & (piece_of == p) & (blk_of == b)
                n = int(m.sum())
                lo = int(ch_off[p, b]) * 128
                idx_c[lo:lo + n] = pos[m].astype(np.int16)
                rel_c[lo:lo + n] = dst[m] - c * ROWS - b * 128
        gidx.append(np.ascontiguousarray(
            np.tile(idx_c.reshape(-1, 16).T, (8, 1))))
        P = np.zeros((128, CH * 128), BF16)
        rel2 = rel_c.reshape(CH, 128)
        ch_i, e_i = np.nonzero(rel2 >= 0)
        P[e_i, ch_i * 128 + rel2[ch_i, e_i]] = 1
        Ps.append(np.ascontiguousarray(P))

    recipT = []
    for c in range(NCORE):
        r = np.ones(IPAD, np.float32)
        r[:ROWS] = recip[c * ROWS:(c + 1) * ROWS]
        recipT.append(np.ascontiguousarray(np.broadcast_to(r, (F, IPAD))))
    return gidx, Ps, recipT, tuple(int(v) for v in S_bp.reshape(-1))


# --------------------------------------------------------------------------
# device program
# --------------------------------------------------------------------------

def _emit_body(nc, tc, d, S_bp, bo_const):
    import concourse.tile as tile
    from concourse import mybir

    f32 = mybir.dt.float32
    bf16 = mybir.dt.bfloat16
    S_bp = [list(S_bp[p * DBLK:(p + 1) * DBLK]) for p in range(NP)]
    NCH = [int(sum(S_bp[p])) for p in range(NP)]       # chunks per piece
    CH = sum(NCH)
    POFF = [0, NCH[0], NCH[0] + NCH[1]]                # piece chunk offsets

    with tc.tile_pool(name="const", bufs=1) as constp, \
         tc.tile_pool(name="main", bufs=1) as main, \
         tc.tile_pool(name="gat", bufs=4) as gat, \
         tc.tile_pool(name="pin", bufs=2) as pin, \
         tc.tile_pool(name="sin", bufs=1) as sin:
        # ---- inputs ----
        KQT = main.tile([128, IPAD], bf16, name="KQT")
        nc.sync.dma_start(out=KQT[0:F1, :], in_=d['KQ'][:, :])
        nc.sync.dma_start(out=KQT[64:64 + F1, :], in_=d['KQ'][:, :])
        xhT = main.tile([128, XW], bf16, name="xhT")
        HW = XW // 4
        for q in range(4):
            nc.sync.dma_start(out=xhT[0:F1, q * HW:(q + 1) * HW],
                              in_=d['xh'][:, q * HW:(q + 1) * HW])
            nc.sync.dma_start(out=xhT[64:64 + F1, q * HW:(q + 1) * HW],
                              in_=d['xh'][:, q * HW:(q + 1) * HW])
        xn = main.tile([128, JT * F1], bf16, name="xn")
        nc.sync.dma_start(out=xn[:], in_=d['xn'][:, :])
        Wva_t = constp.tile([F1, F1], bf16, name="Wva_t")
        nc.sync.dma_start(out=Wva_t[:], in_=d['Wva'][:, :])
        Vl = main.tile([128, DBLK * F], f32, name="Vl")
        nc.sync.dma_start(out=Vl[:], in_=d['Vl'][:, :])
        ident_t = constp.tile([128, 128], bf16, name="ident_t")
        nc.sync.dma_start(out=ident_t[:], in_=d['ident'][:, :])

        hnat = main.tile([128, DBLK, HPAD], bf16, name="hnat")
        nc.vector.memset(hnat[:, :, F:HPAD], 0.0)

        # ---------------- attention ----------------
        # groups of 3 j-chunks; one [128,1536] exp per group (double-buffered
        # PSUM). U' = sum_j x~_j^T exp[j,:] accumulated in two K-half chains
        # (row groups 0/64 -> banks C/D); V-projection applied after.
        GROUPS = [(g * 3, min(3, JT - g * 3)) for g in range((JT + 2) // 3)]
        exp_f = mybir.ActivationFunctionType.Exp
        with tc.tile_pool(name="mm1p", bufs=2, space="PSUM") as mm1p, \
             tc.tile_pool(name="Up", bufs=1, space="PSUM") as Upp, \
             tc.tile_pool(name="esb", bufs=3) as esb, \
             tc.tile_pool(name="usb", bufs=2) as usb, \
             tc.tile_pool(name="hsm", bufs=4) as hsmall:
            UC = Upp.tile([128, ICH], f32, name="UC")
            UD = Upp.tile([128, ICH], f32, name="UD")
            for ci in range(NI):
                prev = None
                for (j0, glen) in GROUPS:
                    ps = mm1p.tile([128, 3 * ICH], f32, space="PSUM",
                                   tag="s", name="pss")
                    for k in range(glen):
                        j = j0 + k
                        r = 64 * (j & 1)
                        for ch in range(2):
                            nc.tensor.matmul(
                                out=ps[64 * ch:64 * ch + 64,
                                       k * ICH:(k + 1) * ICH],
                                lhsT=xhT[r:r + F1,
                                         j * 128 + 64 * ch:
                                         j * 128 + 64 * ch + 64],
                                rhs=KQT[r:r + F1,
                                        ci * ICH:(ci + 1) * ICH],
                                start=True, stop=True)
                    et = esb.tile([128, 3 * ICH], bf16, tag="e", name="et")
                    nc.scalar.activation(out=et[:, :glen * ICH],
                                         in_=ps[:, :glen * ICH], func=exp_f)
                    if prev is not None:
                        pe, pj0, pglen = prev
                        for k in range(pglen):
                            j = pj0 + k
                            for r in range(2):
                                nc.tensor.matmul(
                                    out=(UC if r == 0 else UD)[0:F1, :],
                                    lhsT=xn[64 * r:64 * r + 64, j * F1:(j + 1) * F1],
                                    rhs=pe[64 * r:64 * r + 64,
                                           k * ICH:(k + 1) * ICH],
                                    start=(j == 0), stop=False,
                                    skip_group_check=True)
                    prev = (et, j0, glen)
                pe, pj0, pglen = prev
                for k in range(pglen):
                    j = pj0 + k
                    for r in range(2):
                        nc.tensor.matmul(
                            out=(UC if r == 0 else UD)[0:F1, :],
                            lhsT=xn[64 * r:64 * r + 64, j * F1:(j + 1) * F1],
                            rhs=pe[64 * r:64 * r + 64,
                                   k * ICH:(k + 1) * ICH],
                            start=False, stop=(k == pglen - 1),
                            skip_group_check=True)
                # combine K-halves -> U'sb bf16 [F1, 512]
                # (avoid a two-PSUM-operand tensor_tensor: copy then add)
                Ucs = usb.tile([F1, ICH], f32, tag="ucs", name="Ucs")
                nc.vector.tensor_copy(out=Ucs[:], in_=UC[0:F1, :])
                Usb = usb.tile([F1, ICH], bf16, tag="usb", name="Usb")
                nc.vector.tensor_add(out=Usb[:], in0=Ucs[:],
                                     in1=UD[0:F1, :])
                # h natural: hraw[i,g] = sum_f U'sb[f,i] Wva[f,g]
                # (two 64-col halves to stay in the 64x64 tile grid)
                for t in range(4):
                    blk = ci * 4 + t
                    for ch in range(2):
                        last_att_mm = nc.tensor.matmul(
                            out=UD[64 * ch:64 * ch + 64,
                                   t * 128:t * 128 + F1],
                            lhsT=Usb[:, t * 128 + 64 * ch:
                                     t * 128 + 64 * ch + 64],
                            rhs=Wva_t[:],
                            start=True, stop=True, skip_group_check=True)
                    hraw = UD[:, t * 128:t * 128 + F1]
                    rec = hsmall.tile([128, 1], f32, tag="rec", name="rec")
                    nc.vector.reciprocal(out=rec[:], in_=hraw[:, F:F1])
                    hh = hsmall.tile([128, F], f32, tag="hh", name="hh")
                    nc.vector.scalar_tensor_tensor(
                        out=hh[:], in0=hraw[:, :F], scalar=rec[:],
                        in1=Vl[:, blk * F:(blk + 1) * F],
                        op0=mybir.AluOpType.mult,
                        op1=mybir.AluOpType.add)
                    nc.vector.tensor_scalar_max(out=hnat[:, blk, :F],
                                                in0=hh[:], scalar1=0.0)
                    lo = blk * 128
                    nrows = min(128, max(0, ROWS - lo))
                    if nrows > 0:
                        nc.sync.dma_start(
                            out=d['h_loc'][lo:lo + nrows, :],
                            in_=hnat[:nrows, blk, :])
                if ci < 2:
                    nc.gpsimd.collective_compute(
                        "AllGather", mybir.AluOpType.bypass,
                        replica_groups=[list(range(NCORE))],
                        ins=[d['h_loc'][PLO[ci]:PHI[ci], :]],
                        outs=[d['h_full%d' % ci][:, :]])

        # ---------------- SAGE scatter (+ deferred AG piece 2) -----------
        # The SBUF pools for G/Pt/idx are hoisted to the outer scope so
        # their addresses never alias attention tiles (aliasing would delay
        # the gathers to attention end).  Gathers run free on the GpSimd
        # FIFO; the only forced edges are AG2-trigger after gather0 (so a
        # not-yet-ready trigger can't block it) and gather2 after AG2
        # (matches its real data dependency).
        aggS = main.tile([F, IPAD], f32, name="aggS")
        aggb = main.tile([F, IPAD], bf16, name="aggb")
        hT = main.tile([F, IPAD], bf16, name="hT")
        idx_t = sin.tile([128, CH * 8], mybir.dt.int16, name="idx_t")
        nc.sync.dma_start(out=idx_t[:], in_=d['gidx'][:, :])
        recT_t = sin.tile([F, IPAD], f32, name="recT_t")
        nc.sync.dma_start(out=recT_t[:], in_=d['recipT'][:, :])
        with tc.tile_pool(name="scp", bufs=3, space="PSUM") as scp, \
             tc.tile_pool(name="htp", bufs=2, space="PSUM") as htp:
            GH = (max(NCH) + 1) // 2
            from concourse.tile import add_dep_helper
            g0_insts = []
            # The scheduler's cost model grossly underestimates dma_gather,
            # so it interleaves SAGE PE work into the attention FIFO where
            # it head-of-line blocks (PSUM banks alias attention pools
            # anyway).  Gate the first SAGE matmul of each piece + the hT
            # transposes on the last attention matmul.
            first_mm = [None]

            def gate(inst):
                if first_mm[0] is None:
                    add_dep_helper(inst.ins, last_att_mm.ins,
                                   reason="SAGE PE after attention")
                    first_mm[0] = inst

            def piece(p):
                # two gather calls per piece (separate tiles for precise
                # consumer deps); ~4.6K idxs each pipelines in the ring
                h0 = NCH[p] // 2
                Gs, spans = [], [(0, h0), (h0, NCH[p])]
                for (c0, c1) in spans:
                    G = gat.tile([128, GH, HPAD], bf16, tag="G", name="G")
                    gi = nc.gpsimd.dma_gather(
                        out_ap=G[:, :c1 - c0, :],
                        in_ap=d['h_full%d' % p][:, :],
                        idxs_ap=idx_t[:, (POFF[p] + c0) * 8:
                                      (POFF[p] + c1) * 8],
                        num_idxs=(c1 - c0) * 128,
                        num_idxs_reg=(c1 - c0) * 128,
                        elem_size=HPAD,
                        single_packet=False)
                    Gs.append(G)
                    if p == 0:
                        g0_insts.append(gi.ins)
                    if p == 2:
                        add_dep_helper(gi.ins, ag2.ins,
                                       reason="gather2 after AG2 trigger")
                Pt = pin.tile([128, max(NCH) * 128], bf16, tag="P",
                              name="Pt")
                nc.sync.dma_start(
                    out=Pt[:, :NCH[p] * 128],
                    in_=d['P'][:, POFF[p] * 128:(POFF[p] + NCH[p]) * 128])
                ch = 0
                for b in range(DBLK):
                    if S_bp[p][b] == 0:
                        if p == 0:
                            nc.vector.memset(
                                aggS[:, b * 128:(b + 1) * 128], 0.0)
                        continue
                    acc = scp.tile([F, 128], f32, space="PSUM", tag="agg",
                                   name="acc")
                    for s in range(S_bp[p][b]):
                        gsel = 0 if ch < h0 else 1
                        gch = ch if ch < h0 else ch - h0
                        mi = nc.tensor.matmul(
                            out=acc[:], lhsT=Gs[gsel][:, gch, :F],
                            rhs=Pt[:, ch * 128:(ch + 1) * 128],
                            start=(s == 0), stop=(s == S_bp[p][b] - 1),
                            skip_group_check=True)
                        gate(mi)
                        ch += 1
                    sl = aggS[:, b * 128:(b + 1) * 128]
                    if p == 0:
                        nc.vector.tensor_copy(out=sl, in_=acc[:])
                    else:
                        nc.vector.tensor_add(out=sl, in0=sl, in1=acc[:])

            ag2 = nc.gpsimd.collective_compute(
                "AllGather", mybir.AluOpType.bypass,
                replica_groups=[list(range(NCORE))],
                ins=[d['h_loc'][PLO[2]:PHI[2], :]],
                outs=[d['h_full2'][:, :]])
            piece(0)
            add_dep_helper(ag2.ins, g0_insts[0],
                           reason="AG2 trigger after gather0a")
            # hT (bf16) for SAGE lin_r: transpose the 12 h tiles (PE work
            # that fills the gap while gathers run on GpSimd)
            for t in range(DBLK):
                ps = htp.tile([F, 128], bf16, space="PSUM", tag="ht",
                              name="psht")
                ti = nc.tensor.transpose(out=ps[:], in_=hnat[:, t, :F],
                                         identity=ident_t[:])
                if t == 0:
                    add_dep_helper(ti.ins, last_att_mm.ins,
                                   reason="transposes after attention")
                nc.vector.tensor_copy(out=hT[:, t * 128:(t + 1) * 128],
                                      in_=ps[:])
            piece(1)
            piece(2)
            # normalize by degree -> bf16 for the SAGE linear
            nc.vector.tensor_mul(out=aggb[:], in0=aggS[:], in1=recT_t[:])

        # ---------------- SAGE linear + pool + MLP ----------------
        with tc.tile_pool(name="mlpw", bufs=1) as mlpw, \
             tc.tile_pool(name="mlps", bufs=2) as mlps, \
             tc.tile_pool(name="mlpp", bufs=2, space="PSUM") as mlpp:
            WllT_t = mlpw.tile([F, F], bf16, name="WllT_t")
            nc.sync.dma_start(out=WllT_t[:], in_=d['WllT'][:, :])
            WlrT_t = mlpw.tile([F, F], bf16, name="WlrT_t")
            nc.sync.dma_start(out=WlrT_t[:], in_=d['WlrT'][:, :])
            bll_t = mlpw.tile([F, 1], f32, name="bll_t")
            nc.sync.dma_start(out=bll_t[:], in_=d['bll'][:, :])
            Wg1T_t = mlpw.tile([F, 1500], f32, name="Wg1T_t")
            nc.sync.dma_start(out=Wg1T_t[:], in_=d['Wg1T'][:, :])
            bg1_t = mlpw.tile([128, 12], f32, name="bg1_t")
            nc.sync.dma_start(out=bg1_t[:], in_=d['bg1'][:, :])
            Wg2_t = mlpw.tile([128, 12 * 128], f32, name="Wg2_t")
            nc.sync.dma_start(out=Wg2_t[:], in_=d['Wg2Tr'][:, :])
            bg2_t = mlpw.tile([128, 1], f32, name="bg2_t")
            nc.sync.dma_start(out=bg2_t[:], in_=d['bg2'][:, :])
            WoT_t = mlpw.tile([128, 1], f32, name="WoT_t")
            nc.sync.dma_start(out=WoT_t[:], in_=d['WoT'][:, :])

            relu_f = mybir.ActivationFunctionType.Relu
            h2T = mlps.tile([F, IPAD], f32, tag="h2T", name="h2T")
            for ci in range(NI):
                ps = mlpp.tile([F, ICH], f32, space="PSUM", tag="h2",
                               name="psh2")
                nc.tensor.matmul(out=ps[:], lhsT=WllT_t[:],
                                 rhs=aggb[:, ci * ICH:(ci + 1) * ICH],
                                 start=True, stop=False,
                                 skip_group_check=True)
                nc.tensor.matmul(out=ps[:], lhsT=WlrT_t[:],
                                 rhs=hT[:, ci * ICH:(ci + 1) * ICH],
                                 start=False, stop=True,
                                 skip_group_check=True)
                nc.scalar.activation(out=h2T[:, ci * ICH:(ci + 1) * ICH],
                                     in_=ps[:], func=relu_f, bias=bll_t[:])

            gT = mlps.tile([F, GB], f32, tag="gT", name="gT")
            for g in range(GB):
                lo, hi = GRAPH_BOUNDS[g], GRAPH_BOUNDS[g + 1]
                nc.vector.tensor_reduce(out=gT[:, g:g + 1], in_=h2T[:, lo:hi],
                                        axis=mybir.AxisListType.X,
                                        op=mybir.AluOpType.max)
            g1T = mlps.tile([128, 12, GB], f32, tag="g1T", name="g1T")
            for j in range(12):
                w = min(128, 1500 - j * 128)
                ps = mlpp.tile([128, GB], f32, space="PSUM", tag="g1",
                               name="psg1")
                nc.tensor.matmul(out=ps[:w, :],
                                 lhsT=Wg1T_t[:, j * 128:j * 128 + w],
                                 rhs=gT[:], start=True, stop=True)
                if w < 128:
                    nc.vector.memset(g1T[:, j, :], 0.0)
                nc.scalar.activation(out=g1T[:w, j, :], in_=ps[:w, :],
                                     func=relu_f, bias=bg1_t[:w, j:j + 1])
            g2ps = mlpp.tile([128, GB], f32, space="PSUM", tag="g2",
                             name="g2ps")
            for j in range(12):
                nc.tensor.matmul(out=g2ps[:],
                                 lhsT=Wg2_t[:, j * 128:(j + 1) * 128],
                                 rhs=g1T[:, j, :], start=(j == 0),
                                 stop=(j == 11), skip_group_check=True)
            g2sb = mlps.tile([128, GB], f32, tag="g2sb", name="g2sb")
            nc.vector.tensor_scalar_add(out=g2sb[:], in0=g2ps[:],
                                        scalar1=bg2_t[:])
            ops = mlpp.tile([1, GB], f32, space="PSUM", tag="o", name="ops")
            nc.tensor.matmul(out=ops[:], lhsT=WoT_t[:], rhs=g2sb[:],
                             start=True, stop=True)
            osb = mlps.tile([1, GB], f32, tag="osb", name="osb")
            nc.vector.tensor_scalar_add(out=osb[:], in0=ops[:],
                                        scalar1=float(bo_const))
            nc.sync.dma_start(out=d['out8'][:, :], in_=osb[:])


def _build_program(S_bp, bo_const):
    import concourse.tile as tile
    from concourse import bacc, mybir

    f32 = mybir.dt.float32
    bf16 = mybir.dt.bfloat16
    CH = int(sum(S_bp))
    nc = bacc.Bacc("TRN2", target_bir_lowering=False, debug=False,
                   num_devices=NCORE)

    d = {}

    def dram_in(name, shape, dt=f32):
        d[name] = nc.dram_tensor(name, list(shape), dt, kind="ExternalInput")

    dram_in("xh", (F1, XW), bf16)
    dram_in("xn", (128, JT * F1), bf16)
    dram_in("KQ", (F1, IPAD), bf16)
    dram_in("Wva", (F1, F1), bf16)
    dram_in("Vl", (128, DBLK * F), f32)
    dram_in("ident", (128, 128), bf16)
    dram_in("WllT", (F, F), bf16)
    dram_in("WlrT", (F, F), bf16)
    dram_in("bll", (F, 1))
    dram_in("Wg1T", (F, 1500))
    dram_in("bg1", (128, 12))
    dram_in("Wg2Tr", (128, 12 * 128))
    dram_in("bg2", (128, 1))
    dram_in("WoT", (128, 1))
    dram_in("recipT", (F, IPAD))
    dram_in("P", (128, CH * 128), bf16)
    d['gidx'] = nc.dram_tensor("gidx", [128, CH * 8], mybir.dt.int16,
                               kind="ExternalInput")
    d['out8'] = nc.dram_tensor("out8", [1, GB], f32, kind="ExternalOutput")
    d['h_loc'] = nc.dram_tensor("h_loc", [ROWS, HPAD], bf16)
    for p in range(NP):
        d['h_full%d' % p] = nc.dram_tensor(
            "h_full%d" % p, [NCORE * PLEN[p], HPAD], bf16,
            addr_space="Shared")

    with tile.TileContext(nc) as tc:
        _emit_body(nc, tc, d, S_bp, bo_const)

    nc.compile()
    return nc


# --------------------------------------------------------------------------
# entry point
# --------------------------------------------------------------------------

_CACHE = {}


def _make_in_maps(inputs):
    x = np.asarray(inputs['x'], np.float32)
    edge_index = np.asarray(inputs['edge_index'])
    w = _prep_weights(inputs)
    xh, xn, KQ = _prep_x(x, w['M'], w['Wva'])
    Vl = _prep_vl(x, inputs)
    gidx, Ps, recipT, S_bp = _prep_edges(edge_index)
    ident = np.eye(128, dtype=BF16)
    common = dict(
        xh=xh, xn=xn, Wva=w['Wva'], ident=ident,
        WllT=w['WllT'].astype(BF16), WlrT=w['WlrT'].astype(BF16),
        bll=w['bll'], Wg1T=w['Wg1T'], bg1=w['bg1'], Wg2Tr=w['Wg2Tr'],
        bg2=w['bg2'], WoT=w['WoT'])
    in_maps = []
    for c in range(NCORE):
        m = dict(common)
        m['KQ'] = KQ[c]
        m['Vl'] = Vl[c]
        m['gidx'] = gidx[c]
        m['P'] = Ps[c]
        m['recipT'] = recipT[c]
        in_maps.append(m)
    return in_maps, S_bp, w['bo']


def kernel(**inputs):
    from concourse.bass_utils import run_bass_kernel_spmd

    in_maps, S_bp, bo = _make_in_maps(inputs)
    key = ('prog', S_bp, bo)
    if key not in _CACHE:
        _CACHE[key] = _build_program(S_bp, bo)
    nc = _CACHE[key]

    res = run_bass_kernel_spmd(nc, in_maps, list(range(NCORE)))
    global LAST_RESULT
    LAST_RESULT = res
    out = np.zeros((B, 1), np.float32)
    for c in range(NCORE):
        out[c * GB:(c + 1) * GB, 0] = res.results[c]['out8'].reshape(-1)
    return out


LAST_RESULT = None


# revision 31
# speedup vs baseline: 1.9514x; 1.1366x over previous
"""Trainium2 Bass kernel for nn_GAT_GraphSAGE (N=12000, E=192000, F=35, B=64).

Sharding: attention rows (softmax row i = K_new index) sharded 1500/core on
8 cores; one AllGather of post-attention h (bf16, 2 chunked pieces); SAGE
sharded by dst with a batched dma_gather of h[src] rows + one-hot-matmul
scatter; per-core global-max-pool + MLP head on that core's 8 graphs.

Key structure (v2):
- The whole K branch (Wk/conv-taps/Wl/1/sqrt(F)) and Wq fold on the host
  into one [F1,F1] matrix M (F1=36 with a ones row for biases):
  scores = x~ M x~^T.  The i-side projection KQT = M^T x~_loc^T is also
  host-computed, so the device never projects Q or K.
- All big matmuls run in bf16 at full PE rate, packed 2x2 into the 64x64
  tile grid (contraction 36 <= 64): scores split into two M-halves
  (PSUM col groups), with the j-parity on row groups (operands duplicated
  at partitions 64:100); the attn@V accumulation contracts raw x~ against
  exp(scores) split into two K-halves (row groups -> two PSUM banks), and
  the V projection (Wv augmented with a ones column that also carries the
  softmax denominator) is applied afterwards to the tiny [36,512] result.
  The same post-matmul also produces h in natural layout, so no PE-mode
  switches happen inside the attention loop.
- exp on ACT in [128,1536] batches (3 PSUM banks, double-buffered) is the
  bottleneck engine (~150us); everything else hides under it.
- SAGE scatter: one-hot P matrices are host-precomputed bf16 inputs (DMA,
  not DVE is_equal), gathered h rows are bf16 256B rows.
"""
import math
import numpy as np
import ml_dtypes

BF16 = ml_dtypes.bfloat16

N, E, F, B = 12000, 192000, 35, 64
F1 = F + 1
NCORE = 8
ROWS = N // NCORE            # 1500
ICH = 512
NI = 3
IPAD = ICH * NI              # 1536
JT = 94                      # j chunks of 128
JPAD = JT * 128              # 12032
XW = 12064                   # padded x~^T width (covers 7*1500 + 1536)
DBLK = 12                    # dst blocks (128 each) per core
GB = B // NCORE              # 8 graphs per core
HPAD = 128                   # h row padded to 128 bf16 (256B) for dma_gather
GRAPH_BOUNDS = [int(math.ceil(g * (N / B))) for g in range(GB + 1)]
# 3 AllGather pieces, one per attention i-chunk (local rows 512/512/476).
PLO = [0, 512, 1024]
PHI = [512, 1024, ROWS]
PLEN = [PHI[p] - PLO[p] for p in range(3)]
NP = 3


# --------------------------------------------------------------------------
# host-side preprocessing
# --------------------------------------------------------------------------

def _prep_weights(p):
    f64 = np.float64
    f32 = np.float32
    Wq, bq = p['Wq'].astype(f64), p['bq'].astype(f64)
    Wk, bk = p['Wk'].astype(f64), p['bk'].astype(f64)
    Wv, bv = p['Wv'].astype(f64), p['bv'].astype(f64)
    W3c, b3 = p['W3'][:, :, 1].astype(f64), p['b3'].astype(f64)
    W5c, b5 = p['W5'][:, :, 2].astype(f64), p['b5'].astype(f64)
    Wl, bl = p['Wl'].astype(f64), p['bl'].astype(f64)
    Wl1, Wl2, Wl3 = Wl[:, :F], Wl[:, F:2 * F], Wl[:, 2 * F:]

    # K_new = x~ @ Wkn~  (F1 -> F affine, includes 1/sqrt(F))
    Weff = W3c.T @ Wl1.T + W5c.T @ Wl2.T + Wl3.T
    beff = b3 @ Wl1.T + b5 @ Wl2.T + bl
    Wkn = Wk.T @ Weff
    bkn = bk @ Weff + beff
    s = 1.0 / np.sqrt(F)
    Wkn_aug = np.vstack([Wkn, bkn[None, :]]) * s          # [F1, F]
    Wq_aug = np.vstack([Wq.T, bq[None, :]])               # [F1, F]
    M = Wkn_aug @ Wq_aug.T                                # [F1, F1]

    Wva = np.zeros((F1, F1))
    Wva[:F, :F] = Wv.T
    Wva[F, :F] = bv
    Wva[F, F] = 1.0                                       # denominator column

    out = {'M': M, 'Wva': Wva.astype(BF16)}
    out['WllT'] = np.ascontiguousarray(p['Wll'].T).astype(f32)
    out['WlrT'] = np.ascontiguousarray(p['Wlr'].T).astype(f32)
    out['bll'] = p['bll'].astype(f32).reshape(F, 1)
    out['Wg1T'] = np.ascontiguousarray(p['Wg1'].T).astype(f32)   # [35,1500]
    bg1 = np.zeros((128, 12), f32)
    bg1.T.reshape(-1)[:1500] = p['bg1'].astype(f32)
    out['bg1'] = bg1
    w2 = np.zeros((12 * 128, 128), f32)
    w2[:1500, :] = p['Wg2'].T.astype(f32)
    out['Wg2Tr'] = np.ascontiguousarray(
        w2.reshape(12, 128, 128).transpose(1, 0, 2).reshape(128, 12 * 128))
    out['bg2'] = p['bg2'].astype(f32).reshape(128, 1)
    out['WoT'] = p['Wo'].astype(f32).reshape(1, 128).T.copy()     # [128,1]
    out['bo'] = float(np.asarray(p['bo']).reshape(-1)[0])
    return out


def _prep_x(x, M, Wva_unused):
    """Host: x~^T (bf16), x~ natural chunked (bf16), per-core KQT + Vl."""
    x64 = np.asarray(x, np.float64)
    xa = np.concatenate([x64, np.ones((N, 1))], axis=1)       # [N, F1]
    xaT = np.zeros((F1, XW))
    xaT[:, :N] = xa.T                                         # pad cols zero
    xh = xaT.astype(BF16)                                     # [F1, XW]

    # natural chunks for the U accumulation: [128, JT, F1]
    xn = np.zeros((128, JT, F1))
    flat = xaT[:, :JPAD].T                                    # [JPAD, F1]
    xn[:, :, :] = flat.reshape(JT, 128, F1).transpose(1, 0, 2)
    xn = np.ascontiguousarray(xn.reshape(128, JT * F1)).astype(BF16)

    KQ = []
    for c in range(NCORE):
        sl = xaT[:, c * ROWS: c * ROWS + IPAD]                # [F1, IPAD]
        KQ.append(np.ascontiguousarray(M.T @ sl).astype(BF16))
    return xh, xn, KQ


def _prep_vl(x, p):
    """Per-core natural V' local [128, DBLK*F] f32 (for the residual)."""
    f64 = np.float64
    Wv, bv = p['Wv'].astype(f64), p['bv'].astype(f64)
    x64 = np.asarray(x, np.float64)
    V = x64 @ Wv.T + bv                                       # [N, F]
    out = []
    for c in range(NCORE):
        vl = np.zeros((DBLK * 128, F))
        vl[:ROWS] = V[c * ROWS:(c + 1) * ROWS]
        out.append(np.ascontiguousarray(
            vl.reshape(DBLK, 128, F).transpose(1, 0, 2).reshape(128, DBLK * F)
        ).astype(np.float32))
    return out


def _prep_edges(edge_index):
    """Edges keyed (piece p of src, dst block b): chunk stream is p-major
    [p0: b0..b11][p1: ...][p2: ...] with S_bp (global max over cores)
    128-slot chunks per (p, b).  Returns per-core gather idx (relative to
    that piece's h_full_p tensor), one-hot P, recipT, and S_bp [NP][DBLK].
    """
    src = np.asarray(edge_index[0], np.int64)
    dst = np.asarray(edge_index[1], np.int64)
    deg = np.bincount(dst, minlength=N).astype(np.float64)
    recip = (1.0 / np.maximum(deg, 1.0)).astype(np.float32)

    core_of = dst // ROWS
    blk_of = (dst - core_of * ROWS) // 128
    sc = src // ROWS
    sr = src - sc * ROWS
    piece_of = np.where(sr < PLO[1], 0, np.where(sr < PLO[2], 1, 2))
    # position within piece p's gathered tensor [NCORE*PLEN[p], :]
    plen = np.array(PLEN)[piece_of]
    plo = np.array(PLO)[piece_of]
    pos = sc * plen + (sr - plo)

    counts = np.zeros((NCORE, NP, DBLK), np.int64)
    np.add.at(counts, (core_of, piece_of, blk_of), 1)
    S_bp = np.ceil(counts.max(axis=0) / 128).astype(np.int64)  # [NP, DBLK]
    CH = int(S_bp.sum())

    # chunk start offset for (p, b)
    ch_off = np.zeros((NP, DBLK), np.int64)
    acc = 0
    for p in range(NP):
        for b in range(DBLK):
            ch_off[p, b] = acc
            acc += S_bp[p, b]

    gidx, Ps = [], []
    for c in range(NCORE):
        idx_c = np.zeros(CH * 128, np.int16)
        rel_c = np.full(CH * 128, -1, np.int64)
        for p in range(NP):
            for b in range(DBLK):
                m = (core_of == c) & (piece_of == p) & (blk_of == b)
                n = int(m.sum())
                lo = int(ch_off[p, b]) * 128
                idx_c[lo:lo + n] = pos[m].astype(np.int16)
                rel_c[lo:lo + n] = dst[m] - c * ROWS - b * 128
        gidx.append(np.ascontiguousarray(
            np.tile(idx_c.reshape(-1, 16).T, (8, 1))))
        P = np.zeros((128, CH * 128), BF16)
        rel2 = rel_c.reshape(CH, 128)
        ch_i, e_i = np.nonzero(rel2 >= 0)
        P[e_i, ch_i * 128 + rel2[ch_i, e_i]] = 1
        Ps.append(np.ascontiguousarray(P))

    recipT = []
    for c in range(NCORE):
        r = np.ones(IPAD, np.float32)
        r[:ROWS] = recip[c * ROWS:(c + 1) * ROWS]
        recipT.append(np.ascontiguousarray(np.broadcast_to(r, (F, IPAD))))
    return gidx, Ps, recipT, tuple(int(v) for v in S_bp.reshape(-1))


# --------------------------------------------------------------------------
# device program
# --------------------------------------------------------------------------

def _emit_body(nc, tc, d, S_bp, bo_const):
    import concourse.tile as tile
    from concourse import mybir

    f32 = mybir.dt.float32
    bf16 = mybir.dt.bfloat16
    S_bp = [list(S_bp[p * DBLK:(p + 1) * DBLK]) for p in range(NP)]
    NCH = [int(sum(S_bp[p])) for p in range(NP)]       # chunks per piece
    CH = sum(NCH)
    POFF = [0, NCH[0], NCH[0] + NCH[1]]                # piece chunk offsets

    with tc.tile_pool(name="const", bufs=1) as constp, \
         tc.tile_pool(name="main", bufs=1) as main, \
         tc.tile_pool(name="gat", bufs=8) as gat, \
         tc.tile_pool(name="pin", bufs=2) as pin, \
         tc.tile_pool(name="sin", bufs=1) as sin:
        # ---- inputs ----
        KQT = main.tile([128, IPAD], bf16, name="KQT")
        nc.sync.dma_start(out=KQT[0:F1, :], in_=d['KQ'][:, :])
        nc.sync.dma_start(out=KQT[64:64 + F1, :], in_=d['KQ'][:, :])
        xhT = main.tile([128, XW], bf16, name="xhT")
        HW = XW // 4
        for q in range(4):
            nc.sync.dma_start(out=xhT[0:F1, q * HW:(q + 1) * HW],
                              in_=d['xh'][:, q * HW:(q + 1) * HW])
            nc.sync.dma_start(out=xhT[64:64 + F1, q * HW:(q + 1) * HW],
                              in_=d['xh'][:, q * HW:(q + 1) * HW])
        xn = main.tile([128, JT * F1], bf16, name="xn")
        nc.sync.dma_start(out=xn[:], in_=d['xn'][:, :])
        Wva_t = constp.tile([F1, F1], bf16, name="Wva_t")
        nc.sync.dma_start(out=Wva_t[:], in_=d['Wva'][:, :])
        Vl = main.tile([128, DBLK * F], f32, name="Vl")
        nc.sync.dma_start(out=Vl[:], in_=d['Vl'][:, :])
        ident_t = constp.tile([128, 128], bf16, name="ident_t")
        nc.sync.dma_start(out=ident_t[:], in_=d['ident'][:, :])

        hnat = main.tile([128, DBLK, HPAD], bf16, name="hnat")
        nc.vector.memset(hnat[:, :, F:HPAD], 0.0)

        # ---------------- attention ----------------
        # groups of 3 j-chunks; one [128,1536] exp per group (double-buffered
        # PSUM). U' = sum_j x~_j^T exp[j,:] accumulated in two K-half chains
        # (row groups 0/64 -> banks C/D); V-projection applied after.
        GROUPS = [(g * 3, min(3, JT - g * 3)) for g in range((JT + 2) // 3)]
        exp_f = mybir.ActivationFunctionType.Exp
        with tc.tile_pool(name="mm1p", bufs=2, space="PSUM") as mm1p, \
             tc.tile_pool(name="Up", bufs=1, space="PSUM") as Upp, \
             tc.tile_pool(name="esb", bufs=3) as esb, \
             tc.tile_pool(name="usb", bufs=2) as usb, \
             tc.tile_pool(name="hsm", bufs=4) as hsmall:
            UC = Upp.tile([128, ICH], f32, name="UC")
            UD = Upp.tile([128, ICH], f32, name="UD")
            for ci in range(NI):
                prev = None
                for (j0, glen) in GROUPS:
                    ps = mm1p.tile([128, 3 * ICH], f32, space="PSUM",
                                   tag="s", name="pss")
                    for k in range(glen):
                        j = j0 + k
                        r = 64 * (j & 1)
                        for ch in range(2):
                            nc.tensor.matmul(
                                out=ps[64 * ch:64 * ch + 64,
                                       k * ICH:(k + 1) * ICH],
                                lhsT=xhT[r:r + F1,
                                         j * 128 + 64 * ch:
                                         j * 128 + 64 * ch + 64],
                                rhs=KQT[r:r + F1,
                                        ci * ICH:(ci + 1) * ICH],
                                start=True, stop=True)
                    et = esb.tile([128, 3 * ICH], bf16, tag="e", name="et")
                    nc.scalar.activation(out=et[:, :glen * ICH],
                                         in_=ps[:, :glen * ICH], func=exp_f)
                    if prev is not None:
                        pe, pj0, pglen = prev
                        for k in range(pglen):
                            j = pj0 + k
                            for r in range(2):
                                nc.tensor.matmul(
                                    out=(UC if r == 0 else UD)[0:F1, :],
                                    lhsT=xn[64 * r:64 * r + 64, j * F1:(j + 1) * F1],
                                    rhs=pe[64 * r:64 * r + 64,
                                           k * ICH:(k + 1) * ICH],
                                    start=(j == 0), stop=False,
                                    skip_group_check=True)
                    prev = (et, j0, glen)
                pe, pj0, pglen = prev
                for k in range(pglen):
                    j = pj0 + k
                    for r in range(2):
                        nc.tensor.matmul(
                            out=(UC if r == 0 else UD)[0:F1, :],
                            lhsT=xn[64 * r:64 * r + 64, j * F1:(j + 1) * F1],
                            rhs=pe[64 * r:64 * r + 64,
                                   k * ICH:(k + 1) * ICH],
                            start=False, stop=(k == pglen - 1),
                            skip_group_check=True)
                # combine K-halves -> U'sb bf16 [F1, 512]
                # (avoid a two-PSUM-operand tensor_tensor: copy then add)
                Ucs = usb.tile([F1, ICH], f32, tag="ucs", name="Ucs")
                nc.vector.tensor_copy(out=Ucs[:], in_=UC[0:F1, :])
                Usb = usb.tile([F1, ICH], bf16, tag="usb", name="Usb")
                nc.vector.tensor_add(out=Usb[:], in0=Ucs[:],
                                     in1=UD[0:F1, :])
                # h natural: hraw[i,g] = sum_f U'sb[f,i] Wva[f,g]
                # (two 64-col halves to stay in the 64x64 tile grid)
                for t in range(4):
                    blk = ci * 4 + t
                    for ch in range(2):
                        last_att_mm = nc.tensor.matmul(
                            out=UD[64 * ch:64 * ch + 64,
                                   t * 128:t * 128 + F1],
                            lhsT=Usb[:, t * 128 + 64 * ch:
                                     t * 128 + 64 * ch + 64],
                            rhs=Wva_t[:],
                            start=True, stop=True, skip_group_check=True)
                    hraw = UD[:, t * 128:t * 128 + F1]
                    rec = hsmall.tile([128, 1], f32, tag="rec", name="rec")
                    nc.vector.reciprocal(out=rec[:], in_=hraw[:, F:F1])
                    hh = hsmall.tile([128, F], f32, tag="hh", name="hh")
                    nc.vector.scalar_tensor_tensor(
                        out=hh[:], in0=hraw[:, :F], scalar=rec[:],
                        in1=Vl[:, blk * F:(blk + 1) * F],
                        op0=mybir.AluOpType.mult,
                        op1=mybir.AluOpType.add)
                    nc.vector.tensor_scalar_max(out=hnat[:, blk, :F],
                                                in0=hh[:], scalar1=0.0)
                    lo = blk * 128
                    nrows = min(128, max(0, ROWS - lo))
                    if nrows > 0:
                        nc.sync.dma_start(
                            out=d['h_loc'][lo:lo + nrows, :],
                            in_=hnat[:nrows, blk, :])
                if ci < 2:
                    nc.gpsimd.collective_compute(
                        "AllGather", mybir.AluOpType.bypass,
                        replica_groups=[list(range(NCORE))],
                        ins=[d['h_loc'][PLO[ci]:PHI[ci], :]],
                        outs=[d['h_full%d' % ci][:, :]])

        # ---------------- SAGE scatter (+ deferred AG piece 2) -----------
        # The SBUF pools for G/Pt/idx are hoisted to the outer scope so
        # their addresses never alias attention tiles (aliasing would delay
        # the gathers to attention end).  Gathers run free on the GpSimd
        # FIFO; the only forced edges are AG2-trigger after gather0 (so a
        # not-yet-ready trigger can't block it) and gather2 after AG2
        # (matches its real data dependency).
        aggS = main.tile([F, IPAD], f32, name="aggS")
        aggb = main.tile([F, IPAD], bf16, name="aggb")
        hT = main.tile([F, IPAD], bf16, name="hT")
        idx_t = sin.tile([128, CH * 8], mybir.dt.int16, name="idx_t")
        nc.sync.dma_start(out=idx_t[:], in_=d['gidx'][:, :])
        recT_t = sin.tile([F, IPAD], f32, name="recT_t")
        nc.sync.dma_start(out=recT_t[:], in_=d['recipT'][:, :])
        with tc.tile_pool(name="scp", bufs=3, space="PSUM") as scp, \
             tc.tile_pool(name="htp", bufs=2, space="PSUM") as htp:
            NSPL = 4
            GH = (max(NCH) + NSPL - 1) // NSPL
            from concourse.tile import add_dep_helper
            g0_insts = []
            # The scheduler's cost model grossly underestimates dma_gather,
            # so it interleaves SAGE PE work into the attention FIFO where
            # it head-of-line blocks (PSUM banks alias attention pools
            # anyway).  Gate the first SAGE matmul of each piece + the hT
            # transposes on the last attention matmul.
            first_mm = [None]

            def gate(inst):
                if first_mm[0] is None:
                    add_dep_helper(inst.ins, last_att_mm.ins,
                                   reason="SAGE PE after attention")
                    first_mm[0] = inst

            # NSPL gather calls per piece: small quanta limit how long DVE
            # work can stall behind an in-flight SWDGE call
            def piece(p):
                qs = [NCH[p] * k // NSPL for k in range(NSPL + 1)]
                spans = [(qs[k], qs[k + 1]) for k in range(NSPL)]
                Gs = []
                for (c0, c1) in spans:
                    G = gat.tile([128, GH, HPAD], bf16, tag="G", name="G")
                    gi = nc.gpsimd.dma_gather(
                        out_ap=G[:, :c1 - c0, :],
                        in_ap=d['h_full%d' % p][:, :],
                        idxs_ap=idx_t[:, (POFF[p] + c0) * 8:
                                      (POFF[p] + c1) * 8],
                        num_idxs=(c1 - c0) * 128,
                        num_idxs_reg=(c1 - c0) * 128,
                        elem_size=HPAD,
                        single_packet=False)
                    Gs.append(G)
                    if p == 0:
                        g0_insts.append(gi.ins)
                    if p == 2:
                        add_dep_helper(gi.ins, ag2.ins,
                                       reason="gather2 after AG2 trigger")
                Pt = pin.tile([128, max(NCH) * 128], bf16, tag="P",
                              name="Pt")
                nc.sync.dma_start(
                    out=Pt[:, :NCH[p] * 128],
                    in_=d['P'][:, POFF[p] * 128:(POFF[p] + NCH[p]) * 128])
                ch = 0
                for b in range(DBLK):
                    if S_bp[p][b] == 0:
                        if p == 0:
                            nc.vector.memset(
                                aggS[:, b * 128:(b + 1) * 128], 0.0)
                        continue
                    acc = scp.tile([F, 128], f32, space="PSUM", tag="agg",
                                   name="acc")
                    for s in range(S_bp[p][b]):
                        gsel = 0
                        while ch >= qs[gsel + 1]:
                            gsel += 1
                        mi = nc.tensor.matmul(
                            out=acc[:], lhsT=Gs[gsel][:, ch - qs[gsel], :F],
                            rhs=Pt[:, ch * 128:(ch + 1) * 128],
                            start=(s == 0), stop=(s == S_bp[p][b] - 1),
                            skip_group_check=True)
                        gate(mi)
                        ch += 1
                    sl = aggS[:, b * 128:(b + 1) * 128]
                    if p == 0:
                        nc.vector.tensor_copy(out=sl, in_=acc[:])
                    else:
                        nc.vector.tensor_add(out=sl, in0=sl, in1=acc[:])

            ag2 = nc.gpsimd.collective_compute(
                "AllGather", mybir.AluOpType.bypass,
                replica_groups=[list(range(NCORE))],
                ins=[d['h_loc'][PLO[2]:PHI[2], :]],
                outs=[d['h_full2'][:, :]])
            piece(0)
            add_dep_helper(ag2.ins, g0_insts[0],
                           reason="AG2 trigger after gather0a")
            # hT (bf16) for SAGE lin_r: transpose the 12 h tiles (PE work
            # that fills the gap while gathers run on GpSimd)
            for t in range(DBLK):
                ps = htp.tile([F, 128], bf16, space="PSUM", tag="ht",
                              name="psht")
                ti = nc.tensor.transpose(out=ps[:], in_=hnat[:, t, :F],
                                         identity=ident_t[:])
                if t == 0:
                    add_dep_helper(ti.ins, last_att_mm.ins,
                                   reason="transposes after attention")
                nc.vector.tensor_copy(out=hT[:, t * 128:(t + 1) * 128],
                                      in_=ps[:])
            piece(1)
            piece(2)
            # normalize by degree -> bf16 for the SAGE linear
            nc.vector.tensor_mul(out=aggb[:], in0=aggS[:], in1=recT_t[:])

        # ---------------- SAGE linear + pool + MLP ----------------
        with tc.tile_pool(name="mlpw", bufs=1) as mlpw, \
             tc.tile_pool(name="mlps", bufs=2) as mlps, \
             tc.tile_pool(name="mlpp", bufs=2, space="PSUM") as mlpp:
            WllT_t = mlpw.tile([F, F], bf16, name="WllT_t")
            nc.sync.dma_start(out=WllT_t[:], in_=d['WllT'][:, :])
            WlrT_t = mlpw.tile([F, F], bf16, name="WlrT_t")
            nc.sync.dma_start(out=WlrT_t[:], in_=d['WlrT'][:, :])
            bll_t = mlpw.tile([F, 1], f32, name="bll_t")
            nc.sync.dma_start(out=bll_t[:], in_=d['bll'][:, :])
            Wg1T_t = mlpw.tile([F, 1500], f32, name="Wg1T_t")
            nc.sync.dma_start(out=Wg1T_t[:], in_=d['Wg1T'][:, :])
            bg1_t = mlpw.tile([128, 12], f32, name="bg1_t")
            nc.sync.dma_start(out=bg1_t[:], in_=d['bg1'][:, :])
            Wg2_t = mlpw.tile([128, 12 * 128], f32, name="Wg2_t")
            nc.sync.dma_start(out=Wg2_t[:], in_=d['Wg2Tr'][:, :])
            bg2_t = mlpw.tile([128, 1], f32, name="bg2_t")
            nc.sync.dma_start(out=bg2_t[:], in_=d['bg2'][:, :])
            WoT_t = mlpw.tile([128, 1], f32, name="WoT_t")
            nc.sync.dma_start(out=WoT_t[:], in_=d['WoT'][:, :])

            relu_f = mybir.ActivationFunctionType.Relu
            h2T = mlps.tile([F, IPAD], f32, tag="h2T", name="h2T")
            for ci in range(NI):
                ps = mlpp.tile([F, ICH], f32, space="PSUM", tag="h2",
                               name="psh2")
                nc.tensor.matmul(out=ps[:], lhsT=WllT_t[:],
                                 rhs=aggb[:, ci * ICH:(ci + 1) * ICH],
                                 start=True, stop=False,
                                 skip_group_check=True)
                nc.tensor.matmul(out=ps[:], lhsT=WlrT_t[:],
                                 rhs=hT[:, ci * ICH:(ci + 1) * ICH],
                                 start=False, stop=True,
                                 skip_group_check=True)
                nc.scalar.activation(out=h2T[:, ci * ICH:(ci + 1) * ICH],
                                     in_=ps[:], func=relu_f, bias=bll_t[:])

            gT = mlps.tile([F, GB], f32, tag="gT", name="gT")
            for g in range(GB):
                lo, hi = GRAPH_BOUNDS[g], GRAPH_BOUNDS[g + 1]
                nc.vector.tensor_reduce(out=gT[:, g:g + 1], in_=h2T[:, lo:hi],
                                        axis=mybir.AxisListType.X,
                                        op=mybir.AluOpType.max)
            g1T = mlps.tile([128, 12, GB], f32, tag="g1T", name="g1T")
            for j in range(12):
                w = min(128, 1500 - j * 128)
                ps = mlpp.tile([128, GB], f32, space="PSUM", tag="g1",
                               name="psg1")
                nc.tensor.matmul(out=ps[:w, :],
                                 lhsT=Wg1T_t[:, j * 128:j * 128 + w],
                                 rhs=gT[:], start=True, stop=True)
                if w < 128:
                    nc.vector.memset(g1T[:, j, :], 0.0)
                nc.scalar.activation(out=g1T[:w, j, :], in_=ps[:w, :],
                                     func=relu_f, bias=bg1_t[:w, j:j + 1])
            g2ps = mlpp.tile([128, GB], f32, space="PSUM", tag="g2",
                             name="g2ps")
            for j in range(12):
                nc.tensor.matmul(out=g2ps[:],
                                 lhsT=Wg2_t[:, j * 128:(j + 1) * 128],
                                 rhs=g1T[:, j, :], start=(j == 0),
                                 stop=(j == 11), skip_group_check=True)
            g2sb = mlps.tile([128, GB], f32, tag="g2sb", name="g2sb")
            nc.vector.tensor_scalar_add(out=g2sb[:], in0=g2ps[:],
                                        scalar1=bg2_t[:])
            ops = mlpp.tile([1, GB], f32, space="PSUM", tag="o", name="ops")
            nc.tensor.matmul(out=ops[:], lhsT=WoT_t[:], rhs=g2sb[:],
                             start=True, stop=True)
            osb = mlps.tile([1, GB], f32, tag="osb", name="osb")
            nc.vector.tensor_scalar_add(out=osb[:], in0=ops[:],
                                        scalar1=float(bo_const))
            nc.sync.dma_start(out=d['out8'][:, :], in_=osb[:])


def _build_program(S_bp, bo_const):
    import concourse.tile as tile
    from concourse import bacc, mybir

    f32 = mybir.dt.float32
    bf16 = mybir.dt.bfloat16
    CH = int(sum(S_bp))
    nc = bacc.Bacc("TRN2", target_bir_lowering=False, debug=False,
                   num_devices=NCORE)

    d = {}

    def dram_in(name, shape, dt=f32):
        d[name] = nc.dram_tensor(name, list(shape), dt, kind="ExternalInput")

    dram_in("xh", (F1, XW), bf16)
    dram_in("xn", (128, JT * F1), bf16)
    dram_in("KQ", (F1, IPAD), bf16)
    dram_in("Wva", (F1, F1), bf16)
    dram_in("Vl", (128, DBLK * F), f32)
    dram_in("ident", (128, 128), bf16)
    dram_in("WllT", (F, F), bf16)
    dram_in("WlrT", (F, F), bf16)
    dram_in("bll", (F, 1))
    dram_in("Wg1T", (F, 1500))
    dram_in("bg1", (128, 12))
    dram_in("Wg2Tr", (128, 12 * 128))
    dram_in("bg2", (128, 1))
    dram_in("WoT", (128, 1))
    dram_in("recipT", (F, IPAD))
    dram_in("P", (128, CH * 128), bf16)
    d['gidx'] = nc.dram_tensor("gidx", [128, CH * 8], mybir.dt.int16,
                               kind="ExternalInput")
    d['out8'] = nc.dram_tensor("out8", [1, GB], f32, kind="ExternalOutput")
    d['h_loc'] = nc.dram_tensor("h_loc", [ROWS, HPAD], bf16)
    for p in range(NP):
        d['h_full%d' % p] = nc.dram_tensor(
            "h_full%d" % p, [NCORE * PLEN[p], HPAD], bf16,
            addr_space="Shared")

    with tile.TileContext(nc) as tc:
        _emit_body(nc, tc, d, S_bp, bo_const)

    nc.compile()
    return nc


# --------------------------------------------------------------------------
# entry point
# --------------------------------------------------------------------------

_CACHE = {}


def _make_in_maps(inputs):
    x = np.asarray(inputs['x'], np.float32)
    edge_index = np.asarray(inputs['edge_index'])
    w = _prep_weights(inputs)
    xh, xn, KQ = _prep_x(x, w['M'], w['Wva'])
    Vl = _prep_vl(x, inputs)
    gidx, Ps, recipT, S_bp = _prep_edges(edge_index)
    ident = np.eye(128, dtype=BF16)
    common = dict(
        xh=xh, xn=xn, Wva=w['Wva'], ident=ident,
        WllT=w['WllT'].astype(BF16), WlrT=w['WlrT'].astype(BF16),
        bll=w['bll'], Wg1T=w['Wg1T'], bg1=w['bg1'], Wg2Tr=w['Wg2Tr'],
        bg2=w['bg2'], WoT=w['WoT'])
    in_maps = []
    for c in range(NCORE):
        m = dict(common)
        m['KQ'] = KQ[c]
        m['Vl'] = Vl[c]
        m['gidx'] = gidx[c]
        m['P'] = Ps[c]
        m['recipT'] = recipT[c]
        in_maps.append(m)
    return in_maps, S_bp, w['bo']


def kernel(**inputs):
    from concourse.bass_utils import run_bass_kernel_spmd

    in_maps, S_bp, bo = _make_in_maps(inputs)
    key = ('prog', S_bp, bo)
    if key not in _CACHE:
        _CACHE[key] = _build_program(S_bp, bo)
    nc = _CACHE[key]

    res = run_bass_kernel_spmd(nc, in_maps, list(range(NCORE)))
    global LAST_RESULT
    LAST_RESULT = res
    out = np.zeros((B, 1), np.float32)
    for c in range(NCORE):
        out[c * GB:(c + 1) * GB, 0] = res.results[c]['out8'].reshape(-1)
    return out


LAST_RESULT = None
